# revision 19
# baseline (speedup 1.0000x reference)
"""GATv2 (2 layers, H=4, C=128, head-mean) on 8 TRN2 cores, dst-partitioned.

v4 design (per layer, one SPMD launch of a shared single-layer NEFF):
  dense: xl'' = x @ (Wl .* |att|) for ALL nodes -> fp16 to DRAM (the value
         path uses full fp16; the logit path uses each fp16's HIGH BYTE,
         which is exactly the e5m2-truncated value, via a stride-2 bitcast
         AP - no separate fp8 conversion pass). xr'' per local block ->
         fp16 slot 0 of the gather tile.
  per 128-edge chunk (edges sorted by dst, 10 blocks x 128 dst/core):
    - SWDGE row-gather of fp16 xl''[src] into slot k+1 of g; trailing
      pad-edge indices are negative so the DGE skips their descriptors
    - front: per head one fp8e5 DoubleRow matmul on the high-byte view:
      uT[c,e] = sum_d xr_e5[d,c]*oht[d,e] + g_e5[e,c]
    - prelu on ACT, batched per chunk-pair ([128,1024] per op); the
      e5m2 truncation bias is compensated by scaling sgn by 1.09
    - logits: per head a [128,1] matmul  z[e] = sum_c s'T[c,e]*sgn[c]
    - exp batched over 8 chunks (one ACT op per [128,32] group); ezb =
      min(ez, 3e4) fp16 guards fp16/inf for skipped pad edges
    - eg[e,hc] = g * ez via ONE tensor_tensor with a broadcast AP per
      chunk (DVE/POOL round-robin) -> bf16
    - agg += oh^T @ eg, den += oh^T @ ezb (fp8e5 one-hots)
  All stages run in a flattened cross-block per-chunk software pipeline.
  tail per block: alpha = agg*rden, .*(0.25/|att|), head-sum, +residual,
  then mean-center and relu ON DEVICE; the 1/sqrt(var+eps) row scale is
  applied on the HOST (relu commutes with the positive scale), so the ACT
  engine only ever runs {Prelu, Exp, Relu} -> zero act-table swaps.
Host: edge sorting, fp8e5 one-hot (oht|id) and oh arrays, wrapped gather
idxs, per-row LayerNorm scale between layers.
"""

from contextlib import ExitStack

import numpy as np
import ml_dtypes

import concourse.bacc as bacc
import concourse.tile as tile
from concourse import mybir
from concourse.bass_utils import run_bass_kernel_spmd

BF16 = ml_dtypes.bfloat16
FP8E5 = ml_dtypes.float8_e5m2
F16 = np.float16

N_NODES = 10000
D = 128
H = 4
C = 128
HC = H * C
NEG_SLOPE = 0.2
LN_EPS = 1e-5
L = 2
SGN_COMP = 1.09     # compensates the e5m2 truncation shrink of logits
EZ_CLAMP = 30000.0  # keeps exp() of stale pad-edge logits finite in fp16

N_CORES = 8
NODES_PER_CORE = 1280
BLOCKS = 10
BLK = 128
N_PAD = N_CORES * NODES_PER_CORE    # 10240
N_ROWS = 10112                      # 79*128
N_TILES = N_ROWS // 128

_NC_CACHE = {}
LAST_RESULTS = []   # BassKernelResults per launch (for test harness)

# engine round-robin patterns (tuned against the cost model)
EG_PAT = ["POOL", "DVE", "POOL"]
SABS_PAT = ["ACT", "ACT", "ACT", "ACT", "ACT", "ACT", "DVE"]
ROW = 640   # gathered row length (f16): 512 xl'' + 4 q + pad (1280B, 256-aligned)
GROUP = 8             # chunks per exp batch
OFF_EG = GROUP + 2    # eg stage offset (must trail the group exp)
OFF_AGG = OFF_EG + 2


def _prep_edges(edge_index):
    src = np.concatenate([np.asarray(edge_index[0], np.int64),
                          np.arange(N_NODES, dtype=np.int64)])
    dst = np.concatenate([np.asarray(edge_index[1], np.int64),
                          np.arange(N_NODES, dtype=np.int64)])
    pad_nodes = np.arange(N_NODES, N_PAD, dtype=np.int64)
    src = np.concatenate([src, np.zeros_like(pad_nodes)])
    dst = np.concatenate([dst, pad_nodes])

    order = np.argsort(dst, kind="stable")
    src = src[order]
    dst = dst[order]

    blk_of_edge = dst // BLK
    n_blocks_total = N_PAD // BLK
    counts = np.bincount(blk_of_edge, minlength=n_blocks_total)
    K = int(np.max((counts + BLK - 1) // BLK))
    K += K % 2  # even, so we can process chunk pairs

    cap = K * BLK
    src_arr = np.zeros((n_blocks_total, cap), np.int32)
    dpos_arr = np.full((n_blocks_total, cap), -1, np.int32)
    block_starts = np.zeros(n_blocks_total + 1, np.int64)
    np.cumsum(counts, out=block_starts[1:])
    slot = np.arange(len(dst)) - block_starts[blk_of_edge]
    src_arr[blk_of_edge, slot] = src.astype(np.int32)
    dpos_arr[blk_of_edge, slot] = (dst - blk_of_edge * BLK).astype(np.int32)

    return (K, src_arr.reshape(N_CORES, BLOCKS, cap),
            dpos_arr.reshape(N_CORES, BLOCKS, cap))


def _build_ship_arrays(K, src_arr, dpos_arr):
    cap = K * BLK
    # wrapped gather indices: idx i lives at [i % 16, i // 16]; the 16-row
    # pattern is tiled 8x along partitions (one copy per SWDGE Q7 core).
    # pad slots are -1: the DGE skips trailing negative indices.
    s = src_arr.reshape(N_CORES, BLOCKS, cap // 16, 16)
    s = np.swapaxes(s, 2, 3)                                  # [c,b,16,cap/16]
    sidx = np.tile(s, (1, 1, 8, 1)).astype(np.int16)          # [c,b,128,cap/16]

    # fp8e5 one-hots:
    # ohtid [c,b, d(128), (K+2)*128]: slot 0 = identity, slot 1+k =
    #   oht chunk k (col (1+k)*128+e -> 1 iff dst(chunk k, e) == d),
    #   slot K+1 = identity.  Identities at both ends let the DoubleRow
    #   matmul pair (g, xr@middle) with (id, oht) using positive AP steps.
    # ohflat [c,b, e(128), cap]: col k*128+d -> oh[e, k, d]
    ohtid = np.zeros((N_CORES, BLOCKS, BLK, (K + 2) * BLK), FP8E5)
    ohflat = np.zeros((N_CORES, BLOCKS, BLK, cap), FP8E5)
    cc, bb, ss = np.nonzero(dpos_arr >= 0)
    kk = (ss // BLK).astype(np.int64)
    ee = (ss % BLK).astype(np.int64)
    dd = dpos_arr[cc, bb, ss].astype(np.int64)
    ohtid[cc, bb, dd, (kk + 1) * BLK + ee] = 1
    ohflat[cc, bb, ee, kk * BLK + dd] = 1
    i = np.arange(BLK)
    ohtid[:, :, i, i] = 1
    ohtid[:, :, i, (K + 1) * BLK + i] = 1
    return (np.ascontiguousarray(sidx), np.ascontiguousarray(ohtid),
            np.ascontiguousarray(ohflat))


def _bcast(v, rows=128):
    v = np.asarray(v, np.float32)
    return np.ascontiguousarray(np.broadcast_to(v[None, :], (rows, v.shape[0])))


def _build_nc(K, bias_zero, ln_triv):
    nc = bacc.Bacc("TRN2", target_bir_lowering=False, debug=False,
                   num_devices=N_CORES)
    f32, bf16, i16 = mybir.dt.float32, mybir.dt.bfloat16, mybir.dt.int16
    f16 = mybir.dt.float16
    fp8e5 = mybir.dt.float8e5
    AF = mybir.ActivationFunctionType
    ALU = mybir.AluOpType
    PM = mybir.MatmulPerfMode
    X = mybir.AxisListType.X
    cap = K * BLK

    xT = nc.dram_tensor("xT", [128, N_ROWS], bf16, kind="ExternalInput")
    xlocT = nc.dram_tensor("xlocT", [128, NODES_PER_CORE], bf16,
                           kind="ExternalInput")
    xloc = nc.dram_tensor("xloc", [NODES_PER_CORE, 128], f32,
                          kind="ExternalInput")
    WlS = nc.dram_tensor("WlS", [128, HC], bf16, kind="ExternalInput")
    WrS = nc.dram_tensor("WrS", [128, HC], bf16, kind="ExternalInput")
    blB = nc.dram_tensor("blB", [128, HC], f32, kind="ExternalInput")
    brB = nc.dram_tensor("brB", [128, HC], f32, kind="ExternalInput")
    sgnT = nc.dram_tensor("sgnT", [128, H], bf16, kind="ExternalInput")
    WlQ = nc.dram_tensor("WlQ", [128, H], bf16, kind="ExternalInput")
    WrQ = nc.dram_tensor("WrQ", [128, H], bf16, kind="ExternalInput")
    invatt4B = nc.dram_tensor("invatt4B", [128, HC], f32, kind="ExternalInput")
    biasB = nc.dram_tensor("biasB", [128, 128], f32, kind="ExternalInput")
    lngB = nc.dram_tensor("lngB", [128, 128], f32, kind="ExternalInput")
    lnbB = nc.dram_tensor("lnbB", [128, 128], f32, kind="ExternalInput")
    ohtidd = nc.dram_tensor("ohtidd", [BLOCKS, BLK, (K + 2) * BLK], fp8e5,
                            kind="ExternalInput")
    ohd = nc.dram_tensor("ohd", [BLOCKS, BLK, cap], fp8e5,
                         kind="ExternalInput")
    sidxd = nc.dram_tensor("sidxd", [BLOCKS, 128, cap // 16], i16,
                           kind="ExternalInput")

    xnew = nc.dram_tensor("xnew", [NODES_PER_CORE, 128], f32,
                          kind="ExternalOutput")
    vsd = nc.dram_tensor("vsd", [BLOCKS, 128], f32, kind="ExternalOutput")

    with tile.TileContext(nc) as tc, ExitStack() as ctx:
        consts = ctx.enter_context(tc.tile_pool(name="consts", bufs=1))
        lhsp = ctx.enter_context(tc.tile_pool(name="lhs", bufs=3))
        densep = ctx.enter_context(tc.tile_pool(name="dense", bufs=2))
        g8p = ctx.enter_context(tc.tile_pool(name="g8", bufs=2))
        otp = ctx.enter_context(tc.tile_pool(name="ot", bufs=3))
        ohp = ctx.enter_context(tc.tile_pool(name="ohf", bufs=3))
        sxp = ctx.enter_context(tc.tile_pool(name="sx", bufs=3))
        sp = ctx.enter_context(tc.tile_pool(name="s", bufs=3))
        ezp = ctx.enter_context(tc.tile_pool(name="ez", bufs=3))
        egp = ctx.enter_context(tc.tile_pool(name="eg", bufs=4))
        lnp = ctx.enter_context(tc.tile_pool(name="ln", bufs=2))
        lgp = ctx.enter_context(tc.tile_pool(name="lg", bufs=4))
        outp = ctx.enter_context(tc.tile_pool(name="out", bufs=2))
        dramp = ctx.enter_context(tc.tile_pool(name="dram", bufs=1,
                                               space="DRAM"))
        pup = ctx.enter_context(tc.tile_pool(name="pu", bufs=2, space="PSUM"))
        pzp = ctx.enter_context(tc.tile_pool(name="pz", bufs=1, space="PSUM"))
        pdenp = ctx.enter_context(tc.tile_pool(name="pden", bufs=1,
                                               space="PSUM"))
        paggp = ctx.enter_context(tc.tile_pool(name="pagg", bufs=2,
                                               space="PSUM"))

        def load_const(src_ap, shape, dtype, name):
            t = consts.tile(shape, dtype, tag=name)
            nc.sync.dma_start(t[:], src_ap)
            return t

        wl_sb = load_const(WlS[:], [128, HC], bf16, "wl")
        wr_sb = load_const(WrS[:], [128, HC], bf16, "wr")
        sgn_sb = load_const(sgnT[:], [128, H], bf16, "sgn")
        wlq_sb = load_const(WlQ[:], [128, H], bf16, "wlq")
        wrq_sb = load_const(WrQ[:], [128, H], bf16, "wrq")
        invatt_sb = load_const(invatt4B[:], [128, HC], f32, "invatt")
        if not bias_zero:
            blB_sb = load_const(blB[:], [128, HC], f32, "blB")
            brB_sb = load_const(brB[:], [128, HC], f32, "brB")
            biasB_sb = load_const(biasB[:], [128, 128], f32, "biasB")
        if not ln_triv:
            lngB_sb = load_const(lngB[:], [128, 128], f32, "lngB")
            lnbB_sb = load_const(lnbB[:], [128, 128], f32, "lnbB")

        xl_dram = dramp.tile([N_ROWS, ROW], f16)

        alphaP = consts.tile([128, 1], f32, tag="alphaP")
        nc.vector.memset(alphaP[:], NEG_SLOPE)
        epsP = consts.tile([128, 1], f32, tag="epsP")
        nc.vector.memset(epsP[:], LN_EPS)

        blk_loads = {}

        def prefetch_loads(b):
            six = sxp.tile([128, cap // 16], i16, tag="sidx")
            nc.sync.dma_start(six[:], sidxd[b])
            ot = otp.tile([128, K + 2, BLK], fp8e5, tag="ot")
            nc.sync.dma_start(
                ot[:], ohtidd[b].rearrange("p (k e) -> p k e", e=BLK))
            ohb = ohp.tile([128, cap], fp8e5, tag="oh")
            nc.sync.dma_start(ohb[:], ohd[b])
            blk_loads[b] = (six, ot, ohb)

        st = {"g8": {}, "agg": {}, "den": {}, "s": {}, "u": {},
              "ezf": {}, "ezb": {}, "eg": {}, "zp": {}, "p06": None}

        POS = (K // 2) // GROUP * GROUP   # middle slot, group-aligned

        def prefetch_gather(b):
            six, ot, ohb = blk_loads[b]
            g8 = g8p.tile([128, K + 1, ROW], f16, tag="g8")
            st["g8"][b] = g8
            # chunk k -> tile slot k (k < POS) or k+1 (k >= POS)
            # splits sized under the 1024-descriptor SWDGE FIFO carveout
            ranges = []
            for lo, hi in ((0, POS), (POS, K)):
                n_sp = -(-(hi - lo) * BLK // 1008)
                bnds = [lo + (hi - lo) * i // n_sp for i in range(n_sp + 1)]
                ranges += list(zip(bnds[:-1], bnds[1:]))
            for k0, k1 in ranges:
                s0 = k0 if k1 <= POS else k0 + 1
                n_idx = (k1 - k0) * BLK
                nc.gpsimd.dma_gather(
                    out_ap=g8[:, s0:s0 + (k1 - k0), :], in_ap=xl_dram[:],
                    idxs_ap=six[:, k0 * BLK // 16:k1 * BLK // 16],
                    num_idxs=n_idx, num_idxs_reg=n_idx, elem_size=ROW,
                    single_packet=False)

        # ---- dense: xl'' for all nodes -> DRAM fp16 ----
        xT_sb = consts.tile([128, N_ROWS], bf16, tag="xT")
        for q in range(4):
            c0 = (N_ROWS // 4 // 128) * 128 * q
            c1 = N_ROWS if q == 3 else (N_ROWS // 4 // 128) * 128 * (q + 1)
            nc.sync.dma_start(xT_sb[:, c0:c1], xT[:, c0:c1])
        prefetch_loads(0)
        prefetch_loads(1)
        GB = 8
        for t0 in range(0, N_TILES, GB):
            n_sub = min(GB, N_TILES - t0)
            xs4 = densep.tile([128, GB, ROW], f16, tag="xs4")
            for j in range(n_sub):
                t_i = t0 + j
                xt_ap = xT_sb[:, t_i * 128:(t_i + 1) * 128]
                ps4 = pup.tile([128, 2, HC], f32, tag="uT")
                ps = ps4[:, 0, :]
                nc.tensor.matmul(ps, xt_ap, wl_sb[:], start=True, stop=True)
                if bias_zero:
                    if t_i % 2 == 0:
                        nc.vector.tensor_scalar(out=xs4[:, j, :HC], in0=ps,
                                                scalar1=1.0, scalar2=None,
                                                op0=ALU.mult)
                    else:
                        nc.scalar.activation(xs4[:, j, :HC], ps, AF.Copy)
                else:
                    nc.vector.tensor_tensor(out=xs4[:, j, :HC], in0=ps,
                                            in1=blB_sb[:], op=ALU.add)
                if bias_zero:
                    # q06 = x @ WlQ (sgn-folded tiny weight), parked in the
                    # unused second bank of the pair tile
                    qp = ps4[:, 1, 0:4]
                    nc.tensor.matmul(qp, xt_ap, wlq_sb[:], start=True,
                                     stop=True)
                    nc.vector.tensor_scalar(out=xs4[:, j, HC:HC + 4],
                                            in0=qp, scalar1=1.0,
                                            scalar2=None, op0=ALU.mult)
            nc.scalar.dma_start(
                xl_dram[t0 * 128:(t0 + n_sub) * 128, :].rearrange(
                    "(t p) c -> p t c", p=128),
                xs4[:, :n_sub, :])

        # ---- edge phase: flattened per-chunk software pipeline ----
        prefetch_gather(0)
        P = K // 2

        def blk_state(b):
            if b not in st["agg"]:
                g8 = st["g8"][b]
                # xr'' for this block -> fp16 slot 0
                lhs = lhsp.tile([128, 128], bf16, tag="lhs")
                nc.sync.dma_start(lhs[:], xlocT[:, b * 128:(b + 1) * 128])
                psr4 = pup.tile([128, 2, HC], f32, tag="uT")
                psr = psr4[:, 0, :]
                nc.tensor.matmul(psr, lhs[:], wr_sb[:], start=True,
                                 stop=True)
                if bias_zero:
                    nc.vector.tensor_scalar(out=g8[:, POS, :HC], in0=psr,
                                            scalar1=1.0, scalar2=None,
                                            op0=ALU.mult)
                    pp_ = psr4[:, 1, 0:4]
                    nc.tensor.matmul(pp_, lhs[:], wrq_sb[:], start=True,
                                     stop=True)
                    p06 = lgp.tile([128, 4], bf16, tag="p06")
                    nc.vector.tensor_scalar(out=p06[:], in0=pp_,
                                            scalar1=1.0, scalar2=None,
                                            op0=ALU.mult)
                    st["p06"] = p06
                else:
                    nc.vector.tensor_tensor(out=g8[:, POS, :HC], in0=psr,
                                            in1=brB_sb[:], op=ALU.add)
                agg_t = paggp.tile([128, HC], f32, tag="agg")
                den_t = pdenp.tile([128, 4], f32, tag="den")
                st["agg"][b] = agg_t
                st["den"][b] = den_t

        def front(b, k):
            if k == 0:
                blk_state(b)
            g8 = st["g8"][b]
            ge5 = g8[:].bitcast(fp8e5)     # [128, K+1, 2*ROW]
            ot = blk_loads[b][1]
            m, half = divmod(k, 2)
            if half == 0:
                uT_t = pup.tile([128, 2, HC], f32, tag="uT")
                st["u"][(b, m)] = uT_t
            uT = st["u"][(b, m)]
            gslot = k if k < POS else k + 1
            for h in range(H):
                lo = 2 * h * 128 + 1
                out_ap = uT[:, half, h * 128:(h + 1) * 128]
                if k < POS:
                    # lhs halves (g, xr) pair with rhs halves (id, oht)
                    lhs_ap = ge5[:, gslot:POS + 1:POS - gslot, lo:lo + 255:2]
                    rhs_ap = ot[:, 0:k + 2:k + 1, :]
                else:
                    # lhs halves (xr, g) pair with rhs halves (oht, id)
                    lhs_ap = ge5[:, POS:gslot + 1:gslot - POS, lo:lo + 255:2]
                    rhs_ap = ot[:, k + 1:K + 2:K - k, :]
                nc.tensor.matmul(out_ap, lhs_ap, rhs_ap, start=True,
                                 stop=True, perf_mode=PM.DoubleRow)
            if k == 16 and b + 2 < BLOCKS:
                prefetch_loads(b + 2)
            if k == 20 and b + 1 < BLOCKS:
                prefetch_gather(b + 1)

        def sabs(b, m):
            # |u|: with z = 0.6*(q[src]+p[dst]) + 0.4*sgn.|u| the prelu
            # becomes an abs, which any engine can apply in one PSUM read
            uT = st["u"].pop((b, m))
            s_ = sp.tile([128, 2, HC], bf16, tag="s")
            if not bias_zero:
                nc.scalar.activation(s_[:], uT[:], AF.Prelu, alpha=alphaP[:])
            elif SABS_PAT[m % len(SABS_PAT)] == "ACT":
                nc.scalar.activation(s_[:], uT[:], AF.Abs)
            else:
                # |u| off-ACT: copy u to SBUF, then max(-u, u) in one
                # PSUM-read scalar_tensor_tensor (abs is ACT-only in ISA)
                cp_ = sp.tile([128, 2, HC], bf16, tag="cp")
                nc.vector.tensor_scalar(out=cp_[:], in0=uT[:], scalar1=1.0,
                                        scalar2=None, op0=ALU.mult)
                nc.vector.scalar_tensor_tensor(
                    out=s_[:], in0=uT[:], scalar=-1.0, in1=cp_[:],
                    op0=ALU.mult, op1=ALU.max)
            st["s"][(b, m)] = s_

        def zmm(b, k):
            g, slot = divmod(k, GROUP)
            if slot == 0:
                zP_t = pzp.tile([128, 4 * GROUP], f32, tag="zP")
                st["zp"][b] = zP_t
            zP = st["zp"][b]
            m, half = divmod(k, 2)
            s_ = st["s"][(b, m)]
            c4 = zP[:, slot * 4:slot * 4 + 4]
            if bias_zero:
                # 0.6*p[dst] via the dst one-hot; 0.4*sgn.|u| per head
                nc.tensor.matmul(c4, blk_loads[b][1][:, k + 1, :],
                                 st["p06"][:], start=True, stop=False,
                                 skip_group_check=True)
            for h in range(H):
                nc.tensor.matmul(zP[:, slot * 4 + h:slot * 4 + h + 1],
                                 s_[:, half, h * 128:(h + 1) * 128],
                                 sgn_sb[:, h:h + 1],
                                 start=not bias_zero, stop=True,
                                 skip_group_check=True)
            if half == 1:
                st["s"].pop((b, m))
            if slot == GROUP - 1 or k == K - 1:
                n4 = (slot + 1) * 4
                g8 = st["g8"][b]
                k0 = g * GROUP
                s0 = k0 if k0 < POS else k0 + 1
                if bias_zero:
                    # z = zP + 0.6*q[src] (q rides the gathered rows)
                    zf = ezp.tile([128, 4 * GROUP], f32, tag="zf")
                    nc.vector.tensor_tensor(
                        out=zf[:, :n4].rearrange("p (s f) -> p s f", f=4),
                        in0=zP[:, :n4].rearrange("p (s f) -> p s f", f=4),
                        in1=g8[:, s0:s0 + n4 // 4, HC:HC + 4],
                        op=ALU.add)
                    zsrc = zf
                else:
                    zsrc = zP
                ezf = ezp.tile([128, 4 * GROUP], f32, tag="ezf")
                nc.scalar.activation(ezf[:, :n4], zsrc[:, :n4], AF.Exp)
                ezb = ezp.tile([128, 4 * GROUP], f16, tag="ezb")
                nc.gpsimd.tensor_scalar(out=ezb[:, :n4], in0=ezf[:, :n4],
                                        scalar1=EZ_CLAMP, scalar2=None,
                                        op0=ALU.min)
                st["ezf"][(b, g)] = ezf
                st["ezb"][(b, g)] = ezb

        def eg_stage(b, k):
            g8 = st["g8"][b]
            m, half = divmod(k, 2)
            if half == 0:
                egt_t = egp.tile([128, 2, HC], bf16, tag="eg")
                st["eg"][(b, m)] = egt_t
            egt = st["eg"][(b, m)]
            grp, slot = divmod(k, GROUP)
            ezb = st["ezb"][(b, grp)]
            gslot = k if k < POS else k + 1
            in0 = g8[:, gslot, :HC].rearrange("p (h c) -> p h c", h=H)
            in1 = ezb[:, slot * 4:slot * 4 + 4].unsqueeze(-1).broadcast_to(
                [128, H, 128])
            out = egt[:, half, :].rearrange("p (h c) -> p h c", h=H)
            eng = EG_PAT[k % len(EG_PAT)]
            if eng == "DVE":
                nc.vector.tensor_tensor(out=out, in0=in0, in1=in1,
                                        op=ALU.mult)
            else:
                nc.gpsimd.tensor_tensor(out=out, in0=in0, in1=in1,
                                        op=ALU.mult)

        def aggden(b, m):
            agg = st["agg"][b]
            den = st["den"][b]
            ohb = blk_loads[b][2]
            egt = st["eg"].pop((b, m))
            for half in (0, 1):
                k = 2 * m + half
                grp, slot = divmod(k, GROUP)
                ezb = st["ezb"][(b, grp)]
                nc.tensor.matmul(agg[:], ohb[:, k * BLK:(k + 1) * BLK],
                                 egt[:, half, :],
                                 start=(k == 0), stop=(k == K - 1))
                nc.tensor.matmul(den[:], ohb[:, k * BLK:(k + 1) * BLK],
                                 ezb[:, slot * 4:slot * 4 + 4],
                                 start=(k == 0), stop=(k == K - 1))
            if m == P - 1:
                tail(b)

        def tail(b):
            agg = st["agg"].pop(b)
            den = st["den"].pop(b)
            st["g8"].pop(b, None)
            st["zp"].pop(b, None)
            blk_loads.pop(b, None)
            for key in [x for x in st["ezf"] if x[0] == b]:
                st["ezf"].pop(key)
            for key in [x for x in st["ezb"] if x[0] == b]:
                st["ezb"].pop(key)
            rden = lgp.tile([128, 4], f32, tag="rden")
            nc.vector.reciprocal(rden[:], den[:])
            # tq = agg * rden (head-broadcast) on DVE
            tq = lnp.tile([128, HC], f32, tag="tq")
            nc.vector.tensor_tensor(
                out=tq[:].rearrange("p (h c) -> p h c", h=H),
                in0=agg[:].rearrange("p (h c) -> p h c", h=H),
                in1=rden[:].unsqueeze(-1).broadcast_to([128, H, 128]),
                op=ALU.mult)
            tq2 = lnp.tile([128, HC], f32, tag="tq2")
            nc.gpsimd.tensor_tensor(out=tq2[:], in0=tq[:], in1=invatt_sb[:],
                                    op=ALU.mult)
            hm = outp.tile([128, 128], f32, tag="hm")
            nc.vector.tensor_reduce(
                out=hm[:], in_=tq2[:].rearrange("p (h c) -> p c h", h=H),
                axis=X, op=ALU.add)
            xt = outp.tile([128, 128], f32, tag="xres")
            nc.sync.dma_start(xt[:], xloc[b * 128:(b + 1) * 128, :])
            if bias_zero:
                r2 = outp.tile([128, 128], f32, tag="r2")
                nc.gpsimd.tensor_tensor(out=r2[:], in0=hm[:], in1=xt[:],
                                        op=ALU.add)
            else:
                r1 = outp.tile([128, 128], f32, tag="r1")
                nc.vector.tensor_tensor(out=r1[:], in0=hm[:],
                                        in1=biasB_sb[:], op=ALU.add)
                r2 = outp.tile([128, 128], f32, tag="r2")
                nc.gpsimd.tensor_tensor(out=r2[:], in0=r1[:], in1=xt[:],
                                        op=ALU.add)
            mu = lgp.tile([128, 1], f32, tag="mu")
            nc.vector.tensor_reduce(out=mu[:], in_=r2[:], axis=X, op=ALU.add)
            mun = lgp.tile([128, 1], f32, tag="mun")
            nc.vector.tensor_scalar_mul(mun[:], mu[:], 1.0 / 128)
            xc = outp.tile([128, 128], f32, tag="xc")
            nc.vector.tensor_scalar(out=xc[:], in0=r2[:], scalar1=mun[:],
                                    scalar2=None, op0=ALU.subtract)
            junk = outp.tile([128, 128], f32, tag="junk")
            vs = lgp.tile([128, 1], f32, tag="vs")
            nc.vector.scalar_tensor_tensor(
                out=junk[:], in0=r2[:], scalar=mun[:], in1=xc[:],
                op0=ALU.subtract, op1=ALU.mult, accum_out=vs[:])
            nc.sync.dma_start(vsd[b], vs[:, 0])
            xout = outp.tile([128, 128], f32, tag="xout")
            if ln_triv:
                # relu only; the 1/sqrt(var+eps) row scale is applied on
                # the host (relu commutes with a positive per-row scale)
                nc.scalar.activation(xout[:], xc[:], AF.Relu)
            else:
                lt = lgp.tile([128, 1], f32, tag="lt")
                nc.scalar.activation(lt[:], vs[:], AF.Ln, bias=epsP[:],
                                     scale=1.0 / 128)
                rstd = lgp.tile([128, 1], f32, tag="rstd")
                nc.scalar.activation(rstd[:], lt[:], AF.Exp, scale=-0.5)
                xn = outp.tile([128, 128], f32, tag="xn")
                nc.vector.tensor_scalar(out=xn[:], in0=xc[:],
                                        scalar1=rstd[:],
                                        scalar2=None, op0=ALU.mult)
                xg = outp.tile([128, 128], f32, tag="xg")
                nc.vector.tensor_tensor(out=xg[:], in0=xn[:], in1=lngB_sb[:],
                                        op=ALU.mult)
                xgb = outp.tile([128, 128], f32, tag="xgb")
                nc.vector.tensor_tensor(out=xgb[:], in0=xg[:],
                                        in1=lnbB_sb[:], op=ALU.add)
                nc.scalar.activation(xout[:], xgb[:], AF.Relu)
            nc.sync.dma_start(xnew[b * 128:(b + 1) * 128, :], xout[:])

        chunks = [(b, k) for b in range(BLOCKS) for k in range(K)]
        NCH = len(chunks)
        for i in range(NCH + OFF_AGG + 1):
            if i < NCH:
                front(*chunks[i])
            if 0 <= i - 1 < NCH and chunks[i - 1][1] % 2 == 1:
                b, k = chunks[i - 1]
                sabs(b, k // 2)
            if 0 <= i - 2 < NCH:
                zmm(*chunks[i - 2])
            if 0 <= i - OFF_EG < NCH:
                eg_stage(*chunks[i - OFF_EG])
            if 0 <= i - OFF_AGG < NCH and chunks[i - OFF_AGG][1] % 2 == 0:
                b, k = chunks[i - OFF_AGG]
                aggden(b, k // 2)

    nc.compile()
    return nc


def kernel(x, edge_index, Wl, bl, Wr, br, att, bias, ln_g, ln_b):
    x = np.asarray(x, np.float32)
    edge_index = np.asarray(edge_index)
    Wl = np.asarray(Wl, np.float32); bl = np.asarray(bl, np.float32)
    Wr = np.asarray(Wr, np.float32); br = np.asarray(br, np.float32)
    att = np.asarray(att, np.float32); bias = np.asarray(bias, np.float32)
    ln_g = np.asarray(ln_g, np.float32); ln_b = np.asarray(ln_b, np.float32)

    K, src_arr, dpos_arr = _prep_edges(edge_index)
    sidx, ohtid, ohflat = _build_ship_arrays(K, src_arr, dpos_arr)

    bias_zero = not (np.any(bias) or np.any(bl) or np.any(br))
    ln_triv = (np.all(ln_g == 1.0) and not np.any(ln_b))
    key = (K, bias_zero, ln_triv)
    if key not in _NC_CACHE:
        _NC_CACHE[key] = _build_nc(K, bias_zero, ln_triv)
    nc = _NC_CACHE[key]

    aatt = np.maximum(np.abs(att), 1e-30)
    sgn = np.sign(att).astype(np.float32)
    sgn[sgn == 0] = 1.0

    LAST_RESULTS.clear()
    cur = x
    for l in range(L):
        a_flat = aatt[l].reshape(HC)
        WlSf = Wl[l] * a_flat[None, :]
        WrSf = Wr[l] * a_flat[None, :]
        WlS = WlSf.astype(BF16)
        WrS = WrSf.astype(BF16)
        sgnT = np.ascontiguousarray(
            (sgn[l] * 0.4 * SGN_COMP).T).astype(BF16)   # [C, H]
        sgn3 = sgn[l].reshape(H, C)
        WlQ = np.einsum('dhc,hc->dh', WlSf.reshape(D, H, C),
                        sgn3) * (0.6 * SGN_COMP)
        WrQ = np.einsum('dhc,hc->dh', WrSf.reshape(D, H, C),
                        sgn3) * (0.6 * SGN_COMP)

        xpad = np.zeros((N_ROWS, 128), np.float32)
        xpad[:N_NODES] = cur
        xT = np.ascontiguousarray(xpad.T.astype(BF16))
        xloc_full = np.zeros((N_PAD, 128), np.float32)
        xloc_full[:N_NODES] = cur

        common = {
            "xT": xT, "WlS": WlS, "WrS": WrS,
            "blB": _bcast(bl[l] * a_flat), "brB": _bcast(br[l] * a_flat),
            "sgnT": sgnT, "WlQ": WlQ.astype(BF16), "WrQ": WrQ.astype(BF16),
            "invatt4B": _bcast(0.25 / a_flat),
            "biasB": _bcast(bias[l]), "lngB": _bcast(ln_g[l]),
            "lnbB": _bcast(ln_b[l]),
        }
        in_maps = []
        for c in range(N_CORES):
            xl_c = np.ascontiguousarray(
                xloc_full[c * NODES_PER_CORE:(c + 1) * NODES_PER_CORE])
            in_maps.append({
                **common,
                "xloc": xl_c,
                "xlocT": np.ascontiguousarray(xl_c.T.astype(BF16)),
                "ohtidd": ohtid[c], "ohd": ohflat[c], "sidxd": sidx[c],
            })

        res = run_bass_kernel_spmd(nc, in_maps, core_ids=list(range(N_CORES)))
        LAST_RESULTS.append(res)
        nxt = np.concatenate([res.results[c]["xnew"] for c in range(N_CORES)],
                             axis=0)
        if ln_triv:
            vsall = np.concatenate(
                [np.asarray(res.results[c]["vsd"]).reshape(-1)
                 for c in range(N_CORES)])
            rstd = 1.0 / np.sqrt(vsall / 128.0 + LN_EPS)
            nxt = nxt * rstd[:, None]
        cur = np.ascontiguousarray(nxt[:N_NODES]).astype(np.float32)

    return cur.astype(np.float32)


# revision 20
# speedup vs baseline: 1.1880x; 1.1880x over previous
"""GATv2 (2 layers, H=4, C=128, head-mean) on 8 TRN2 cores, dst-partitioned.

v4 design (per layer, one SPMD launch of a shared single-layer NEFF):
  dense: xl'' = x @ (Wl .* |att|) for ALL nodes -> fp16 to DRAM (the value
         path uses full fp16; the logit path uses each fp16's HIGH BYTE,
         which is exactly the e5m2-truncated value, via a stride-2 bitcast
         AP - no separate fp8 conversion pass). xr'' per local block ->
         fp16 slot 0 of the gather tile.
  per 128-edge chunk (edges sorted by dst, 10 blocks x 128 dst/core):
    - SWDGE row-gather of fp16 xl''[src] into slot k+1 of g; trailing
      pad-edge indices are negative so the DGE skips their descriptors
    - front: per head one fp8e5 DoubleRow matmul on the high-byte view:
      uT[c,e] = sum_d xr_e5[d,c]*oht[d,e] + g_e5[e,c]
    - prelu on ACT, batched per chunk-pair ([128,1024] per op); the
      e5m2 truncation bias is compensated by scaling sgn by 1.09
    - logits: per head a [128,1] matmul  z[e] = sum_c s'T[c,e]*sgn[c]
    - exp batched over 8 chunks (one ACT op per [128,32] group); ezb =
      min(ez, 3e4) fp16 guards fp16/inf for skipped pad edges
    - eg[e,hc] = g * ez via ONE tensor_tensor with a broadcast AP per
      chunk (DVE/POOL round-robin) -> bf16
    - agg += oh^T @ eg, den += oh^T @ ezb (fp8e5 one-hots)
  All stages run in a flattened cross-block per-chunk software pipeline.
  tail per block: alpha = agg*rden, .*(0.25/|att|), head-sum, +residual,
  then mean-center and relu ON DEVICE; the 1/sqrt(var+eps) row scale is
  applied on the HOST (relu commutes with the positive scale), so the ACT
  engine only ever runs {Prelu, Exp, Relu} -> zero act-table swaps.
Host: edge sorting, fp8e5 one-hot (oht|id) and oh arrays, wrapped gather
idxs, per-row LayerNorm scale between layers.
"""

from contextlib import ExitStack

import numpy as np
import ml_dtypes

import concourse.bacc as bacc
import concourse.tile as tile
from concourse import mybir
from concourse.bass_utils import run_bass_kernel_spmd

BF16 = ml_dtypes.bfloat16
FP8E5 = ml_dtypes.float8_e5m2
F16 = np.float16

N_NODES = 10000
D = 128
H = 4
C = 128
HC = H * C
NEG_SLOPE = 0.2
LN_EPS = 1e-5
L = 2
SGN_COMP = 1.09     # compensates the e5m2 truncation shrink of logits
EZ_CLAMP = 30000.0  # keeps exp() of stale pad-edge logits finite in fp16

N_CORES = 8
NODES_PER_CORE = 1280
BLOCKS = 10
BLK = 128
N_PAD = N_CORES * NODES_PER_CORE    # 10240
N_ROWS = 10112                      # 79*128
N_TILES = N_ROWS // 128

_NC_CACHE = {}
LAST_RESULTS = []   # BassKernelResults per launch (for test harness)

# engine round-robin patterns (tuned against the cost model)
EG_PAT = ["POOL", "DVE", "POOL"]
SABS_PAT = ["ACT", "ACT", "ACT", "ACT", "ACT", "ACT", "DVE"]
GROUP = 8             # chunks per exp batch
OFF_EG = GROUP + 2    # eg stage offset (must trail the group exp)
OFF_AGG = OFF_EG + 2


def _prep_edges(edge_index):
    src = np.concatenate([np.asarray(edge_index[0], np.int64),
                          np.arange(N_NODES, dtype=np.int64)])
    dst = np.concatenate([np.asarray(edge_index[1], np.int64),
                          np.arange(N_NODES, dtype=np.int64)])
    pad_nodes = np.arange(N_NODES, N_PAD, dtype=np.int64)
    src = np.concatenate([src, np.zeros_like(pad_nodes)])
    dst = np.concatenate([dst, pad_nodes])

    order = np.argsort(dst, kind="stable")
    src = src[order]
    dst = dst[order]

    blk_of_edge = dst // BLK
    n_blocks_total = N_PAD // BLK
    counts = np.bincount(blk_of_edge, minlength=n_blocks_total)
    K = int(np.max((counts + BLK - 1) // BLK))
    K += K % 2  # even, so we can process chunk pairs

    cap = K * BLK
    src_arr = np.zeros((n_blocks_total, cap), np.int32)
    dpos_arr = np.full((n_blocks_total, cap), -1, np.int32)
    block_starts = np.zeros(n_blocks_total + 1, np.int64)
    np.cumsum(counts, out=block_starts[1:])
    slot = np.arange(len(dst)) - block_starts[blk_of_edge]
    src_arr[blk_of_edge, slot] = src.astype(np.int32)
    dpos_arr[blk_of_edge, slot] = (dst - blk_of_edge * BLK).astype(np.int32)

    return (K, src_arr.reshape(N_CORES, BLOCKS, cap),
            dpos_arr.reshape(N_CORES, BLOCKS, cap))


def _build_ship_arrays(K, src_arr, dpos_arr):
    cap = K * BLK
    # wrapped gather indices: idx i lives at [i % 16, i // 16]; the 16-row
    # pattern is tiled 8x along partitions (one copy per SWDGE Q7 core).
    # pad slots are -1: the DGE skips trailing negative indices.
    s = src_arr.reshape(N_CORES, BLOCKS, cap // 16, 16)
    s = np.swapaxes(s, 2, 3)                                  # [c,b,16,cap/16]
    sidx = np.tile(s, (1, 1, 8, 1)).astype(np.int16)          # [c,b,128,cap/16]

    # fp8e5 one-hots:
    # ohtid [c,b, d(128), (K+2)*128]: slot 0 = identity, slot 1+k =
    #   oht chunk k (col (1+k)*128+e -> 1 iff dst(chunk k, e) == d),
    #   slot K+1 = identity.  Identities at both ends let the DoubleRow
    #   matmul pair (g, xr@middle) with (id, oht) using positive AP steps.
    # ohflat [c,b, e(128), cap]: col k*128+d -> oh[e, k, d]
    ohtid = np.zeros((N_CORES, BLOCKS, BLK, (K + 2) * BLK), FP8E5)
    ohflat = np.zeros((N_CORES, BLOCKS, BLK, cap), FP8E5)
    cc, bb, ss = np.nonzero(dpos_arr >= 0)
    kk = (ss // BLK).astype(np.int64)
    ee = (ss % BLK).astype(np.int64)
    dd = dpos_arr[cc, bb, ss].astype(np.int64)
    ohtid[cc, bb, dd, (kk + 1) * BLK + ee] = 1
    ohflat[cc, bb, ee, kk * BLK + dd] = 1
    i = np.arange(BLK)
    ohtid[:, :, i, i] = 1
    ohtid[:, :, i, (K + 1) * BLK + i] = 1
    return (np.ascontiguousarray(sidx), np.ascontiguousarray(ohtid),
            np.ascontiguousarray(ohflat))


def _bcast(v, rows=128):
    v = np.asarray(v, np.float32)
    return np.ascontiguousarray(np.broadcast_to(v[None, :], (rows, v.shape[0])))


def _build_nc(K, bias_zero, ln_triv):
    nc = bacc.Bacc("TRN2", target_bir_lowering=False, debug=False,
                   num_devices=N_CORES)
    f32, bf16, i16 = mybir.dt.float32, mybir.dt.bfloat16, mybir.dt.int16
    f16 = mybir.dt.float16
    fp8e5 = mybir.dt.float8e5
    AF = mybir.ActivationFunctionType
    ALU = mybir.AluOpType
    PM = mybir.MatmulPerfMode
    X = mybir.AxisListType.X
    cap = K * BLK

    xT = nc.dram_tensor("xT", [128, N_ROWS], bf16, kind="ExternalInput")
    xlocT = nc.dram_tensor("xlocT", [128, NODES_PER_CORE], bf16,
                           kind="ExternalInput")
    xloc = nc.dram_tensor("xloc", [NODES_PER_CORE, 128], f32,
                          kind="ExternalInput")
    WlS = nc.dram_tensor("WlS", [128, HC], bf16, kind="ExternalInput")
    WrS = nc.dram_tensor("WrS", [128, HC], bf16, kind="ExternalInput")
    blB = nc.dram_tensor("blB", [128, HC], f32, kind="ExternalInput")
    brB = nc.dram_tensor("brB", [128, HC], f32, kind="ExternalInput")
    sgnT = nc.dram_tensor("sgnT", [128, H], bf16, kind="ExternalInput")
    WlQ = nc.dram_tensor("WlQ", [128, H], bf16, kind="ExternalInput")
    WrQ = nc.dram_tensor("WrQ", [128, H], bf16, kind="ExternalInput")
    invatt4B = nc.dram_tensor("invatt4B", [128, HC], f32, kind="ExternalInput")
    biasB = nc.dram_tensor("biasB", [128, 128], f32, kind="ExternalInput")
    lngB = nc.dram_tensor("lngB", [128, 128], f32, kind="ExternalInput")
    lnbB = nc.dram_tensor("lnbB", [128, 128], f32, kind="ExternalInput")
    ohtidd = nc.dram_tensor("ohtidd", [BLOCKS, BLK, (K + 2) * BLK], fp8e5,
                            kind="ExternalInput")
    ohd = nc.dram_tensor("ohd", [BLOCKS, BLK, cap], fp8e5,
                         kind="ExternalInput")
    sidxd = nc.dram_tensor("sidxd", [BLOCKS, 128, cap // 16], i16,
                           kind="ExternalInput")

    xnew = nc.dram_tensor("xnew", [NODES_PER_CORE, 128], f32,
                          kind="ExternalOutput")
    vsd = nc.dram_tensor("vsd", [BLOCKS, 128], f32, kind="ExternalOutput")

    with tile.TileContext(nc) as tc, ExitStack() as ctx:
        consts = ctx.enter_context(tc.tile_pool(name="consts", bufs=1))
        lhsp = ctx.enter_context(tc.tile_pool(name="lhs", bufs=3))
        densep = ctx.enter_context(tc.tile_pool(name="dense", bufs=2))
        g8p = ctx.enter_context(tc.tile_pool(name="g8", bufs=2))
        otp = ctx.enter_context(tc.tile_pool(name="ot", bufs=3))
        ohp = ctx.enter_context(tc.tile_pool(name="ohf", bufs=3))
        sxp = ctx.enter_context(tc.tile_pool(name="sx", bufs=3))
        sp = ctx.enter_context(tc.tile_pool(name="s", bufs=3))
        ezp = ctx.enter_context(tc.tile_pool(name="ez", bufs=3))
        egp = ctx.enter_context(tc.tile_pool(name="eg", bufs=4))
        lnp = ctx.enter_context(tc.tile_pool(name="ln", bufs=2))
        lgp = ctx.enter_context(tc.tile_pool(name="lg", bufs=4))
        outp = ctx.enter_context(tc.tile_pool(name="out", bufs=2))
        dramp = ctx.enter_context(tc.tile_pool(name="dram", bufs=1,
                                               space="DRAM"))
        pup = ctx.enter_context(tc.tile_pool(name="pu", bufs=2, space="PSUM"))
        pzp = ctx.enter_context(tc.tile_pool(name="pz", bufs=1, space="PSUM"))
        pdenp = ctx.enter_context(tc.tile_pool(name="pden", bufs=1,
                                               space="PSUM"))
        paggp = ctx.enter_context(tc.tile_pool(name="pagg", bufs=2,
                                               space="PSUM"))

        def load_const(src_ap, shape, dtype, name):
            t = consts.tile(shape, dtype, tag=name)
            nc.sync.dma_start(t[:], src_ap)
            return t

        wl_sb = load_const(WlS[:], [128, HC], bf16, "wl")
        wr_sb = load_const(WrS[:], [128, HC], bf16, "wr")
        sgn_sb = load_const(sgnT[:], [128, H], bf16, "sgn")
        wlq_sb = load_const(WlQ[:], [128, H], bf16, "wlq")
        wrq_sb = load_const(WrQ[:], [128, H], bf16, "wrq")
        invatt_sb = load_const(invatt4B[:], [128, HC], f32, "invatt")
        if not bias_zero:
            blB_sb = load_const(blB[:], [128, HC], f32, "blB")
            brB_sb = load_const(brB[:], [128, HC], f32, "brB")
            biasB_sb = load_const(biasB[:], [128, 128], f32, "biasB")
        if not ln_triv:
            lngB_sb = load_const(lngB[:], [128, 128], f32, "lngB")
            lnbB_sb = load_const(lnbB[:], [128, 128], f32, "lnbB")

        xl_dram = dramp.tile([N_ROWS, HC], f16)

        alphaP = consts.tile([128, 1], f32, tag="alphaP")
        nc.vector.memset(alphaP[:], NEG_SLOPE)
        epsP = consts.tile([128, 1], f32, tag="epsP")
        nc.vector.memset(epsP[:], LN_EPS)

        blk_loads = {}

        def prefetch_loads(b):
            six = sxp.tile([128, cap // 16], i16, tag="sidx")
            nc.sync.dma_start(six[:], sidxd[b])
            ot = otp.tile([128, K + 2, BLK], fp8e5, tag="ot")
            nc.sync.dma_start(
                ot[:], ohtidd[b].rearrange("p (k e) -> p k e", e=BLK))
            ohb = ohp.tile([128, cap], fp8e5, tag="oh")
            nc.sync.dma_start(ohb[:], ohd[b])
            blk_loads[b] = (six, ot, ohb)

        st = {"g8": {}, "agg": {}, "den": {}, "s": {}, "u": {},
              "ezf": {}, "ezb": {}, "eg": {}, "zp": {}, "p06": None}

        POS = (K // 2) // GROUP * GROUP   # middle slot, group-aligned

        def prefetch_gather(b):
            six, ot, ohb = blk_loads[b]
            g8 = g8p.tile([128, K + 1, HC], f16, tag="g8")
            st["g8"][b] = g8
            # chunk k -> tile slot k (k < POS) or k+1 (k >= POS)
            # splits sized under the 1024-descriptor SWDGE FIFO carveout
            ranges = []
            for lo, hi in ((0, POS), (POS, K)):
                n_sp = -(-(hi - lo) * BLK // 1008)
                bnds = [lo + (hi - lo) * i // n_sp for i in range(n_sp + 1)]
                ranges += list(zip(bnds[:-1], bnds[1:]))
            for k0, k1 in ranges:
                s0 = k0 if k1 <= POS else k0 + 1
                n_idx = (k1 - k0) * BLK
                nc.gpsimd.dma_gather(
                    out_ap=g8[:, s0:s0 + (k1 - k0), :], in_ap=xl_dram[:],
                    idxs_ap=six[:, k0 * BLK // 16:k1 * BLK // 16],
                    num_idxs=n_idx, num_idxs_reg=n_idx, elem_size=HC,
                    single_packet=False)

        # ---- dense: xl'' for all nodes -> DRAM fp16 ----
        xT_sb = consts.tile([128, N_ROWS], bf16, tag="xT")
        for q in range(4):
            c0 = (N_ROWS // 4 // 128) * 128 * q
            c1 = N_ROWS if q == 3 else (N_ROWS // 4 // 128) * 128 * (q + 1)
            nc.sync.dma_start(xT_sb[:, c0:c1], xT[:, c0:c1])
        prefetch_loads(0)
        prefetch_loads(1)
        GB = 8
        for t0 in range(0, N_TILES, GB):
            n_sub = min(GB, N_TILES - t0)
            xs4 = densep.tile([128, GB, HC], f16, tag="xs4")
            for j in range(n_sub):
                t_i = t0 + j
                xt_ap = xT_sb[:, t_i * 128:(t_i + 1) * 128]
                ps4 = pup.tile([128, 2, HC], f32, tag="uT")
                ps = ps4[:, 0, :]
                nc.tensor.matmul(ps, xt_ap, wl_sb[:], start=True, stop=True)
                if bias_zero:
                    if t_i % 2 == 0:
                        nc.vector.tensor_scalar(out=xs4[:, j, :HC], in0=ps,
                                                scalar1=1.0, scalar2=None,
                                                op0=ALU.mult)
                    else:
                        nc.scalar.activation(xs4[:, j, :HC], ps, AF.Copy)
                else:
                    nc.vector.tensor_tensor(out=xs4[:, j, :HC], in0=ps,
                                            in1=blB_sb[:], op=ALU.add)
            nc.scalar.dma_start(
                xl_dram[t0 * 128:(t0 + n_sub) * 128, :].rearrange(
                    "(t p) c -> p t c", p=128),
                xs4[:, :n_sub, :])

        # ---- edge phase: flattened per-chunk software pipeline ----
        prefetch_gather(0)
        P = K // 2

        def blk_state(b):
            if b not in st["agg"]:
                g8 = st["g8"][b]
                # xr'' for this block -> fp16 slot 0
                lhs = lhsp.tile([128, 128], bf16, tag="lhs")
                nc.sync.dma_start(lhs[:], xlocT[:, b * 128:(b + 1) * 128])
                psr4 = pup.tile([128, 2, HC], f32, tag="uT")
                psr = psr4[:, 0, :]
                nc.tensor.matmul(psr, lhs[:], wr_sb[:], start=True,
                                 stop=True)
                if bias_zero:
                    nc.vector.tensor_scalar(out=g8[:, POS, :HC], in0=psr,
                                            scalar1=1.0, scalar2=None,
                                            op0=ALU.mult)
                else:
                    nc.vector.tensor_tensor(out=g8[:, POS, :HC], in0=psr,
                                            in1=brB_sb[:], op=ALU.add)
                agg_t = paggp.tile([128, HC], f32, tag="agg")
                den_t = pdenp.tile([128, 4], f32, tag="den")
                st["agg"][b] = agg_t
                st["den"][b] = den_t

        def front(b, k):
            if k == 0:
                blk_state(b)
            g8 = st["g8"][b]
            ge5 = g8[:].bitcast(fp8e5)     # [128, K+1, 2*HC]
            ot = blk_loads[b][1]
            m, half = divmod(k, 2)
            if half == 0:
                uT_t = pup.tile([128, 2, HC], f32, tag="uT")
                st["u"][(b, m)] = uT_t
            uT = st["u"][(b, m)]
            gslot = k if k < POS else k + 1
            for h in range(H):
                lo = 2 * h * 128 + 1
                out_ap = uT[:, half, h * 128:(h + 1) * 128]
                if k < POS:
                    # lhs halves (g, xr) pair with rhs halves (id, oht)
                    lhs_ap = ge5[:, gslot:POS + 1:POS - gslot, lo:lo + 255:2]
                    rhs_ap = ot[:, 0:k + 2:k + 1, :]
                else:
                    # lhs halves (xr, g) pair with rhs halves (oht, id)
                    lhs_ap = ge5[:, POS:gslot + 1:gslot - POS, lo:lo + 255:2]
                    rhs_ap = ot[:, k + 1:K + 2:K - k, :]
                nc.tensor.matmul(out_ap, lhs_ap, rhs_ap, start=True,
                                 stop=True, perf_mode=PM.DoubleRow)
            if k == 16 and b + 2 < BLOCKS:
                prefetch_loads(b + 2)
            if k == 20 and b + 1 < BLOCKS:
                prefetch_gather(b + 1)

        def sabs(b, m):
            uT = st["u"].pop((b, m))
            s_ = sp.tile([128, 2, HC], bf16, tag="s")
            nc.scalar.activation(s_[:], uT[:], AF.Prelu, alpha=alphaP[:])
            st["s"][(b, m)] = s_

        def zmm(b, k):
            g, slot = divmod(k, GROUP)
            if slot == 0:
                zP_t = pzp.tile([128, 4 * GROUP], f32, tag="zP")
                st["zp"][b] = zP_t
            zP = st["zp"][b]
            m, half = divmod(k, 2)
            s_ = st["s"][(b, m)]
            for h in range(H):
                nc.tensor.matmul(zP[:, slot * 4 + h:slot * 4 + h + 1],
                                 s_[:, half, h * 128:(h + 1) * 128],
                                 sgn_sb[:, h:h + 1],
                                 start=True, stop=True)
            if half == 1:
                st["s"].pop((b, m))
            if slot == GROUP - 1 or k == K - 1:
                n4 = (slot + 1) * 4
                ezf = ezp.tile([128, 4 * GROUP], f32, tag="ezf")
                nc.scalar.activation(ezf[:, :n4], zP[:, :n4], AF.Exp)
                ezb = ezp.tile([128, 4 * GROUP], f16, tag="ezb")
                nc.vector.tensor_scalar(out=ezb[:, :n4], in0=ezf[:, :n4],
                                        scalar1=EZ_CLAMP, scalar2=None,
                                        op0=ALU.min)
                st["ezf"][(b, g)] = ezf
                st["ezb"][(b, g)] = ezb

        def eg_stage(b, k):
            g8 = st["g8"][b]
            m, half = divmod(k, 2)
            if half == 0:
                egt_t = egp.tile([128, 2, HC], bf16, tag="eg")
                st["eg"][(b, m)] = egt_t
            egt = st["eg"][(b, m)]
            grp, slot = divmod(k, GROUP)
            ezb = st["ezb"][(b, grp)]
            gslot = k if k < POS else k + 1
            in0 = g8[:, gslot, :HC].rearrange("p (h c) -> p h c", h=H)
            in1 = ezb[:, slot * 4:slot * 4 + 4].unsqueeze(-1).broadcast_to(
                [128, H, 128])
            out = egt[:, half, :].rearrange("p (h c) -> p h c", h=H)
            eng = EG_PAT[k % len(EG_PAT)]
            if eng == "DVE":
                nc.vector.tensor_tensor(out=out, in0=in0, in1=in1,
                                        op=ALU.mult)
            else:
                nc.gpsimd.tensor_tensor(out=out, in0=in0, in1=in1,
                                        op=ALU.mult)

        def aggden(b, m):
            agg = st["agg"][b]
            den = st["den"][b]
            ohb = blk_loads[b][2]
            egt = st["eg"].pop((b, m))
            for half in (0, 1):
                k = 2 * m + half
                grp, slot = divmod(k, GROUP)
                ezb = st["ezb"][(b, grp)]
                nc.tensor.matmul(agg[:], ohb[:, k * BLK:(k + 1) * BLK],
                                 egt[:, half, :],
                                 start=(k == 0), stop=(k == K - 1))
                nc.tensor.matmul(den[:], ohb[:, k * BLK:(k + 1) * BLK],
                                 ezb[:, slot * 4:slot * 4 + 4],
                                 start=(k == 0), stop=(k == K - 1))
            if m == P - 1:
                tail(b)

        def tail(b):
            agg = st["agg"].pop(b)
            den = st["den"].pop(b)
            st["g8"].pop(b, None)
            st["zp"].pop(b, None)
            blk_loads.pop(b, None)
            for key in [x for x in st["ezf"] if x[0] == b]:
                st["ezf"].pop(key)
            for key in [x for x in st["ezb"] if x[0] == b]:
                st["ezb"].pop(key)
            rden = lgp.tile([128, 4], f32, tag="rden")
            nc.vector.reciprocal(rden[:], den[:])
            # tq = agg * rden (head-broadcast) on DVE
            tq = lnp.tile([128, HC], f32, tag="tq")
            nc.vector.tensor_tensor(
                out=tq[:].rearrange("p (h c) -> p h c", h=H),
                in0=agg[:].rearrange("p (h c) -> p h c", h=H),
                in1=rden[:].unsqueeze(-1).broadcast_to([128, H, 128]),
                op=ALU.mult)
            tq2 = lnp.tile([128, HC], f32, tag="tq2")
            nc.gpsimd.tensor_tensor(out=tq2[:], in0=tq[:], in1=invatt_sb[:],
                                    op=ALU.mult)
            hm = outp.tile([128, 128], f32, tag="hm")
            nc.vector.tensor_reduce(
                out=hm[:], in_=tq2[:].rearrange("p (h c) -> p c h", h=H),
                axis=X, op=ALU.add)
            xt = outp.tile([128, 128], f32, tag="xres")
            nc.sync.dma_start(xt[:], xloc[b * 128:(b + 1) * 128, :])
            if bias_zero:
                r2 = outp.tile([128, 128], f32, tag="r2")
                nc.gpsimd.tensor_tensor(out=r2[:], in0=hm[:], in1=xt[:],
                                        op=ALU.add)
            else:
                r1 = outp.tile([128, 128], f32, tag="r1")
                nc.vector.tensor_tensor(out=r1[:], in0=hm[:],
                                        in1=biasB_sb[:], op=ALU.add)
                r2 = outp.tile([128, 128], f32, tag="r2")
                nc.gpsimd.tensor_tensor(out=r2[:], in0=r1[:], in1=xt[:],
                                        op=ALU.add)
            mu = lgp.tile([128, 1], f32, tag="mu")
            nc.vector.tensor_reduce(out=mu[:], in_=r2[:], axis=X, op=ALU.add)
            mun = lgp.tile([128, 1], f32, tag="mun")
            nc.vector.tensor_scalar_mul(mun[:], mu[:], 1.0 / 128)
            xc = outp.tile([128, 128], f32, tag="xc")
            nc.vector.tensor_scalar(out=xc[:], in0=r2[:], scalar1=mun[:],
                                    scalar2=None, op0=ALU.subtract)
            junk = outp.tile([128, 128], f32, tag="junk")
            vs = lgp.tile([128, 1], f32, tag="vs")
            nc.vector.scalar_tensor_tensor(
                out=junk[:], in0=r2[:], scalar=mun[:], in1=xc[:],
                op0=ALU.subtract, op1=ALU.mult, accum_out=vs[:])
            nc.sync.dma_start(vsd[b], vs[:, 0])
            xout = outp.tile([128, 128], f32, tag="xout")
            if ln_triv:
                # relu only; the 1/sqrt(var+eps) row scale is applied on
                # the host (relu commutes with a positive per-row scale)
                nc.scalar.activation(xout[:], xc[:], AF.Relu)
            else:
                lt = lgp.tile([128, 1], f32, tag="lt")
                nc.scalar.activation(lt[:], vs[:], AF.Ln, bias=epsP[:],
                                     scale=1.0 / 128)
                rstd = lgp.tile([128, 1], f32, tag="rstd")
                nc.scalar.activation(rstd[:], lt[:], AF.Exp, scale=-0.5)
                xn = outp.tile([128, 128], f32, tag="xn")
                nc.vector.tensor_scalar(out=xn[:], in0=xc[:],
                                        scalar1=rstd[:],
                                        scalar2=None, op0=ALU.mult)
                xg = outp.tile([128, 128], f32, tag="xg")
                nc.vector.tensor_tensor(out=xg[:], in0=xn[:], in1=lngB_sb[:],
                                        op=ALU.mult)
                xgb = outp.tile([128, 128], f32, tag="xgb")
                nc.vector.tensor_tensor(out=xgb[:], in0=xg[:],
                                        in1=lnbB_sb[:], op=ALU.add)
                nc.scalar.activation(xout[:], xgb[:], AF.Relu)
            nc.sync.dma_start(xnew[b * 128:(b + 1) * 128, :], xout[:])

        chunks = [(b, k) for b in range(BLOCKS) for k in range(K)]
        NCH = len(chunks)
        for i in range(NCH + OFF_AGG + 1):
            if i < NCH:
                front(*chunks[i])
            if 0 <= i - 1 < NCH and chunks[i - 1][1] % 2 == 1:
                b, k = chunks[i - 1]
                sabs(b, k // 2)
            if 0 <= i - 2 < NCH:
                zmm(*chunks[i - 2])
            if 0 <= i - OFF_EG < NCH:
                eg_stage(*chunks[i - OFF_EG])
            if 0 <= i - OFF_AGG < NCH and chunks[i - OFF_AGG][1] % 2 == 0:
                b, k = chunks[i - OFF_AGG]
                aggden(b, k // 2)

    nc.compile()
    return nc


def kernel(x, edge_index, Wl, bl, Wr, br, att, bias, ln_g, ln_b):
    x = np.asarray(x, np.float32)
    edge_index = np.asarray(edge_index)
    Wl = np.asarray(Wl, np.float32); bl = np.asarray(bl, np.float32)
    Wr = np.asarray(Wr, np.float32); br = np.asarray(br, np.float32)
    att = np.asarray(att, np.float32); bias = np.asarray(bias, np.float32)
    ln_g = np.asarray(ln_g, np.float32); ln_b = np.asarray(ln_b, np.float32)

    K, src_arr, dpos_arr = _prep_edges(edge_index)
    sidx, ohtid, ohflat = _build_ship_arrays(K, src_arr, dpos_arr)

    bias_zero = not (np.any(bias) or np.any(bl) or np.any(br))
    ln_triv = (np.all(ln_g == 1.0) and not np.any(ln_b))
    key = (K, bias_zero, ln_triv)
    if key not in _NC_CACHE:
        _NC_CACHE[key] = _build_nc(K, bias_zero, ln_triv)
    nc = _NC_CACHE[key]

    aatt = np.maximum(np.abs(att), 1e-30)
    sgn = np.sign(att).astype(np.float32)
    sgn[sgn == 0] = 1.0

    LAST_RESULTS.clear()
    cur = x
    for l in range(L):
        a_flat = aatt[l].reshape(HC)
        WlSf = Wl[l] * a_flat[None, :]
        WrSf = Wr[l] * a_flat[None, :]
        WlS = WlSf.astype(BF16)
        WrS = WrSf.astype(BF16)
        sgnT = np.ascontiguousarray(
            (sgn[l] * SGN_COMP).T).astype(BF16)   # [C, H]
        WlQ = np.zeros((D, H), np.float32)
        WrQ = np.zeros((D, H), np.float32)

        xpad = np.zeros((N_ROWS, 128), np.float32)
        xpad[:N_NODES] = cur
        xT = np.ascontiguousarray(xpad.T.astype(BF16))
        xloc_full = np.zeros((N_PAD, 128), np.float32)
        xloc_full[:N_NODES] = cur

        common = {
            "xT": xT, "WlS": WlS, "WrS": WrS,
            "blB": _bcast(bl[l] * a_flat), "brB": _bcast(br[l] * a_flat),
            "sgnT": sgnT, "WlQ": WlQ.astype(BF16), "WrQ": WrQ.astype(BF16),
            "invatt4B": _bcast(0.25 / a_flat),
            "biasB": _bcast(bias[l]), "lngB": _bcast(ln_g[l]),
            "lnbB": _bcast(ln_b[l]),
        }
        in_maps = []
        for c in range(N_CORES):
            xl_c = np.ascontiguousarray(
                xloc_full[c * NODES_PER_CORE:(c + 1) * NODES_PER_CORE])
            in_maps.append({
                **common,
                "xloc": xl_c,
                "xlocT": np.ascontiguousarray(xl_c.T.astype(BF16)),
                "ohtidd": ohtid[c], "ohd": ohflat[c], "sidxd": sidx[c],
            })

        res = run_bass_kernel_spmd(nc, in_maps, core_ids=list(range(N_CORES)))
        LAST_RESULTS.append(res)
        nxt = np.concatenate([res.results[c]["xnew"] for c in range(N_CORES)],
                             axis=0)
        if ln_triv:
            vsall = np.concatenate(
                [np.asarray(res.results[c]["vsd"]).reshape(-1)
                 for c in range(N_CORES)])
            rstd = 1.0 / np.sqrt(vsall / 128.0 + LN_EPS)
            nxt = nxt * rstd[:, None]
        cur = np.ascontiguousarray(nxt[:N_NODES]).astype(np.float32)

    return cur.astype(np.float32)


# revision 21
# speedup vs baseline: 1.2037x; 1.0132x over previous
"""GATv2 (2 layers, H=4, C=128, head-mean) on 8 TRN2 cores, dst-partitioned.

v4 design (per layer, one SPMD launch of a shared single-layer NEFF):
  dense: xl'' = x @ (Wl .* |att|) for ALL nodes -> fp16 to DRAM (the value
         path uses full fp16; the logit path uses each fp16's HIGH BYTE,
         which is exactly the e5m2-truncated value, via a stride-2 bitcast
         AP - no separate fp8 conversion pass). xr'' per local block ->
         fp16 slot 0 of the gather tile.
  per 128-edge chunk (edges sorted by dst, 10 blocks x 128 dst/core):
    - SWDGE row-gather of fp16 xl''[src] into slot k+1 of g; trailing
      pad-edge indices are negative so the DGE skips their descriptors
    - front: per head one fp8e5 DoubleRow matmul on the high-byte view:
      uT[c,e] = sum_d xr_e5[d,c]*oht[d,e] + g_e5[e,c]
    - prelu on ACT, batched per chunk-pair ([128,1024] per op); the
      e5m2 truncation bias is compensated by scaling sgn by 1.09
    - logits: per head a [128,1] matmul  z[e] = sum_c s'T[c,e]*sgn[c]
    - exp batched over 8 chunks (one ACT op per [128,32] group); ezb =
      min(ez, 3e4) fp16 guards fp16/inf for skipped pad edges
    - eg[e,hc] = g * ez via ONE tensor_tensor with a broadcast AP per
      chunk (DVE/POOL round-robin) -> bf16
    - agg += oh^T @ eg, den += oh^T @ ezb (fp8e5 one-hots)
  All stages run in a flattened cross-block per-chunk software pipeline.
  tail per block: alpha = agg*rden, .*(0.25/|att|), head-sum, +residual,
  then mean-center and relu ON DEVICE; the 1/sqrt(var+eps) row scale is
  applied on the HOST (relu commutes with the positive scale), so the ACT
  engine only ever runs {Prelu, Exp, Relu} -> zero act-table swaps.
Host: edge sorting, fp8e5 one-hot (oht|id) and oh arrays, wrapped gather
idxs, per-row LayerNorm scale between layers.
"""

from contextlib import ExitStack

import numpy as np
import ml_dtypes

import concourse.bacc as bacc
import concourse.tile as tile
from concourse import mybir
from concourse.bass_utils import run_bass_kernel_spmd

BF16 = ml_dtypes.bfloat16
FP8E5 = ml_dtypes.float8_e5m2
F16 = np.float16

N_NODES = 10000
D = 128
H = 4
C = 128
HC = H * C
NEG_SLOPE = 0.2
LN_EPS = 1e-5
L = 2
SGN_COMP = 1.09     # compensates the e5m2 truncation shrink of logits
EZ_CLAMP = 30000.0  # keeps exp() of stale pad-edge logits finite in fp16

N_CORES = 8
NODES_PER_CORE = 1280
BLOCKS = 10
BLK = 128
N_PAD = N_CORES * NODES_PER_CORE    # 10240
N_ROWS = 10112                      # 79*128
N_TILES = N_ROWS // 128

_NC_CACHE = {}
LAST_RESULTS = []   # BassKernelResults per launch (for test harness)

# engine round-robin patterns (tuned against the cost model)
EG_PAT = ["POOL", "DVE", "POOL"]
SABS_PAT = ["ACT", "ACT", "ACT", "ACT", "ACT", "ACT", "DVE"]
GROUP = 8             # chunks per exp batch
OFF_EG = GROUP + 2    # eg stage offset (must trail the group exp)
OFF_AGG = OFF_EG + 2


def _prep_edges(edge_index):
    src = np.concatenate([np.asarray(edge_index[0], np.int64),
                          np.arange(N_NODES, dtype=np.int64)])
    dst = np.concatenate([np.asarray(edge_index[1], np.int64),
                          np.arange(N_NODES, dtype=np.int64)])
    pad_nodes = np.arange(N_NODES, N_PAD, dtype=np.int64)
    src = np.concatenate([src, np.zeros_like(pad_nodes)])
    dst = np.concatenate([dst, pad_nodes])

    order = np.argsort(dst, kind="stable")
    src = src[order]
    dst = dst[order]

    blk_of_edge = dst // BLK
    n_blocks_total = N_PAD // BLK
    counts = np.bincount(blk_of_edge, minlength=n_blocks_total)
    K = int(np.max((counts + BLK - 1) // BLK))
    K += K % 2  # even, so we can process chunk pairs

    cap = K * BLK
    src_arr = np.zeros((n_blocks_total, cap), np.int32)
    dpos_arr = np.full((n_blocks_total, cap), -1, np.int32)
    block_starts = np.zeros(n_blocks_total + 1, np.int64)
    np.cumsum(counts, out=block_starts[1:])
    slot = np.arange(len(dst)) - block_starts[blk_of_edge]
    src_arr[blk_of_edge, slot] = src.astype(np.int32)
    dpos_arr[blk_of_edge, slot] = (dst - blk_of_edge * BLK).astype(np.int32)

    return (K, src_arr.reshape(N_CORES, BLOCKS, cap),
            dpos_arr.reshape(N_CORES, BLOCKS, cap))


def _build_ship_arrays(K, src_arr, dpos_arr):
    cap = K * BLK
    # wrapped gather indices: idx i lives at [i % 16, i // 16]; the 16-row
    # pattern is tiled 8x along partitions (one copy per SWDGE Q7 core).
    # pad slots are -1: the DGE skips trailing negative indices.
    s = src_arr.reshape(N_CORES, BLOCKS, cap // 16, 16)
    s = np.swapaxes(s, 2, 3)                                  # [c,b,16,cap/16]
    sidx = np.tile(s, (1, 1, 8, 1)).astype(np.int16)          # [c,b,128,cap/16]

    # fp8e5 one-hots:
    # ohtid [c,b, d(128), (K+2)*128]: slot 0 = identity, slot 1+k =
    #   oht chunk k (col (1+k)*128+e -> 1 iff dst(chunk k, e) == d),
    #   slot K+1 = identity.  Identities at both ends let the DoubleRow
    #   matmul pair (g, xr@middle) with (id, oht) using positive AP steps.
    # ohflat [c,b, e(128), cap]: col k*128+d -> oh[e, k, d]
    ohtid = np.zeros((N_CORES, BLOCKS, BLK, (K + 2) * BLK), FP8E5)
    ohflat = np.zeros((N_CORES, BLOCKS, BLK, cap), FP8E5)
    cc, bb, ss = np.nonzero(dpos_arr >= 0)
    kk = (ss // BLK).astype(np.int64)
    ee = (ss % BLK).astype(np.int64)
    dd = dpos_arr[cc, bb, ss].astype(np.int64)
    ohtid[cc, bb, dd, (kk + 1) * BLK + ee] = 1
    ohflat[cc, bb, ee, kk * BLK + dd] = 1
    i = np.arange(BLK)
    ohtid[:, :, i, i] = 1
    ohtid[:, :, i, (K + 1) * BLK + i] = 1
    return (np.ascontiguousarray(sidx), np.ascontiguousarray(ohtid),
            np.ascontiguousarray(ohflat))


def _bcast(v, rows=128):
    v = np.asarray(v, np.float32)
    return np.ascontiguousarray(np.broadcast_to(v[None, :], (rows, v.shape[0])))


def _build_nc(K, bias_zero, ln_triv):
    nc = bacc.Bacc("TRN2", target_bir_lowering=False, debug=False,
                   num_devices=N_CORES)
    f32, bf16, i16 = mybir.dt.float32, mybir.dt.bfloat16, mybir.dt.int16
    f16 = mybir.dt.float16
    fp8e5 = mybir.dt.float8e5
    AF = mybir.ActivationFunctionType
    ALU = mybir.AluOpType
    PM = mybir.MatmulPerfMode
    X = mybir.AxisListType.X
    cap = K * BLK

    xT = nc.dram_tensor("xT", [128, N_ROWS], bf16, kind="ExternalInput")
    xlocT = nc.dram_tensor("xlocT", [128, NODES_PER_CORE], bf16,
                           kind="ExternalInput")
    xloc = nc.dram_tensor("xloc", [NODES_PER_CORE, 128], f32,
                          kind="ExternalInput")
    WlS = nc.dram_tensor("WlS", [128, HC], bf16, kind="ExternalInput")
    WrS = nc.dram_tensor("WrS", [128, HC], bf16, kind="ExternalInput")
    blB = nc.dram_tensor("blB", [128, HC], f32, kind="ExternalInput")
    brB = nc.dram_tensor("brB", [128, HC], f32, kind="ExternalInput")
    sgnT = nc.dram_tensor("sgnT", [128, H], bf16, kind="ExternalInput")
    WlQ = nc.dram_tensor("WlQ", [128, H], bf16, kind="ExternalInput")
    WrQ = nc.dram_tensor("WrQ", [128, H], bf16, kind="ExternalInput")
    invatt4B = nc.dram_tensor("invatt4B", [128, HC], f32, kind="ExternalInput")
    biasB = nc.dram_tensor("biasB", [128, 128], f32, kind="ExternalInput")
    lngB = nc.dram_tensor("lngB", [128, 128], f32, kind="ExternalInput")
    lnbB = nc.dram_tensor("lnbB", [128, 128], f32, kind="ExternalInput")
    ohtidd = nc.dram_tensor("ohtidd", [BLOCKS, BLK, (K + 2) * BLK], fp8e5,
                            kind="ExternalInput")
    ohd = nc.dram_tensor("ohd", [BLOCKS, BLK, cap], fp8e5,
                         kind="ExternalInput")
    sidxd = nc.dram_tensor("sidxd", [BLOCKS, 128, cap // 16], i16,
                           kind="ExternalInput")

    xnew = nc.dram_tensor("xnew", [NODES_PER_CORE, 128], f32,
                          kind="ExternalOutput")
    vsd = nc.dram_tensor("vsd", [BLOCKS, 128], f32, kind="ExternalOutput")

    with tile.TileContext(nc) as tc, ExitStack() as ctx:
        consts = ctx.enter_context(tc.tile_pool(name="consts", bufs=1))
        lhsp = ctx.enter_context(tc.tile_pool(name="lhs", bufs=3))
        densep = ctx.enter_context(tc.tile_pool(name="dense", bufs=2))
        g8p = ctx.enter_context(tc.tile_pool(name="g8", bufs=2))
        otp = ctx.enter_context(tc.tile_pool(name="ot", bufs=3))
        ohp = ctx.enter_context(tc.tile_pool(name="ohf", bufs=3))
        sxp = ctx.enter_context(tc.tile_pool(name="sx", bufs=3))
        sp = ctx.enter_context(tc.tile_pool(name="s", bufs=3))
        ezp = ctx.enter_context(tc.tile_pool(name="ez", bufs=3))
        egp = ctx.enter_context(tc.tile_pool(name="eg", bufs=4))
        lnp = ctx.enter_context(tc.tile_pool(name="ln", bufs=2))
        lgp = ctx.enter_context(tc.tile_pool(name="lg", bufs=4))
        outp = ctx.enter_context(tc.tile_pool(name="out", bufs=2))
        dramp = ctx.enter_context(tc.tile_pool(name="dram", bufs=1,
                                               space="DRAM"))
        pup = ctx.enter_context(tc.tile_pool(name="pu", bufs=2, space="PSUM"))
        pzp = ctx.enter_context(tc.tile_pool(name="pz", bufs=1, space="PSUM"))
        pdenp = ctx.enter_context(tc.tile_pool(name="pden", bufs=1,
                                               space="PSUM"))
        paggp = ctx.enter_context(tc.tile_pool(name="pagg", bufs=2,
                                               space="PSUM"))

        def load_const(src_ap, shape, dtype, name):
            t = consts.tile(shape, dtype, tag=name)
            nc.sync.dma_start(t[:], src_ap)
            return t

        wl_sb = load_const(WlS[:], [128, HC], bf16, "wl")
        wr_sb = load_const(WrS[:], [128, HC], bf16, "wr")
        sgn_sb = load_const(sgnT[:], [128, H], bf16, "sgn")
        wlq_sb = load_const(WlQ[:], [128, H], bf16, "wlq")
        wrq_sb = load_const(WrQ[:], [128, H], bf16, "wrq")
        invatt_sb = load_const(invatt4B[:], [128, HC], f32, "invatt")
        if not bias_zero:
            blB_sb = load_const(blB[:], [128, HC], f32, "blB")
            brB_sb = load_const(brB[:], [128, HC], f32, "brB")
            biasB_sb = load_const(biasB[:], [128, 128], f32, "biasB")
        if not ln_triv:
            lngB_sb = load_const(lngB[:], [128, 128], f32, "lngB")
            lnbB_sb = load_const(lnbB[:], [128, 128], f32, "lnbB")

        xl_dram = dramp.tile([N_ROWS, HC], f16)

        alphaP = consts.tile([128, 1], f32, tag="alphaP")
        nc.vector.memset(alphaP[:], NEG_SLOPE)
        epsP = consts.tile([128, 1], f32, tag="epsP")
        nc.vector.memset(epsP[:], LN_EPS)

        blk_loads = {}

        def prefetch_loads(b):
            six = sxp.tile([128, cap // 16], i16, tag="sidx")
            nc.sync.dma_start(six[:], sidxd[b])
            ot = otp.tile([128, K + 2, BLK], fp8e5, tag="ot")
            nc.sync.dma_start(
                ot[:], ohtidd[b].rearrange("p (k e) -> p k e", e=BLK))
            ohb = ohp.tile([128, cap], fp8e5, tag="oh")
            nc.sync.dma_start(ohb[:], ohd[b])
            blk_loads[b] = (six, ot, ohb)

        st = {"g8": {}, "agg": {}, "den": {}, "s": {}, "u": {},
              "ezf": {}, "ezb": {}, "eg": {}, "zp": {}, "p06": None}

        POS = K // 2   # xr'' lives at the middle slot of g8

        def prefetch_gather(b):
            six, ot, ohb = blk_loads[b]
            g8 = g8p.tile([128, K + 1, HC], f16, tag="g8")
            st["g8"][b] = g8
            # chunk k -> tile slot k (k < POS) or k+1 (k >= POS)
            # splits sized under the 1024-descriptor SWDGE FIFO carveout
            ranges = []
            for lo, hi in ((0, POS), (POS, K)):
                n_sp = -(-(hi - lo) * BLK // 1008)
                bnds = [lo + (hi - lo) * i // n_sp for i in range(n_sp + 1)]
                ranges += list(zip(bnds[:-1], bnds[1:]))
            for k0, k1 in ranges:
                s0 = k0 if k1 <= POS else k0 + 1
                n_idx = (k1 - k0) * BLK
                nc.gpsimd.dma_gather(
                    out_ap=g8[:, s0:s0 + (k1 - k0), :], in_ap=xl_dram[:],
                    idxs_ap=six[:, k0 * BLK // 16:k1 * BLK // 16],
                    num_idxs=n_idx, num_idxs_reg=n_idx, elem_size=HC,
                    single_packet=False)

        # ---- dense: xl'' for all nodes -> DRAM fp16 ----
        xT_sb = consts.tile([128, N_ROWS], bf16, tag="xT")
        for q in range(4):
            c0 = (N_ROWS // 4 // 128) * 128 * q
            c1 = N_ROWS if q == 3 else (N_ROWS // 4 // 128) * 128 * (q + 1)
            nc.sync.dma_start(xT_sb[:, c0:c1], xT[:, c0:c1])
        prefetch_loads(0)
        prefetch_loads(1)
        GB = 4
        for t0 in range(0, N_TILES, GB):
            n_sub = min(GB, N_TILES - t0)
            xs4 = densep.tile([128, GB, HC], f16, tag="xs4")
            for j in range(n_sub):
                t_i = t0 + j
                xt_ap = xT_sb[:, t_i * 128:(t_i + 1) * 128]
                ps4 = pup.tile([128, 2, HC], f32, tag="uT")
                ps = ps4[:, 0, :]
                nc.tensor.matmul(ps, xt_ap, wl_sb[:], start=True, stop=True)
                if bias_zero:
                    if t_i % 2 == 0:
                        nc.vector.tensor_scalar(out=xs4[:, j, :HC], in0=ps,
                                                scalar1=1.0, scalar2=None,
                                                op0=ALU.mult)
                    else:
                        nc.scalar.activation(xs4[:, j, :HC], ps, AF.Copy)
                else:
                    nc.vector.tensor_tensor(out=xs4[:, j, :HC], in0=ps,
                                            in1=blB_sb[:], op=ALU.add)
            nc.scalar.dma_start(
                xl_dram[t0 * 128:(t0 + n_sub) * 128, :].rearrange(
                    "(t p) c -> p t c", p=128),
                xs4[:, :n_sub, :])

        # ---- edge phase: flattened per-chunk software pipeline ----
        prefetch_gather(0)
        P = K // 2

        def blk_state(b):
            if b not in st["agg"]:
                g8 = st["g8"][b]
                # xr'' for this block -> fp16 slot 0
                lhs = lhsp.tile([128, 128], bf16, tag="lhs")
                nc.sync.dma_start(lhs[:], xlocT[:, b * 128:(b + 1) * 128])
                psr4 = pup.tile([128, 2, HC], f32, tag="uT")
                psr = psr4[:, 0, :]
                nc.tensor.matmul(psr, lhs[:], wr_sb[:], start=True,
                                 stop=True)
                if bias_zero:
                    nc.vector.tensor_scalar(out=g8[:, POS, :HC], in0=psr,
                                            scalar1=1.0, scalar2=None,
                                            op0=ALU.mult)
                else:
                    nc.vector.tensor_tensor(out=g8[:, POS, :HC], in0=psr,
                                            in1=brB_sb[:], op=ALU.add)
                agg_t = paggp.tile([128, HC], f32, tag="agg")
                den_t = pdenp.tile([128, 4], f32, tag="den")
                st["agg"][b] = agg_t
                st["den"][b] = den_t

        def front(b, k):
            if k == 0:
                blk_state(b)
            g8 = st["g8"][b]
            ge5 = g8[:].bitcast(fp8e5)     # [128, K+1, 2*HC]
            ot = blk_loads[b][1]
            m, half = divmod(k, 2)
            if half == 0:
                uT_t = pup.tile([128, 2, HC], f32, tag="uT")
                st["u"][(b, m)] = uT_t
            uT = st["u"][(b, m)]
            gslot = k if k < POS else k + 1
            for h in range(H):
                lo = 2 * h * 128 + 1
                out_ap = uT[:, half, h * 128:(h + 1) * 128]
                if k < POS:
                    # lhs halves (g, xr) pair with rhs halves (id, oht)
                    lhs_ap = ge5[:, gslot:POS + 1:POS - gslot, lo:lo + 255:2]
                    rhs_ap = ot[:, 0:k + 2:k + 1, :]
                else:
                    # lhs halves (xr, g) pair with rhs halves (oht, id)
                    lhs_ap = ge5[:, POS:gslot + 1:gslot - POS, lo:lo + 255:2]
                    rhs_ap = ot[:, k + 1:K + 2:K - k, :]
                nc.tensor.matmul(out_ap, lhs_ap, rhs_ap, start=True,
                                 stop=True, perf_mode=PM.DoubleRow)
            if k == 16 and b + 2 < BLOCKS:
                prefetch_loads(b + 2)
            if k == 20 and b + 1 < BLOCKS:
                prefetch_gather(b + 1)

        def sabs(b, m):
            uT = st["u"].pop((b, m))
            s_ = sp.tile([128, 2, HC], bf16, tag="s")
            nc.scalar.activation(s_[:], uT[:], AF.Prelu, alpha=alphaP[:])
            st["s"][(b, m)] = s_

        def zmm(b, k):
            g, slot = divmod(k, GROUP)
            if slot == 0:
                zP_t = pzp.tile([128, 4 * GROUP], f32, tag="zP")
                st["zp"][b] = zP_t
            zP = st["zp"][b]
            m, half = divmod(k, 2)
            s_ = st["s"][(b, m)]
            for h in range(H):
                nc.tensor.matmul(zP[:, slot * 4 + h:slot * 4 + h + 1],
                                 s_[:, half, h * 128:(h + 1) * 128],
                                 sgn_sb[:, h:h + 1],
                                 start=True, stop=True)
            if half == 1:
                st["s"].pop((b, m))
            if slot == GROUP - 1 or k == K - 1:
                n4 = (slot + 1) * 4
                ezf = ezp.tile([128, 4 * GROUP], f32, tag="ezf")
                nc.scalar.activation(ezf[:, :n4], zP[:, :n4], AF.Exp)
                ezb = ezp.tile([128, 4 * GROUP], f16, tag="ezb")
                nc.vector.tensor_scalar(out=ezb[:, :n4], in0=ezf[:, :n4],
                                        scalar1=EZ_CLAMP, scalar2=None,
                                        op0=ALU.min)
                st["ezf"][(b, g)] = ezf
                st["ezb"][(b, g)] = ezb

        def eg_stage(b, k):
            g8 = st["g8"][b]
            m, half = divmod(k, 2)
            if half == 0:
                egt_t = egp.tile([128, 2, HC], bf16, tag="eg")
                st["eg"][(b, m)] = egt_t
            egt = st["eg"][(b, m)]
            grp, slot = divmod(k, GROUP)
            ezb = st["ezb"][(b, grp)]
            gslot = k if k < POS else k + 1
            in0 = g8[:, gslot, :HC].rearrange("p (h c) -> p h c", h=H)
            in1 = ezb[:, slot * 4:slot * 4 + 4].unsqueeze(-1).broadcast_to(
                [128, H, 128])
            out = egt[:, half, :].rearrange("p (h c) -> p h c", h=H)
            eng = EG_PAT[k % len(EG_PAT)]
            if eng == "DVE":
                nc.vector.tensor_tensor(out=out, in0=in0, in1=in1,
                                        op=ALU.mult)
            else:
                nc.gpsimd.tensor_tensor(out=out, in0=in0, in1=in1,
                                        op=ALU.mult)

        def aggden(b, m):
            agg = st["agg"][b]
            den = st["den"][b]
            ohb = blk_loads[b][2]
            egt = st["eg"].pop((b, m))
            for half in (0, 1):
                k = 2 * m + half
                grp, slot = divmod(k, GROUP)
                ezb = st["ezb"][(b, grp)]
                nc.tensor.matmul(agg[:], ohb[:, k * BLK:(k + 1) * BLK],
                                 egt[:, half, :],
                                 start=(k == 0), stop=(k == K - 1))
                nc.tensor.matmul(den[:], ohb[:, k * BLK:(k + 1) * BLK],
                                 ezb[:, slot * 4:slot * 4 + 4],
                                 start=(k == 0), stop=(k == K - 1))
            if m == P - 1:
                tail(b)

        def tail(b):
            agg = st["agg"].pop(b)
            den = st["den"].pop(b)
            st["g8"].pop(b, None)
            st["zp"].pop(b, None)
            blk_loads.pop(b, None)
            for key in [x for x in st["ezf"] if x[0] == b]:
                st["ezf"].pop(key)
            for key in [x for x in st["ezb"] if x[0] == b]:
                st["ezb"].pop(key)
            rden = lgp.tile([128, 4], f32, tag="rden")
            nc.vector.reciprocal(rden[:], den[:])
            # tq = agg * rden (head-broadcast) on DVE
            tq = lnp.tile([128, HC], f32, tag="tq")
            nc.vector.tensor_tensor(
                out=tq[:].rearrange("p (h c) -> p h c", h=H),
                in0=agg[:].rearrange("p (h c) -> p h c", h=H),
                in1=rden[:].unsqueeze(-1).broadcast_to([128, H, 128]),
                op=ALU.mult)
            tq2 = lnp.tile([128, HC], f32, tag="tq2")
            nc.gpsimd.tensor_tensor(out=tq2[:], in0=tq[:], in1=invatt_sb[:],
                                    op=ALU.mult)
            hm = outp.tile([128, 128], f32, tag="hm")
            nc.vector.tensor_reduce(
                out=hm[:], in_=tq2[:].rearrange("p (h c) -> p c h", h=H),
                axis=X, op=ALU.add)
            xt = outp.tile([128, 128], f32, tag="xres")
            nc.sync.dma_start(xt[:], xloc[b * 128:(b + 1) * 128, :])
            if bias_zero:
                r2 = outp.tile([128, 128], f32, tag="r2")
                nc.gpsimd.tensor_tensor(out=r2[:], in0=hm[:], in1=xt[:],
                                        op=ALU.add)
            else:
                r1 = outp.tile([128, 128], f32, tag="r1")
                nc.vector.tensor_tensor(out=r1[:], in0=hm[:],
                                        in1=biasB_sb[:], op=ALU.add)
                r2 = outp.tile([128, 128], f32, tag="r2")
                nc.gpsimd.tensor_tensor(out=r2[:], in0=r1[:], in1=xt[:],
                                        op=ALU.add)
            mu = lgp.tile([128, 1], f32, tag="mu")
            nc.vector.tensor_reduce(out=mu[:], in_=r2[:], axis=X, op=ALU.add)
            mun = lgp.tile([128, 1], f32, tag="mun")
            nc.vector.tensor_scalar_mul(mun[:], mu[:], 1.0 / 128)
            xc = outp.tile([128, 128], f32, tag="xc")
            nc.vector.tensor_scalar(out=xc[:], in0=r2[:], scalar1=mun[:],
                                    scalar2=None, op0=ALU.subtract)
            junk = outp.tile([128, 128], f32, tag="junk")
            vs = lgp.tile([128, 1], f32, tag="vs")
            nc.vector.scalar_tensor_tensor(
                out=junk[:], in0=r2[:], scalar=mun[:], in1=xc[:],
                op0=ALU.subtract, op1=ALU.mult, accum_out=vs[:])
            nc.sync.dma_start(vsd[b], vs[:, 0])
            xout = outp.tile([128, 128], f32, tag="xout")
            if ln_triv:
                # relu only; the 1/sqrt(var+eps) row scale is applied on
                # the host (relu commutes with a positive per-row scale)
                nc.scalar.activation(xout[:], xc[:], AF.Relu)
            else:
                lt = lgp.tile([128, 1], f32, tag="lt")
                nc.scalar.activation(lt[:], vs[:], AF.Ln, bias=epsP[:],
                                     scale=1.0 / 128)
                rstd = lgp.tile([128, 1], f32, tag="rstd")
                nc.scalar.activation(rstd[:], lt[:], AF.Exp, scale=-0.5)
                xn = outp.tile([128, 128], f32, tag="xn")
                nc.vector.tensor_scalar(out=xn[:], in0=xc[:],
                                        scalar1=rstd[:],
                                        scalar2=None, op0=ALU.mult)
                xg = outp.tile([128, 128], f32, tag="xg")
                nc.vector.tensor_tensor(out=xg[:], in0=xn[:], in1=lngB_sb[:],
                                        op=ALU.mult)
                xgb = outp.tile([128, 128], f32, tag="xgb")
                nc.vector.tensor_tensor(out=xgb[:], in0=xg[:],
                                        in1=lnbB_sb[:], op=ALU.add)
                nc.scalar.activation(xout[:], xgb[:], AF.Relu)
            nc.sync.dma_start(xnew[b * 128:(b + 1) * 128, :], xout[:])

        chunks = [(b, k) for b in range(BLOCKS) for k in range(K)]
        NCH = len(chunks)
        for i in range(NCH + OFF_AGG + 1):
            if i < NCH:
                front(*chunks[i])
            if 0 <= i - 1 < NCH and chunks[i - 1][1] % 2 == 1:
                b, k = chunks[i - 1]
                sabs(b, k // 2)
            if 0 <= i - 2 < NCH:
                zmm(*chunks[i - 2])
            if 0 <= i - OFF_EG < NCH:
                eg_stage(*chunks[i - OFF_EG])
            if 0 <= i - OFF_AGG < NCH and chunks[i - OFF_AGG][1] % 2 == 0:
                b, k = chunks[i - OFF_AGG]
                aggden(b, k // 2)

    nc.compile()
    return nc


def kernel(x, edge_index, Wl, bl, Wr, br, att, bias, ln_g, ln_b):
    x = np.asarray(x, np.float32)
    edge_index = np.asarray(edge_index)
    Wl = np.asarray(Wl, np.float32); bl = np.asarray(bl, np.float32)
    Wr = np.asarray(Wr, np.float32); br = np.asarray(br, np.float32)
    att = np.asarray(att, np.float32); bias = np.asarray(bias, np.float32)
    ln_g = np.asarray(ln_g, np.float32); ln_b = np.asarray(ln_b, np.float32)

    K, src_arr, dpos_arr = _prep_edges(edge_index)
    sidx, ohtid, ohflat = _build_ship_arrays(K, src_arr, dpos_arr)

    bias_zero = not (np.any(bias) or np.any(bl) or np.any(br))
    ln_triv = (np.all(ln_g == 1.0) and not np.any(ln_b))
    key = (K, bias_zero, ln_triv)
    if key not in _NC_CACHE:
        _NC_CACHE[key] = _build_nc(K, bias_zero, ln_triv)
    nc = _NC_CACHE[key]

    aatt = np.maximum(np.abs(att), 1e-30)
    sgn = np.sign(att).astype(np.float32)
    sgn[sgn == 0] = 1.0

    LAST_RESULTS.clear()
    cur = x
    for l in range(L):
        a_flat = aatt[l].reshape(HC)
        WlSf = Wl[l] * a_flat[None, :]
        WrSf = Wr[l] * a_flat[None, :]
        WlS = WlSf.astype(BF16)
        WrS = WrSf.astype(BF16)
        sgnT = np.ascontiguousarray(
            (sgn[l] * SGN_COMP).T).astype(BF16)   # [C, H]
        WlQ = np.zeros((D, H), np.float32)
        WrQ = np.zeros((D, H), np.float32)

        xpad = np.zeros((N_ROWS, 128), np.float32)
        xpad[:N_NODES] = cur
        xT = np.ascontiguousarray(xpad.T.astype(BF16))
        xloc_full = np.zeros((N_PAD, 128), np.float32)
        xloc_full[:N_NODES] = cur

        common = {
            "xT": xT, "WlS": WlS, "WrS": WrS,
            "blB": _bcast(bl[l] * a_flat), "brB": _bcast(br[l] * a_flat),
            "sgnT": sgnT, "WlQ": WlQ.astype(BF16), "WrQ": WrQ.astype(BF16),
            "invatt4B": _bcast(0.25 / a_flat),
            "biasB": _bcast(bias[l]), "lngB": _bcast(ln_g[l]),
            "lnbB": _bcast(ln_b[l]),
        }
        in_maps = []
        for c in range(N_CORES):
            xl_c = np.ascontiguousarray(
                xloc_full[c * NODES_PER_CORE:(c + 1) * NODES_PER_CORE])
            in_maps.append({
                **common,
                "xloc": xl_c,
                "xlocT": np.ascontiguousarray(xl_c.T.astype(BF16)),
                "ohtidd": ohtid[c], "ohd": ohflat[c], "sidxd": sidx[c],
            })

        res = run_bass_kernel_spmd(nc, in_maps, core_ids=list(range(N_CORES)))
        LAST_RESULTS.append(res)
        nxt = np.concatenate([res.results[c]["xnew"] for c in range(N_CORES)],
                             axis=0)
        if ln_triv:
            vsall = np.concatenate(
                [np.asarray(res.results[c]["vsd"]).reshape(-1)
                 for c in range(N_CORES)])
            rstd = 1.0 / np.sqrt(vsall / 128.0 + LN_EPS)
            nxt = nxt * rstd[:, None]
        cur = np.ascontiguousarray(nxt[:N_NODES]).astype(np.float32)

    return cur.astype(np.float32)


# revision 22
# speedup vs baseline: 1.2181x; 1.0120x over previous
"""GATv2 (2 layers, H=4, C=128, head-mean) on 8 TRN2 cores, dst-partitioned.

v4 design (per layer, one SPMD launch of a shared single-layer NEFF):
  dense: xl'' = x @ (Wl .* |att|) for ALL nodes -> fp16 to DRAM (the value
         path uses full fp16; the logit path uses each fp16's HIGH BYTE,
         which is exactly the e5m2-truncated value, via a stride-2 bitcast
         AP - no separate fp8 conversion pass). xr'' per local block ->
         fp16 slot 0 of the gather tile.
  per 128-edge chunk (edges sorted by dst, 10 blocks x 128 dst/core):
    - SWDGE row-gather of fp16 xl''[src] into slot k+1 of g; trailing
      pad-edge indices are negative so the DGE skips their descriptors
    - front: per head one fp8e5 DoubleRow matmul on the high-byte view:
      uT[c,e] = sum_d xr_e5[d,c]*oht[d,e] + g_e5[e,c]
    - prelu on ACT, batched per chunk-pair ([128,1024] per op); the
      e5m2 truncation bias is compensated by scaling sgn by 1.09
    - logits: per head a [128,1] matmul  z[e] = sum_c s'T[c,e]*sgn[c]
    - exp batched over 8 chunks (one ACT op per [128,32] group); ezb =
      min(ez, 3e4) fp16 guards fp16/inf for skipped pad edges
    - eg[e,hc] = g * ez via ONE tensor_tensor with a broadcast AP per
      chunk (DVE/POOL round-robin) -> bf16
    - agg += oh^T @ eg, den += oh^T @ ezb (fp8e5 one-hots)
  All stages run in a flattened cross-block per-chunk software pipeline.
  tail per block: alpha = agg*rden, .*(0.25/|att|), head-sum, +residual,
  then mean-center and relu ON DEVICE; the 1/sqrt(var+eps) row scale is
  applied on the HOST (relu commutes with the positive scale), so the ACT
  engine only ever runs {Prelu, Exp, Relu} -> zero act-table swaps.
Host: edge sorting, fp8e5 one-hot (oht|id) and oh arrays, wrapped gather
idxs, per-row LayerNorm scale between layers.
"""

from contextlib import ExitStack

import numpy as np
import ml_dtypes

import concourse.bacc as bacc
import concourse.tile as tile
from concourse import mybir
from concourse.bass_utils import run_bass_kernel_spmd

BF16 = ml_dtypes.bfloat16
FP8E5 = ml_dtypes.float8_e5m2
F16 = np.float16

N_NODES = 10000
D = 128
H = 4
C = 128
HC = H * C
NEG_SLOPE = 0.2
LN_EPS = 1e-5
L = 2
SGN_COMP = 1.09     # compensates the e5m2 truncation shrink of logits
EZ_CLAMP = 30000.0  # keeps exp() of stale pad-edge logits finite in fp16

N_CORES = 8
NODES_PER_CORE = 1280
BLOCKS = 10
BLK = 128
N_PAD = N_CORES * NODES_PER_CORE    # 10240
N_ROWS = 10112                      # 79*128
N_TILES = N_ROWS // 128

_NC_CACHE = {}
LAST_RESULTS = []   # BassKernelResults per launch (for test harness)

# engine round-robin patterns (tuned against the cost model)
EG_PAT = ["POOL", "DVE", "POOL"]
SABS_PAT = ["ACT", "ACT", "ACT", "ACT", "ACT", "ACT", "DVE"]
GROUP = 8             # chunks per exp batch
OFF_EG = GROUP + 2    # eg stage offset (must trail the group exp)
OFF_AGG = OFF_EG + 2


def _prep_edges(edge_index):
    src = np.concatenate([np.asarray(edge_index[0], np.int64),
                          np.arange(N_NODES, dtype=np.int64)])
    dst = np.concatenate([np.asarray(edge_index[1], np.int64),
                          np.arange(N_NODES, dtype=np.int64)])
    pad_nodes = np.arange(N_NODES, N_PAD, dtype=np.int64)
    src = np.concatenate([src, np.zeros_like(pad_nodes)])
    dst = np.concatenate([dst, pad_nodes])

    order = np.argsort(dst, kind="stable")
    src = src[order]
    dst = dst[order]

    blk_of_edge = dst // BLK
    n_blocks_total = N_PAD // BLK
    counts = np.bincount(blk_of_edge, minlength=n_blocks_total)
    K = int(np.max((counts + BLK - 1) // BLK))
    K += K % 2  # even, so we can process chunk pairs

    cap = K * BLK
    src_arr = np.zeros((n_blocks_total, cap), np.int32)
    dpos_arr = np.full((n_blocks_total, cap), -1, np.int32)
    block_starts = np.zeros(n_blocks_total + 1, np.int64)
    np.cumsum(counts, out=block_starts[1:])
    slot = np.arange(len(dst)) - block_starts[blk_of_edge]
    src_arr[blk_of_edge, slot] = src.astype(np.int32)
    dpos_arr[blk_of_edge, slot] = (dst - blk_of_edge * BLK).astype(np.int32)

    return (K, src_arr.reshape(N_CORES, BLOCKS, cap),
            dpos_arr.reshape(N_CORES, BLOCKS, cap))


def _build_ship_arrays(K, src_arr, dpos_arr):
    cap = K * BLK
    # wrapped gather indices: idx i lives at [i % 16, i // 16]; the 16-row
    # pattern is tiled 8x along partitions (one copy per SWDGE Q7 core).
    # pad slots are -1: the DGE skips trailing negative indices.
    s = src_arr.reshape(N_CORES, BLOCKS, cap // 16, 16)
    s = np.swapaxes(s, 2, 3)                                  # [c,b,16,cap/16]
    sidx = np.tile(s, (1, 1, 8, 1)).astype(np.int16)          # [c,b,128,cap/16]

    # fp8e5 one-hots:
    # ohtid [c,b, d(128), (K+2)*128]: slot 0 = identity, slot 1+k =
    #   oht chunk k (col (1+k)*128+e -> 1 iff dst(chunk k, e) == d),
    #   slot K+1 = identity.  Identities at both ends let the DoubleRow
    #   matmul pair (g, xr@middle) with (id, oht) using positive AP steps.
    # ohflat [c,b, e(128), cap]: col k*128+d -> oh[e, k, d]
    ohtid = np.zeros((N_CORES, BLOCKS, BLK, (K + 2) * BLK), FP8E5)
    ohflat = np.zeros((N_CORES, BLOCKS, BLK, cap), FP8E5)
    cc, bb, ss = np.nonzero(dpos_arr >= 0)
    kk = (ss // BLK).astype(np.int64)
    ee = (ss % BLK).astype(np.int64)
    dd = dpos_arr[cc, bb, ss].astype(np.int64)
    ohtid[cc, bb, dd, (kk + 1) * BLK + ee] = 1
    ohflat[cc, bb, ee, kk * BLK + dd] = 1
    i = np.arange(BLK)
    ohtid[:, :, i, i] = 1
    ohtid[:, :, i, (K + 1) * BLK + i] = 1
    return (np.ascontiguousarray(sidx), np.ascontiguousarray(ohtid),
            np.ascontiguousarray(ohflat))


def _bcast(v, rows=128):
    v = np.asarray(v, np.float32)
    return np.ascontiguousarray(np.broadcast_to(v[None, :], (rows, v.shape[0])))


def _build_nc(K, bias_zero, ln_triv):
    nc = bacc.Bacc("TRN2", target_bir_lowering=False, debug=False,
                   num_devices=N_CORES)
    f32, bf16, i16 = mybir.dt.float32, mybir.dt.bfloat16, mybir.dt.int16
    f16 = mybir.dt.float16
    fp8e5 = mybir.dt.float8e5
    AF = mybir.ActivationFunctionType
    ALU = mybir.AluOpType
    PM = mybir.MatmulPerfMode
    X = mybir.AxisListType.X
    cap = K * BLK

    xT = nc.dram_tensor("xT", [128, N_ROWS], bf16, kind="ExternalInput")
    xlocT = nc.dram_tensor("xlocT", [128, NODES_PER_CORE], bf16,
                           kind="ExternalInput")
    xloc = nc.dram_tensor("xloc", [NODES_PER_CORE, 128], f32,
                          kind="ExternalInput")
    WlS = nc.dram_tensor("WlS", [128, HC], bf16, kind="ExternalInput")
    WrS = nc.dram_tensor("WrS", [128, HC], bf16, kind="ExternalInput")
    blB = nc.dram_tensor("blB", [128, HC], f32, kind="ExternalInput")
    brB = nc.dram_tensor("brB", [128, HC], f32, kind="ExternalInput")
    sgnT = nc.dram_tensor("sgnT", [128, H], bf16, kind="ExternalInput")
    WlQ = nc.dram_tensor("WlQ", [128, H], bf16, kind="ExternalInput")
    WrQ = nc.dram_tensor("WrQ", [128, H], bf16, kind="ExternalInput")
    invatt4B = nc.dram_tensor("invatt4B", [128, HC], f32, kind="ExternalInput")
    biasB = nc.dram_tensor("biasB", [128, 128], f32, kind="ExternalInput")
    lngB = nc.dram_tensor("lngB", [128, 128], f32, kind="ExternalInput")
    lnbB = nc.dram_tensor("lnbB", [128, 128], f32, kind="ExternalInput")
    ohtidd = nc.dram_tensor("ohtidd", [BLOCKS, BLK, (K + 2) * BLK], fp8e5,
                            kind="ExternalInput")
    ohd = nc.dram_tensor("ohd", [BLOCKS, BLK, cap], fp8e5,
                         kind="ExternalInput")
    sidxd = nc.dram_tensor("sidxd", [BLOCKS, 128, cap // 16], i16,
                           kind="ExternalInput")

    xnew = nc.dram_tensor("xnew", [NODES_PER_CORE, 128], f32,
                          kind="ExternalOutput")
    vsd = nc.dram_tensor("vsd", [BLOCKS, 128], f32, kind="ExternalOutput")

    with tile.TileContext(nc) as tc, ExitStack() as ctx:
        consts = ctx.enter_context(tc.tile_pool(name="consts", bufs=1))
        lhsp = ctx.enter_context(tc.tile_pool(name="lhs", bufs=3))
        densep = ctx.enter_context(tc.tile_pool(name="dense", bufs=4))
        g8p = ctx.enter_context(tc.tile_pool(name="g8", bufs=2))
        otp = ctx.enter_context(tc.tile_pool(name="ot", bufs=3))
        ohp = ctx.enter_context(tc.tile_pool(name="ohf", bufs=3))
        sxp = ctx.enter_context(tc.tile_pool(name="sx", bufs=3))
        sp = ctx.enter_context(tc.tile_pool(name="s", bufs=3))
        ezp = ctx.enter_context(tc.tile_pool(name="ez", bufs=3))
        egp = ctx.enter_context(tc.tile_pool(name="eg", bufs=4))
        lnp = ctx.enter_context(tc.tile_pool(name="ln", bufs=2))
        lgp = ctx.enter_context(tc.tile_pool(name="lg", bufs=4))
        outp = ctx.enter_context(tc.tile_pool(name="out", bufs=2))
        dramp = ctx.enter_context(tc.tile_pool(name="dram", bufs=1,
                                               space="DRAM"))
        pup = ctx.enter_context(tc.tile_pool(name="pu", bufs=2, space="PSUM"))
        pzp = ctx.enter_context(tc.tile_pool(name="pz", bufs=1, space="PSUM"))
        pdenp = ctx.enter_context(tc.tile_pool(name="pden", bufs=1,
                                               space="PSUM"))
        paggp = ctx.enter_context(tc.tile_pool(name="pagg", bufs=2,
                                               space="PSUM"))

        def load_const(src_ap, shape, dtype, name):
            t = consts.tile(shape, dtype, tag=name)
            nc.sync.dma_start(t[:], src_ap)
            return t

        wl_sb = load_const(WlS[:], [128, HC], bf16, "wl")
        wr_sb = load_const(WrS[:], [128, HC], bf16, "wr")
        sgn_sb = load_const(sgnT[:], [128, H], bf16, "sgn")
        wlq_sb = load_const(WlQ[:], [128, H], bf16, "wlq")
        wrq_sb = load_const(WrQ[:], [128, H], bf16, "wrq")
        invatt_sb = load_const(invatt4B[:], [128, HC], f32, "invatt")
        if not bias_zero:
            blB_sb = load_const(blB[:], [128, HC], f32, "blB")
            brB_sb = load_const(brB[:], [128, HC], f32, "brB")
            biasB_sb = load_const(biasB[:], [128, 128], f32, "biasB")
        if not ln_triv:
            lngB_sb = load_const(lngB[:], [128, 128], f32, "lngB")
            lnbB_sb = load_const(lnbB[:], [128, 128], f32, "lnbB")

        xl_dram = dramp.tile([N_ROWS, HC], f16)

        alphaP = consts.tile([128, 1], f32, tag="alphaP")
        nc.vector.memset(alphaP[:], NEG_SLOPE)
        epsP = consts.tile([128, 1], f32, tag="epsP")
        nc.vector.memset(epsP[:], LN_EPS)

        blk_loads = {}

        def prefetch_loads(b):
            six = sxp.tile([128, cap // 16], i16, tag="sidx")
            nc.sync.dma_start(six[:], sidxd[b])
            ot = otp.tile([128, K + 2, BLK], fp8e5, tag="ot")
            nc.sync.dma_start(
                ot[:], ohtidd[b].rearrange("p (k e) -> p k e", e=BLK))
            ohb = ohp.tile([128, cap], fp8e5, tag="oh")
            nc.sync.dma_start(ohb[:], ohd[b])
            blk_loads[b] = (six, ot, ohb)

        st = {"g8": {}, "agg": {}, "den": {}, "s": {}, "u": {},
              "ezf": {}, "ezb": {}, "eg": {}, "zp": {}, "p06": None}

        POS = K // 2   # xr'' lives at the middle slot of g8

        def prefetch_gather(b):
            six, ot, ohb = blk_loads[b]
            g8 = g8p.tile([128, K + 1, HC], f16, tag="g8")
            st["g8"][b] = g8
            # chunk k -> tile slot k (k < POS) or k+1 (k >= POS)
            # splits sized under the 1024-descriptor SWDGE FIFO carveout
            ranges = []
            for lo, hi in ((0, POS), (POS, K)):
                n_sp = -(-(hi - lo) * BLK // 1008)
                bnds = [lo + (hi - lo) * i // n_sp for i in range(n_sp + 1)]
                ranges += list(zip(bnds[:-1], bnds[1:]))
            for k0, k1 in ranges:
                s0 = k0 if k1 <= POS else k0 + 1
                n_idx = (k1 - k0) * BLK
                nc.gpsimd.dma_gather(
                    out_ap=g8[:, s0:s0 + (k1 - k0), :], in_ap=xl_dram[:],
                    idxs_ap=six[:, k0 * BLK // 16:k1 * BLK // 16],
                    num_idxs=n_idx, num_idxs_reg=n_idx, elem_size=HC,
                    single_packet=False)

        # ---- dense: xl'' for all nodes -> DRAM fp16 ----
        xT_sb = consts.tile([128, N_ROWS], bf16, tag="xT")
        for q in range(4):
            c0 = (N_ROWS // 4 // 128) * 128 * q
            c1 = N_ROWS if q == 3 else (N_ROWS // 4 // 128) * 128 * (q + 1)
            nc.sync.dma_start(xT_sb[:, c0:c1], xT[:, c0:c1])
        prefetch_loads(0)
        prefetch_loads(1)
        GB = 4
        for t0 in range(0, N_TILES, GB):
            n_sub = min(GB, N_TILES - t0)
            xs4 = densep.tile([128, GB, HC], f16, tag="xs4")
            for j in range(n_sub):
                t_i = t0 + j
                xt_ap = xT_sb[:, t_i * 128:(t_i + 1) * 128]
                ps4 = pup.tile([128, 2, HC], f32, tag="uT")
                ps = ps4[:, 0, :]
                nc.tensor.matmul(ps, xt_ap, wl_sb[:], start=True, stop=True)
                if bias_zero:
                    if t_i % 2 == 0:
                        nc.vector.tensor_scalar(out=xs4[:, j, :HC], in0=ps,
                                                scalar1=1.0, scalar2=None,
                                                op0=ALU.mult)
                    else:
                        nc.scalar.activation(xs4[:, j, :HC], ps, AF.Copy)
                else:
                    nc.vector.tensor_tensor(out=xs4[:, j, :HC], in0=ps,
                                            in1=blB_sb[:], op=ALU.add)
            nc.scalar.dma_start(
                xl_dram[t0 * 128:(t0 + n_sub) * 128, :].rearrange(
                    "(t p) c -> p t c", p=128),
                xs4[:, :n_sub, :])

        # ---- edge phase: flattened per-chunk software pipeline ----
        prefetch_gather(0)
        P = K // 2

        def blk_state(b):
            if b not in st["agg"]:
                g8 = st["g8"][b]
                # xr'' for this block -> fp16 slot 0
                lhs = lhsp.tile([128, 128], bf16, tag="lhs")
                nc.sync.dma_start(lhs[:], xlocT[:, b * 128:(b + 1) * 128])
                psr4 = pup.tile([128, 2, HC], f32, tag="uT")
                psr = psr4[:, 0, :]
                nc.tensor.matmul(psr, lhs[:], wr_sb[:], start=True,
                                 stop=True)
                if bias_zero:
                    nc.vector.tensor_scalar(out=g8[:, POS, :HC], in0=psr,
                                            scalar1=1.0, scalar2=None,
                                            op0=ALU.mult)
                else:
                    nc.vector.tensor_tensor(out=g8[:, POS, :HC], in0=psr,
                                            in1=brB_sb[:], op=ALU.add)
                agg_t = paggp.tile([128, HC], f32, tag="agg")
                den_t = pdenp.tile([128, 4], f32, tag="den")
                st["agg"][b] = agg_t
                st["den"][b] = den_t

        def front(b, k):
            if k == 0:
                blk_state(b)
            g8 = st["g8"][b]
            ge5 = g8[:].bitcast(fp8e5)     # [128, K+1, 2*HC]
            ot = blk_loads[b][1]
            m, half = divmod(k, 2)
            if half == 0:
                uT_t = pup.tile([128, 2, HC], f32, tag="uT")
                st["u"][(b, m)] = uT_t
            uT = st["u"][(b, m)]
            gslot = k if k < POS else k + 1
            for h in range(H):
                lo = 2 * h * 128 + 1
                out_ap = uT[:, half, h * 128:(h + 1) * 128]
                if k < POS:
                    # lhs halves (g, xr) pair with rhs halves (id, oht)
                    lhs_ap = ge5[:, gslot:POS + 1:POS - gslot, lo:lo + 255:2]
                    rhs_ap = ot[:, 0:k + 2:k + 1, :]
                else:
                    # lhs halves (xr, g) pair with rhs halves (oht, id)
                    lhs_ap = ge5[:, POS:gslot + 1:gslot - POS, lo:lo + 255:2]
                    rhs_ap = ot[:, k + 1:K + 2:K - k, :]
                nc.tensor.matmul(out_ap, lhs_ap, rhs_ap, start=True,
                                 stop=True, perf_mode=PM.DoubleRow)
            if k == 16 and b + 2 < BLOCKS:
                prefetch_loads(b + 2)
            if k == 20 and b + 1 < BLOCKS:
                prefetch_gather(b + 1)

        def sabs(b, m):
            uT = st["u"].pop((b, m))
            s_ = sp.tile([128, 2, HC], bf16, tag="s")
            nc.scalar.activation(s_[:], uT[:], AF.Prelu, alpha=alphaP[:])
            st["s"][(b, m)] = s_

        def zmm(b, k):
            g, slot = divmod(k, GROUP)
            if slot == 0:
                zP_t = pzp.tile([128, 4 * GROUP], f32, tag="zP")
                st["zp"][b] = zP_t
            zP = st["zp"][b]
            m, half = divmod(k, 2)
            s_ = st["s"][(b, m)]
            for h in range(H):
                nc.tensor.matmul(zP[:, slot * 4 + h:slot * 4 + h + 1],
                                 s_[:, half, h * 128:(h + 1) * 128],
                                 sgn_sb[:, h:h + 1],
                                 start=True, stop=True)
            if half == 1:
                st["s"].pop((b, m))
            if slot == GROUP - 1 or k == K - 1:
                n4 = (slot + 1) * 4
                ezf = ezp.tile([128, 4 * GROUP], f32, tag="ezf")
                nc.scalar.activation(ezf[:, :n4], zP[:, :n4], AF.Exp)
                ezb = ezp.tile([128, 4 * GROUP], f16, tag="ezb")
                nc.vector.tensor_scalar(out=ezb[:, :n4], in0=ezf[:, :n4],
                                        scalar1=EZ_CLAMP, scalar2=None,
                                        op0=ALU.min)
                st["ezf"][(b, g)] = ezf
                st["ezb"][(b, g)] = ezb

        def eg_stage(b, k):
            g8 = st["g8"][b]
            m, half = divmod(k, 2)
            if half == 0:
                egt_t = egp.tile([128, 2, HC], bf16, tag="eg")
                st["eg"][(b, m)] = egt_t
            egt = st["eg"][(b, m)]
            grp, slot = divmod(k, GROUP)
            ezb = st["ezb"][(b, grp)]
            gslot = k if k < POS else k + 1
            in0 = g8[:, gslot, :HC].rearrange("p (h c) -> p h c", h=H)
            in1 = ezb[:, slot * 4:slot * 4 + 4].unsqueeze(-1).broadcast_to(
                [128, H, 128])
            out = egt[:, half, :].rearrange("p (h c) -> p h c", h=H)
            eng = EG_PAT[k % len(EG_PAT)]
            if eng == "DVE":
                nc.vector.tensor_tensor(out=out, in0=in0, in1=in1,
                                        op=ALU.mult)
            else:
                nc.gpsimd.tensor_tensor(out=out, in0=in0, in1=in1,
                                        op=ALU.mult)

        def aggden(b, m):
            agg = st["agg"][b]
            den = st["den"][b]
            ohb = blk_loads[b][2]
            egt = st["eg"].pop((b, m))
            for half in (0, 1):
                k = 2 * m + half
                grp, slot = divmod(k, GROUP)
                ezb = st["ezb"][(b, grp)]
                nc.tensor.matmul(agg[:], ohb[:, k * BLK:(k + 1) * BLK],
                                 egt[:, half, :],
                                 start=(k == 0), stop=(k == K - 1))
                nc.tensor.matmul(den[:], ohb[:, k * BLK:(k + 1) * BLK],
                                 ezb[:, slot * 4:slot * 4 + 4],
                                 start=(k == 0), stop=(k == K - 1))
            if m == P - 1:
                tail(b)

        def tail(b):
            agg = st["agg"].pop(b)
            den = st["den"].pop(b)
            st["g8"].pop(b, None)
            st["zp"].pop(b, None)
            blk_loads.pop(b, None)
            for key in [x for x in st["ezf"] if x[0] == b]:
                st["ezf"].pop(key)
            for key in [x for x in st["ezb"] if x[0] == b]:
                st["ezb"].pop(key)
            rden = lgp.tile([128, 4], f32, tag="rden")
            nc.vector.reciprocal(rden[:], den[:])
            # tq = agg * rden (head-broadcast) on DVE
            tq = lnp.tile([128, HC], f32, tag="tq")
            nc.vector.tensor_tensor(
                out=tq[:].rearrange("p (h c) -> p h c", h=H),
                in0=agg[:].rearrange("p (h c) -> p h c", h=H),
                in1=rden[:].unsqueeze(-1).broadcast_to([128, H, 128]),
                op=ALU.mult)
            tq2 = lnp.tile([128, HC], f32, tag="tq2")
            nc.gpsimd.tensor_tensor(out=tq2[:], in0=tq[:], in1=invatt_sb[:],
                                    op=ALU.mult)
            hm = outp.tile([128, 128], f32, tag="hm")
            nc.vector.tensor_reduce(
                out=hm[:], in_=tq2[:].rearrange("p (h c) -> p c h", h=H),
                axis=X, op=ALU.add)
            xt = outp.tile([128, 128], f32, tag="xres")
            nc.sync.dma_start(xt[:], xloc[b * 128:(b + 1) * 128, :])
            if bias_zero:
                r2 = outp.tile([128, 128], f32, tag="r2")
                nc.gpsimd.tensor_tensor(out=r2[:], in0=hm[:], in1=xt[:],
                                        op=ALU.add)
            else:
                r1 = outp.tile([128, 128], f32, tag="r1")
                nc.vector.tensor_tensor(out=r1[:], in0=hm[:],
                                        in1=biasB_sb[:], op=ALU.add)
                r2 = outp.tile([128, 128], f32, tag="r2")
                nc.gpsimd.tensor_tensor(out=r2[:], in0=r1[:], in1=xt[:],
                                        op=ALU.add)
            mu = lgp.tile([128, 1], f32, tag="mu")
            nc.vector.tensor_reduce(out=mu[:], in_=r2[:], axis=X, op=ALU.add)
            mun = lgp.tile([128, 1], f32, tag="mun")
            nc.vector.tensor_scalar_mul(mun[:], mu[:], 1.0 / 128)
            xc = outp.tile([128, 128], f32, tag="xc")
            nc.vector.tensor_scalar(out=xc[:], in0=r2[:], scalar1=mun[:],
                                    scalar2=None, op0=ALU.subtract)
            junk = outp.tile([128, 128], f32, tag="junk")
            vs = lgp.tile([128, 1], f32, tag="vs")
            nc.vector.scalar_tensor_tensor(
                out=junk[:], in0=r2[:], scalar=mun[:], in1=xc[:],
                op0=ALU.subtract, op1=ALU.mult, accum_out=vs[:])
            nc.sync.dma_start(vsd[b], vs[:, 0])
            xout = outp.tile([128, 128], f32, tag="xout")
            if ln_triv:
                # relu only; the 1/sqrt(var+eps) row scale is applied on
                # the host (relu commutes with a positive per-row scale)
                nc.scalar.activation(xout[:], xc[:], AF.Relu)
            else:
                lt = lgp.tile([128, 1], f32, tag="lt")
                nc.scalar.activation(lt[:], vs[:], AF.Ln, bias=epsP[:],
                                     scale=1.0 / 128)
                rstd = lgp.tile([128, 1], f32, tag="rstd")
                nc.scalar.activation(rstd[:], lt[:], AF.Exp, scale=-0.5)
                xn = outp.tile([128, 128], f32, tag="xn")
                nc.vector.tensor_scalar(out=xn[:], in0=xc[:],
                                        scalar1=rstd[:],
                                        scalar2=None, op0=ALU.mult)
                xg = outp.tile([128, 128], f32, tag="xg")
                nc.vector.tensor_tensor(out=xg[:], in0=xn[:], in1=lngB_sb[:],
                                        op=ALU.mult)
                xgb = outp.tile([128, 128], f32, tag="xgb")
                nc.vector.tensor_tensor(out=xgb[:], in0=xg[:],
                                        in1=lnbB_sb[:], op=ALU.add)
                nc.scalar.activation(xout[:], xgb[:], AF.Relu)
            nc.sync.dma_start(xnew[b * 128:(b + 1) * 128, :], xout[:])

        chunks = [(b, k) for b in range(BLOCKS) for k in range(K)]
        NCH = len(chunks)
        for i in range(NCH + OFF_AGG + 1):
            if i < NCH:
                front(*chunks[i])
            if 0 <= i - 1 < NCH and chunks[i - 1][1] % 2 == 1:
                b, k = chunks[i - 1]
                sabs(b, k // 2)
            if 0 <= i - 2 < NCH:
                zmm(*chunks[i - 2])
            if 0 <= i - OFF_EG < NCH:
                eg_stage(*chunks[i - OFF_EG])
            if 0 <= i - OFF_AGG < NCH and chunks[i - OFF_AGG][1] % 2 == 0:
                b, k = chunks[i - OFF_AGG]
                aggden(b, k // 2)

    nc.compile()
    return nc


def kernel(x, edge_index, Wl, bl, Wr, br, att, bias, ln_g, ln_b):
    x = np.asarray(x, np.float32)
    edge_index = np.asarray(edge_index)
    Wl = np.asarray(Wl, np.float32); bl = np.asarray(bl, np.float32)
    Wr = np.asarray(Wr, np.float32); br = np.asarray(br, np.float32)
    att = np.asarray(att, np.float32); bias = np.asarray(bias, np.float32)
    ln_g = np.asarray(ln_g, np.float32); ln_b = np.asarray(ln_b, np.float32)

    K, src_arr, dpos_arr = _prep_edges(edge_index)
    sidx, ohtid, ohflat = _build_ship_arrays(K, src_arr, dpos_arr)

    bias_zero = not (np.any(bias) or np.any(bl) or np.any(br))
    ln_triv = (np.all(ln_g == 1.0) and not np.any(ln_b))
    key = (K, bias_zero, ln_triv)
    if key not in _NC_CACHE:
        _NC_CACHE[key] = _build_nc(K, bias_zero, ln_triv)
    nc = _NC_CACHE[key]

    aatt = np.maximum(np.abs(att), 1e-30)
    sgn = np.sign(att).astype(np.float32)
    sgn[sgn == 0] = 1.0

    LAST_RESULTS.clear()
    cur = x
    for l in range(L):
        a_flat = aatt[l].reshape(HC)
        WlSf = Wl[l] * a_flat[None, :]
        WrSf = Wr[l] * a_flat[None, :]
        WlS = WlSf.astype(BF16)
        WrS = WrSf.astype(BF16)
        sgnT = np.ascontiguousarray(
            (sgn[l] * SGN_COMP).T).astype(BF16)   # [C, H]
        WlQ = np.zeros((D, H), np.float32)
        WrQ = np.zeros((D, H), np.float32)

        xpad = np.zeros((N_ROWS, 128), np.float32)
        xpad[:N_NODES] = cur
        xT = np.ascontiguousarray(xpad.T.astype(BF16))
        xloc_full = np.zeros((N_PAD, 128), np.float32)
        xloc_full[:N_NODES] = cur

        common = {
            "xT": xT, "WlS": WlS, "WrS": WrS,
            "blB": _bcast(bl[l] * a_flat), "brB": _bcast(br[l] * a_flat),
            "sgnT": sgnT, "WlQ": WlQ.astype(BF16), "WrQ": WrQ.astype(BF16),
            "invatt4B": _bcast(0.25 / a_flat),
            "biasB": _bcast(bias[l]), "lngB": _bcast(ln_g[l]),
            "lnbB": _bcast(ln_b[l]),
        }
        in_maps = []
        for c in range(N_CORES):
            xl_c = np.ascontiguousarray(
                xloc_full[c * NODES_PER_CORE:(c + 1) * NODES_PER_CORE])
            in_maps.append({
                **common,
                "xloc": xl_c,
                "xlocT": np.ascontiguousarray(xl_c.T.astype(BF16)),
                "ohtidd": ohtid[c], "ohd": ohflat[c], "sidxd": sidx[c],
            })

        res = run_bass_kernel_spmd(nc, in_maps, core_ids=list(range(N_CORES)))
        LAST_RESULTS.append(res)
        nxt = np.concatenate([res.results[c]["xnew"] for c in range(N_CORES)],
                             axis=0)
        if ln_triv:
            vsall = np.concatenate(
                [np.asarray(res.results[c]["vsd"]).reshape(-1)
                 for c in range(N_CORES)])
            rstd = 1.0 / np.sqrt(vsall / 128.0 + LN_EPS)
            nxt = nxt * rstd[:, None]
        cur = np.ascontiguousarray(nxt[:N_NODES]).astype(np.float32)

    return cur.astype(np.float32)


# revision 23
# speedup vs baseline: 1.2695x; 1.0422x over previous
"""GATv2 (2 layers, H=4, C=128, head-mean) on 8 TRN2 cores, dst-partitioned.

v4 design (per layer, one SPMD launch of a shared single-layer NEFF):
  dense: xl'' = x @ (Wl .* |att|) for ALL nodes -> fp16 to DRAM (the value
         path uses full fp16; the logit path uses each fp16's HIGH BYTE,
         which is exactly the e5m2-truncated value, via a stride-2 bitcast
         AP - no separate fp8 conversion pass). xr'' per local block ->
         fp16 slot 0 of the gather tile.
  per 128-edge chunk (edges sorted by dst, 10 blocks x 128 dst/core):
    - SWDGE row-gather of fp16 xl''[src] into slot k+1 of g; trailing
      pad-edge indices are negative so the DGE skips their descriptors
    - front: per head one fp8e5 DoubleRow matmul on the high-byte view:
      uT[c,e] = sum_d xr_e5[d,c]*oht[d,e] + g_e5[e,c]
    - prelu on ACT, batched per chunk-pair ([128,1024] per op); the
      e5m2 truncation bias is compensated by scaling sgn by 1.09
    - logits: per head a [128,1] matmul  z[e] = sum_c s'T[c,e]*sgn[c]
    - exp batched over 8 chunks (one ACT op per [128,32] group); ezb =
      min(ez, 3e4) fp16 guards fp16/inf for skipped pad edges
    - eg[e,hc] = g * ez via ONE tensor_tensor with a broadcast AP per
      chunk (DVE/POOL round-robin) -> bf16
    - agg += oh^T @ eg, den += oh^T @ ezb (fp8e5 one-hots)
  All stages run in a flattened cross-block per-chunk software pipeline.
  tail per block: alpha = agg*rden, .*(0.25/|att|), head-sum, +residual,
  then mean-center and relu ON DEVICE; the 1/sqrt(var+eps) row scale is
  applied on the HOST (relu commutes with the positive scale), so the ACT
  engine only ever runs {Prelu, Exp, Relu} -> zero act-table swaps.
Host: edge sorting, fp8e5 one-hot (oht|id) and oh arrays, wrapped gather
idxs, per-row LayerNorm scale between layers.
"""

from contextlib import ExitStack

import numpy as np
import ml_dtypes

import concourse.bacc as bacc
import concourse.tile as tile
from concourse import mybir
from concourse.bass_utils import run_bass_kernel_spmd

BF16 = ml_dtypes.bfloat16
FP8E5 = ml_dtypes.float8_e5m2
F16 = np.float16

N_NODES = 10000
D = 128
H = 4
C = 128
HC = H * C
NEG_SLOPE = 0.2
LN_EPS = 1e-5
L = 2
SGN_COMP = 1.09     # compensates the e5m2 truncation shrink of logits
EZ_CLAMP = 30000.0  # keeps exp() of stale pad-edge logits finite in fp16

N_CORES = 8
NODES_PER_CORE = 1280
BLOCKS = 10
BLK = 128
N_PAD = N_CORES * NODES_PER_CORE    # 10240
N_ROWS = 10112                      # 79*128
N_TILES = N_ROWS // 128

_NC_CACHE = {}
LAST_RESULTS = []   # BassKernelResults per launch (for test harness)

# engine round-robin patterns (tuned against the cost model)
EG_PAT = ["DVE", "POOL"]
SABS_PAT = ["ACT", "ACT", "ACT", "ACT", "ACT", "ACT", "DVE"]
GROUP = 8             # chunks per exp batch
OFF_EG = GROUP + 2    # eg stage offset (must trail the group exp)
OFF_AGG = OFF_EG + 2


def _prep_edges(edge_index):
    src = np.concatenate([np.asarray(edge_index[0], np.int64),
                          np.arange(N_NODES, dtype=np.int64)])
    dst = np.concatenate([np.asarray(edge_index[1], np.int64),
                          np.arange(N_NODES, dtype=np.int64)])
    pad_nodes = np.arange(N_NODES, N_PAD, dtype=np.int64)
    src = np.concatenate([src, np.zeros_like(pad_nodes)])
    dst = np.concatenate([dst, pad_nodes])

    order = np.argsort(dst, kind="stable")
    src = src[order]
    dst = dst[order]

    blk_of_edge = dst // BLK
    n_blocks_total = N_PAD // BLK
    counts = np.bincount(blk_of_edge, minlength=n_blocks_total)
    K = int(np.max((counts + BLK - 1) // BLK))
    K += K % 2  # even, so we can process chunk pairs

    cap = K * BLK
    src_arr = np.zeros((n_blocks_total, cap), np.int32)
    dpos_arr = np.full((n_blocks_total, cap), -1, np.int32)
    block_starts = np.zeros(n_blocks_total + 1, np.int64)
    np.cumsum(counts, out=block_starts[1:])
    slot = np.arange(len(dst)) - block_starts[blk_of_edge]
    src_arr[blk_of_edge, slot] = src.astype(np.int32)
    dpos_arr[blk_of_edge, slot] = (dst - blk_of_edge * BLK).astype(np.int32)

    return (K, src_arr.reshape(N_CORES, BLOCKS, cap),
            dpos_arr.reshape(N_CORES, BLOCKS, cap))


def _build_ship_arrays(K, src_arr, dpos_arr):
    cap = K * BLK
    # wrapped gather indices: idx i lives at [i % 16, i // 16]; the 16-row
    # pattern is tiled 8x along partitions (one copy per SWDGE Q7 core).
    # pad slots are -1: the DGE skips trailing negative indices.
    s = src_arr.reshape(N_CORES, BLOCKS, cap // 16, 16)
    s = np.swapaxes(s, 2, 3)                                  # [c,b,16,cap/16]
    sidx = np.tile(s, (1, 1, 8, 1)).astype(np.int16)          # [c,b,128,cap/16]

    # fp8e5 one-hots:
    # ohtid [c,b, d(128), (K+2)*128]: slot 0 = identity, slot 1+k =
    #   oht chunk k (col (1+k)*128+e -> 1 iff dst(chunk k, e) == d),
    #   slot K+1 = identity.  Identities at both ends let the DoubleRow
    #   matmul pair (g, xr@middle) with (id, oht) using positive AP steps.
    # ohflat [c,b, e(128), cap]: col k*128+d -> oh[e, k, d]
    ohtid = np.zeros((N_CORES, BLOCKS, BLK, (K + 2) * BLK), FP8E5)
    ohflat = np.zeros((N_CORES, BLOCKS, BLK, cap), FP8E5)
    cc, bb, ss = np.nonzero(dpos_arr >= 0)
    kk = (ss // BLK).astype(np.int64)
    ee = (ss % BLK).astype(np.int64)
    dd = dpos_arr[cc, bb, ss].astype(np.int64)
    ohtid[cc, bb, dd, (kk + 1) * BLK + ee] = 1
    ohflat[cc, bb, ee, kk * BLK + dd] = 1
    i = np.arange(BLK)
    ohtid[:, :, i, i] = 1
    ohtid[:, :, i, (K + 1) * BLK + i] = 1
    return (np.ascontiguousarray(sidx), np.ascontiguousarray(ohtid),
            np.ascontiguousarray(ohflat))


def _bcast(v, rows=128):
    v = np.asarray(v, np.float32)
    return np.ascontiguousarray(np.broadcast_to(v[None, :], (rows, v.shape[0])))


def _build_nc(K, bias_zero, ln_triv):
    nc = bacc.Bacc("TRN2", target_bir_lowering=False, debug=False,
                   num_devices=N_CORES)
    f32, bf16, i16 = mybir.dt.float32, mybir.dt.bfloat16, mybir.dt.int16
    f16 = mybir.dt.float16
    fp8e5 = mybir.dt.float8e5
    AF = mybir.ActivationFunctionType
    ALU = mybir.AluOpType
    PM = mybir.MatmulPerfMode
    X = mybir.AxisListType.X
    cap = K * BLK

    xT = nc.dram_tensor("xT", [128, N_ROWS], bf16, kind="ExternalInput")
    xlocT = nc.dram_tensor("xlocT", [128, NODES_PER_CORE], bf16,
                           kind="ExternalInput")
    xloc = nc.dram_tensor("xloc", [NODES_PER_CORE, 128], f32,
                          kind="ExternalInput")
    WlS = nc.dram_tensor("WlS", [128, HC], bf16, kind="ExternalInput")
    WrS = nc.dram_tensor("WrS", [128, HC], bf16, kind="ExternalInput")
    blB = nc.dram_tensor("blB", [128, HC], f32, kind="ExternalInput")
    brB = nc.dram_tensor("brB", [128, HC], f32, kind="ExternalInput")
    sgnT = nc.dram_tensor("sgnT", [128, H], bf16, kind="ExternalInput")
    WlQ = nc.dram_tensor("WlQ", [128, H], bf16, kind="ExternalInput")
    WrQ = nc.dram_tensor("WrQ", [128, H], bf16, kind="ExternalInput")
    invatt4B = nc.dram_tensor("invatt4B", [128, HC], f32, kind="ExternalInput")
    biasB = nc.dram_tensor("biasB", [128, 128], f32, kind="ExternalInput")
    lngB = nc.dram_tensor("lngB", [128, 128], f32, kind="ExternalInput")
    lnbB = nc.dram_tensor("lnbB", [128, 128], f32, kind="ExternalInput")
    ohtidd = nc.dram_tensor("ohtidd", [BLOCKS, BLK, (K + 2) * BLK], fp8e5,
                            kind="ExternalInput")
    ohd = nc.dram_tensor("ohd", [BLOCKS, BLK, cap], fp8e5,
                         kind="ExternalInput")
    sidxd = nc.dram_tensor("sidxd", [BLOCKS, 128, cap // 16], i16,
                           kind="ExternalInput")

    xnew = nc.dram_tensor("xnew", [NODES_PER_CORE, 128], f32,
                          kind="ExternalOutput")
    vsd = nc.dram_tensor("vsd", [BLOCKS, 128], f32, kind="ExternalOutput")

    with tile.TileContext(nc) as tc, ExitStack() as ctx:
        consts = ctx.enter_context(tc.tile_pool(name="consts", bufs=1))
        lhsp = ctx.enter_context(tc.tile_pool(name="lhs", bufs=3))
        densep = ctx.enter_context(tc.tile_pool(name="dense", bufs=4))
        g8p = ctx.enter_context(tc.tile_pool(name="g8", bufs=2))
        otp = ctx.enter_context(tc.tile_pool(name="ot", bufs=3))
        ohp = ctx.enter_context(tc.tile_pool(name="ohf", bufs=3))
        sxp = ctx.enter_context(tc.tile_pool(name="sx", bufs=3))
        sp = ctx.enter_context(tc.tile_pool(name="s", bufs=3))
        ezp = ctx.enter_context(tc.tile_pool(name="ez", bufs=3))
        egp = ctx.enter_context(tc.tile_pool(name="eg", bufs=4))
        lnp = ctx.enter_context(tc.tile_pool(name="ln", bufs=2))
        lgp = ctx.enter_context(tc.tile_pool(name="lg", bufs=4))
        outp = ctx.enter_context(tc.tile_pool(name="out", bufs=2))
        dramp = ctx.enter_context(tc.tile_pool(name="dram", bufs=1,
                                               space="DRAM"))
        pup = ctx.enter_context(tc.tile_pool(name="pu", bufs=2, space="PSUM"))
        pzp = ctx.enter_context(tc.tile_pool(name="pz", bufs=1, space="PSUM"))
        pdenp = ctx.enter_context(tc.tile_pool(name="pden", bufs=1,
                                               space="PSUM"))
        paggp = ctx.enter_context(tc.tile_pool(name="pagg", bufs=2,
                                               space="PSUM"))

        def load_const(src_ap, shape, dtype, name):
            t = consts.tile(shape, dtype, tag=name)
            nc.sync.dma_start(t[:], src_ap)
            return t

        wl_sb = load_const(WlS[:], [128, HC], bf16, "wl")
        wr_sb = load_const(WrS[:], [128, HC], bf16, "wr")
        sgn_sb = load_const(sgnT[:], [128, H], bf16, "sgn")
        wlq_sb = load_const(WlQ[:], [128, H], bf16, "wlq")
        wrq_sb = load_const(WrQ[:], [128, H], bf16, "wrq")
        invatt_sb = load_const(invatt4B[:], [128, HC], f32, "invatt")
        if not bias_zero:
            blB_sb = load_const(blB[:], [128, HC], f32, "blB")
            brB_sb = load_const(brB[:], [128, HC], f32, "brB")
            biasB_sb = load_const(biasB[:], [128, 128], f32, "biasB")
        if not ln_triv:
            lngB_sb = load_const(lngB[:], [128, 128], f32, "lngB")
            lnbB_sb = load_const(lnbB[:], [128, 128], f32, "lnbB")

        xl_dram = dramp.tile([N_ROWS, HC], f16)

        alphaP = consts.tile([128, 1], f32, tag="alphaP")
        nc.vector.memset(alphaP[:], NEG_SLOPE)
        epsP = consts.tile([128, 1], f32, tag="epsP")
        nc.vector.memset(epsP[:], LN_EPS)

        blk_loads = {}

        def prefetch_loads(b):
            six = sxp.tile([128, cap // 16], i16, tag="sidx")
            nc.sync.dma_start(six[:], sidxd[b])
            ot = otp.tile([128, K + 2, BLK], fp8e5, tag="ot")
            nc.sync.dma_start(
                ot[:], ohtidd[b].rearrange("p (k e) -> p k e", e=BLK))
            ohb = ohp.tile([128, cap], fp8e5, tag="oh")
            nc.sync.dma_start(ohb[:], ohd[b])
            blk_loads[b] = (six, ot, ohb)

        st = {"g8": {}, "agg": {}, "den": {}, "s": {}, "u": {},
              "ezf": {}, "ezb": {}, "eg": {}, "zp": {}, "p06": None}

        POS = K // 2   # xr'' lives at the middle slot of g8

        def prefetch_gather(b):
            six, ot, ohb = blk_loads[b]
            g8 = g8p.tile([128, K + 1, HC], f16, tag="g8")
            st["g8"][b] = g8
            # chunk k -> tile slot k (k < POS) or k+1 (k >= POS)
            # splits sized under the 1024-descriptor SWDGE FIFO carveout
            ranges = []
            for lo, hi in ((0, POS), (POS, K)):
                n_sp = -(-(hi - lo) * BLK // 1008)
                bnds = [lo + (hi - lo) * i // n_sp for i in range(n_sp + 1)]
                ranges += list(zip(bnds[:-1], bnds[1:]))
            for k0, k1 in ranges:
                s0 = k0 if k1 <= POS else k0 + 1
                n_idx = (k1 - k0) * BLK
                nc.gpsimd.dma_gather(
                    out_ap=g8[:, s0:s0 + (k1 - k0), :], in_ap=xl_dram[:],
                    idxs_ap=six[:, k0 * BLK // 16:k1 * BLK // 16],
                    num_idxs=n_idx, num_idxs_reg=n_idx, elem_size=HC,
                    single_packet=False)

        # ---- dense: xl'' for all nodes -> DRAM fp16 ----
        xT_sb = consts.tile([128, N_ROWS], bf16, tag="xT")
        for q in range(4):
            c0 = (N_ROWS // 4 // 128) * 128 * q
            c1 = N_ROWS if q == 3 else (N_ROWS // 4 // 128) * 128 * (q + 1)
            nc.sync.dma_start(xT_sb[:, c0:c1], xT[:, c0:c1])
        prefetch_loads(0)
        prefetch_loads(1)
        GB = 4
        for t0 in range(0, N_TILES, GB):
            n_sub = min(GB, N_TILES - t0)
            xs4 = densep.tile([128, GB, HC], f16, tag="xs4")
            for j in range(n_sub):
                t_i = t0 + j
                xt_ap = xT_sb[:, t_i * 128:(t_i + 1) * 128]
                ps4 = pup.tile([128, 2, HC], f32, tag="uT")
                ps = ps4[:, 0, :]
                nc.tensor.matmul(ps, xt_ap, wl_sb[:], start=True, stop=True)
                if bias_zero:
                    if t_i % 2 == 0:
                        nc.vector.tensor_scalar(out=xs4[:, j, :HC], in0=ps,
                                                scalar1=1.0, scalar2=None,
                                                op0=ALU.mult)
                    else:
                        nc.scalar.activation(xs4[:, j, :HC], ps, AF.Copy)
                else:
                    nc.vector.tensor_tensor(out=xs4[:, j, :HC], in0=ps,
                                            in1=blB_sb[:], op=ALU.add)
            nc.scalar.dma_start(
                xl_dram[t0 * 128:(t0 + n_sub) * 128, :].rearrange(
                    "(t p) c -> p t c", p=128),
                xs4[:, :n_sub, :])

        # ---- edge phase: flattened per-chunk software pipeline ----
        prefetch_gather(0)
        P = K // 2

        def blk_state(b):
            if b not in st["agg"]:
                g8 = st["g8"][b]
                # xr'' for this block -> fp16 slot 0
                lhs = lhsp.tile([128, 128], bf16, tag="lhs")
                nc.sync.dma_start(lhs[:], xlocT[:, b * 128:(b + 1) * 128])
                psr4 = pup.tile([128, 2, HC], f32, tag="uT")
                psr = psr4[:, 0, :]
                nc.tensor.matmul(psr, lhs[:], wr_sb[:], start=True,
                                 stop=True)
                if bias_zero:
                    nc.vector.tensor_scalar(out=g8[:, POS, :HC], in0=psr,
                                            scalar1=1.0, scalar2=None,
                                            op0=ALU.mult)
                else:
                    nc.vector.tensor_tensor(out=g8[:, POS, :HC], in0=psr,
                                            in1=brB_sb[:], op=ALU.add)
                agg_t = paggp.tile([128, HC], f32, tag="agg")
                den_t = pdenp.tile([128, 4], f32, tag="den")
                st["agg"][b] = agg_t
                st["den"][b] = den_t

        def front(b, k):
            if k == 0:
                blk_state(b)
            g8 = st["g8"][b]
            ge5 = g8[:].bitcast(fp8e5)     # [128, K+1, 2*HC]
            ot = blk_loads[b][1]
            m, half = divmod(k, 2)
            if half == 0:
                uT_t = pup.tile([128, 2, HC], f32, tag="uT")
                st["u"][(b, m)] = uT_t
            uT = st["u"][(b, m)]
            gslot = k if k < POS else k + 1
            for h in range(H):
                lo = 2 * h * 128 + 1
                out_ap = uT[:, half, h * 128:(h + 1) * 128]
                if k < POS:
                    # lhs halves (g, xr) pair with rhs halves (id, oht)
                    lhs_ap = ge5[:, gslot:POS + 1:POS - gslot, lo:lo + 255:2]
                    rhs_ap = ot[:, 0:k + 2:k + 1, :]
                else:
                    # lhs halves (xr, g) pair with rhs halves (oht, id)
                    lhs_ap = ge5[:, POS:gslot + 1:gslot - POS, lo:lo + 255:2]
                    rhs_ap = ot[:, k + 1:K + 2:K - k, :]
                nc.tensor.matmul(out_ap, lhs_ap, rhs_ap, start=True,
                                 stop=True, perf_mode=PM.DoubleRow)
            if k == 16 and b + 2 < BLOCKS:
                prefetch_loads(b + 2)
            if k == 20 and b + 1 < BLOCKS:
                prefetch_gather(b + 1)

        def sabs(b, m):
            uT = st["u"].pop((b, m))
            s_ = sp.tile([128, 2, HC], bf16, tag="s")
            nc.scalar.activation(s_[:], uT[:], AF.Prelu, alpha=alphaP[:])
            st["s"][(b, m)] = s_

        def zmm(b, k):
            g, slot = divmod(k, GROUP)
            if slot == 0:
                zP_t = pzp.tile([128, 4 * GROUP], f32, tag="zP")
                st["zp"][b] = zP_t
            zP = st["zp"][b]
            m, half = divmod(k, 2)
            s_ = st["s"][(b, m)]
            for h in range(H):
                nc.tensor.matmul(zP[:, slot * 4 + h:slot * 4 + h + 1],
                                 s_[:, half, h * 128:(h + 1) * 128],
                                 sgn_sb[:, h:h + 1],
                                 start=True, stop=True)
            if half == 1:
                st["s"].pop((b, m))
            if slot == GROUP - 1 or k == K - 1:
                n4 = (slot + 1) * 4
                ezf = ezp.tile([128, 4 * GROUP], f32, tag="ezf")
                nc.scalar.activation(ezf[:, :n4], zP[:, :n4], AF.Exp)
                ezb = ezp.tile([128, 4 * GROUP], f16, tag="ezb")
                nc.vector.tensor_scalar(out=ezb[:, :n4], in0=ezf[:, :n4],
                                        scalar1=EZ_CLAMP, scalar2=None,
                                        op0=ALU.min)
                st["ezf"][(b, g)] = ezf
                st["ezb"][(b, g)] = ezb

        def eg_stage(b, k):
            g8 = st["g8"][b]
            m, half = divmod(k, 2)
            if half == 0:
                egt_t = egp.tile([128, 2, HC], bf16, tag="eg")
                st["eg"][(b, m)] = egt_t
            egt = st["eg"][(b, m)]
            grp, slot = divmod(k, GROUP)
            ezb = st["ezb"][(b, grp)]
            gslot = k if k < POS else k + 1
            in0 = g8[:, gslot, :HC].rearrange("p (h c) -> p h c", h=H)
            in1 = ezb[:, slot * 4:slot * 4 + 4].unsqueeze(-1).broadcast_to(
                [128, H, 128])
            out = egt[:, half, :].rearrange("p (h c) -> p h c", h=H)
            eng = EG_PAT[k % len(EG_PAT)]
            if eng == "DVE":
                nc.vector.tensor_tensor(out=out, in0=in0, in1=in1,
                                        op=ALU.mult)
            else:
                nc.gpsimd.tensor_tensor(out=out, in0=in0, in1=in1,
                                        op=ALU.mult)

        def aggden(b, m):
            agg = st["agg"][b]
            den = st["den"][b]
            ohb = blk_loads[b][2]
            egt = st["eg"].pop((b, m))
            for half in (0, 1):
                k = 2 * m + half
                grp, slot = divmod(k, GROUP)
                ezb = st["ezb"][(b, grp)]
                nc.tensor.matmul(agg[:], ohb[:, k * BLK:(k + 1) * BLK],
                                 egt[:, half, :],
                                 start=(k == 0), stop=(k == K - 1))
                nc.tensor.matmul(den[:], ohb[:, k * BLK:(k + 1) * BLK],
                                 ezb[:, slot * 4:slot * 4 + 4],
                                 start=(k == 0), stop=(k == K - 1))
            if m == P - 1:
                tail(b)

        def tail(b):
            agg = st["agg"].pop(b)
            den = st["den"].pop(b)
            st["g8"].pop(b, None)
            st["zp"].pop(b, None)
            blk_loads.pop(b, None)
            for key in [x for x in st["ezf"] if x[0] == b]:
                st["ezf"].pop(key)
            for key in [x for x in st["ezb"] if x[0] == b]:
                st["ezb"].pop(key)
            rden = lgp.tile([128, 4], f32, tag="rden")
            nc.vector.reciprocal(rden[:], den[:])
            # tq = agg * rden (head-broadcast) on DVE
            tq = lnp.tile([128, HC], f32, tag="tq")
            nc.vector.tensor_tensor(
                out=tq[:].rearrange("p (h c) -> p h c", h=H),
                in0=agg[:].rearrange("p (h c) -> p h c", h=H),
                in1=rden[:].unsqueeze(-1).broadcast_to([128, H, 128]),
                op=ALU.mult)
            tq2 = lnp.tile([128, HC], f32, tag="tq2")
            nc.gpsimd.tensor_tensor(out=tq2[:], in0=tq[:], in1=invatt_sb[:],
                                    op=ALU.mult)
            hm = outp.tile([128, 128], f32, tag="hm")
            nc.vector.tensor_reduce(
                out=hm[:], in_=tq2[:].rearrange("p (h c) -> p c h", h=H),
                axis=X, op=ALU.add)
            xt = outp.tile([128, 128], f32, tag="xres")
            nc.sync.dma_start(xt[:], xloc[b * 128:(b + 1) * 128, :])
            if bias_zero:
                r2 = outp.tile([128, 128], f32, tag="r2")
                nc.gpsimd.tensor_tensor(out=r2[:], in0=hm[:], in1=xt[:],
                                        op=ALU.add)
            else:
                r1 = outp.tile([128, 128], f32, tag="r1")
                nc.vector.tensor_tensor(out=r1[:], in0=hm[:],
                                        in1=biasB_sb[:], op=ALU.add)
                r2 = outp.tile([128, 128], f32, tag="r2")
                nc.gpsimd.tensor_tensor(out=r2[:], in0=r1[:], in1=xt[:],
                                        op=ALU.add)
            mu = lgp.tile([128, 1], f32, tag="mu")
            nc.vector.tensor_reduce(out=mu[:], in_=r2[:], axis=X, op=ALU.add)
            mun = lgp.tile([128, 1], f32, tag="mun")
            nc.vector.tensor_scalar_mul(mun[:], mu[:], 1.0 / 128)
            xc = outp.tile([128, 128], f32, tag="xc")
            nc.vector.tensor_scalar(out=xc[:], in0=r2[:], scalar1=mun[:],
                                    scalar2=None, op0=ALU.subtract)
            junk = outp.tile([128, 128], f32, tag="junk")
            vs = lgp.tile([128, 1], f32, tag="vs")
            nc.vector.scalar_tensor_tensor(
                out=junk[:], in0=r2[:], scalar=mun[:], in1=xc[:],
                op0=ALU.subtract, op1=ALU.mult, accum_out=vs[:])
            nc.sync.dma_start(vsd[b], vs[:, 0])
            xout = outp.tile([128, 128], f32, tag="xout")
            if ln_triv:
                # relu only; the 1/sqrt(var+eps) row scale is applied on
                # the host (relu commutes with a positive per-row scale)
                nc.scalar.activation(xout[:], xc[:], AF.Relu)
            else:
                lt = lgp.tile([128, 1], f32, tag="lt")
                nc.scalar.activation(lt[:], vs[:], AF.Ln, bias=epsP[:],
                                     scale=1.0 / 128)
                rstd = lgp.tile([128, 1], f32, tag="rstd")
                nc.scalar.activation(rstd[:], lt[:], AF.Exp, scale=-0.5)
                xn = outp.tile([128, 128], f32, tag="xn")
                nc.vector.tensor_scalar(out=xn[:], in0=xc[:],
                                        scalar1=rstd[:],
                                        scalar2=None, op0=ALU.mult)
                xg = outp.tile([128, 128], f32, tag="xg")
                nc.vector.tensor_tensor(out=xg[:], in0=xn[:], in1=lngB_sb[:],
                                        op=ALU.mult)
                xgb = outp.tile([128, 128], f32, tag="xgb")
                nc.vector.tensor_tensor(out=xgb[:], in0=xg[:],
                                        in1=lnbB_sb[:], op=ALU.add)
                nc.scalar.activation(xout[:], xgb[:], AF.Relu)
            nc.sync.dma_start(xnew[b * 128:(b + 1) * 128, :], xout[:])

        chunks = [(b, k) for b in range(BLOCKS) for k in range(K)]
        NCH = len(chunks)
        for i in range(NCH + OFF_AGG + 1):
            if i < NCH:
                front(*chunks[i])
            if 0 <= i - 1 < NCH and chunks[i - 1][1] % 2 == 1:
                b, k = chunks[i - 1]
                sabs(b, k // 2)
            if 0 <= i - 2 < NCH:
                zmm(*chunks[i - 2])
            if 0 <= i - OFF_EG < NCH:
                eg_stage(*chunks[i - OFF_EG])
            if 0 <= i - OFF_AGG < NCH and chunks[i - OFF_AGG][1] % 2 == 0:
                b, k = chunks[i - OFF_AGG]
                aggden(b, k // 2)

    nc.compile()
    return nc


def kernel(x, edge_index, Wl, bl, Wr, br, att, bias, ln_g, ln_b):
    x = np.asarray(x, np.float32)
    edge_index = np.asarray(edge_index)
    Wl = np.asarray(Wl, np.float32); bl = np.asarray(bl, np.float32)
    Wr = np.asarray(Wr, np.float32); br = np.asarray(br, np.float32)
    att = np.asarray(att, np.float32); bias = np.asarray(bias, np.float32)
    ln_g = np.asarray(ln_g, np.float32); ln_b = np.asarray(ln_b, np.float32)

    K, src_arr, dpos_arr = _prep_edges(edge_index)
    sidx, ohtid, ohflat = _build_ship_arrays(K, src_arr, dpos_arr)

    bias_zero = not (np.any(bias) or np.any(bl) or np.any(br))
    ln_triv = (np.all(ln_g == 1.0) and not np.any(ln_b))
    key = (K, bias_zero, ln_triv)
    if key not in _NC_CACHE:
        _NC_CACHE[key] = _build_nc(K, bias_zero, ln_triv)
    nc = _NC_CACHE[key]

    aatt = np.maximum(np.abs(att), 1e-30)
    sgn = np.sign(att).astype(np.float32)
    sgn[sgn == 0] = 1.0

    LAST_RESULTS.clear()
    cur = x
    for l in range(L):
        a_flat = aatt[l].reshape(HC)
        WlSf = Wl[l] * a_flat[None, :]
        WrSf = Wr[l] * a_flat[None, :]
        WlS = WlSf.astype(BF16)
        WrS = WrSf.astype(BF16)
        sgnT = np.ascontiguousarray(
            (sgn[l] * SGN_COMP).T).astype(BF16)   # [C, H]
        WlQ = np.zeros((D, H), np.float32)
        WrQ = np.zeros((D, H), np.float32)

        xpad = np.zeros((N_ROWS, 128), np.float32)
        xpad[:N_NODES] = cur
        xT = np.ascontiguousarray(xpad.T.astype(BF16))
        xloc_full = np.zeros((N_PAD, 128), np.float32)
        xloc_full[:N_NODES] = cur

        common = {
            "xT": xT, "WlS": WlS, "WrS": WrS,
            "blB": _bcast(bl[l] * a_flat), "brB": _bcast(br[l] * a_flat),
            "sgnT": sgnT, "WlQ": WlQ.astype(BF16), "WrQ": WrQ.astype(BF16),
            "invatt4B": _bcast(0.25 / a_flat),
            "biasB": _bcast(bias[l]), "lngB": _bcast(ln_g[l]),
            "lnbB": _bcast(ln_b[l]),
        }
        in_maps = []
        for c in range(N_CORES):
            xl_c = np.ascontiguousarray(
                xloc_full[c * NODES_PER_CORE:(c + 1) * NODES_PER_CORE])
            in_maps.append({
                **common,
                "xloc": xl_c,
                "xlocT": np.ascontiguousarray(xl_c.T.astype(BF16)),
                "ohtidd": ohtid[c], "ohd": ohflat[c], "sidxd": sidx[c],
            })

        res = run_bass_kernel_spmd(nc, in_maps, core_ids=list(range(N_CORES)))
        LAST_RESULTS.append(res)
        nxt = np.concatenate([res.results[c]["xnew"] for c in range(N_CORES)],
                             axis=0)
        if ln_triv:
            vsall = np.concatenate(
                [np.asarray(res.results[c]["vsd"]).reshape(-1)
                 for c in range(N_CORES)])
            rstd = 1.0 / np.sqrt(vsall / 128.0 + LN_EPS)
            nxt = nxt * rstd[:, None]
        cur = np.ascontiguousarray(nxt[:N_NODES]).astype(np.float32)

    return cur.astype(np.float32)


# revision 25
# speedup vs baseline: 1.2880x; 1.0146x over previous
"""GATv2 (2 layers, H=4, C=128, head-mean) on 8 TRN2 cores, dst-partitioned.

v4 design (per layer, one SPMD launch of a shared single-layer NEFF):
  dense: xl'' = x @ (Wl .* |att|) for ALL nodes -> fp16 to DRAM (the value
         path uses full fp16; the logit path uses each fp16's HIGH BYTE,
         which is exactly the e5m2-truncated value, via a stride-2 bitcast
         AP - no separate fp8 conversion pass). xr'' per local block ->
         fp16 slot 0 of the gather tile.
  per 128-edge chunk (edges sorted by dst, 10 blocks x 128 dst/core):
    - SWDGE row-gather of fp16 xl''[src] into slot k+1 of g; trailing
      pad-edge indices are negative so the DGE skips their descriptors
    - front: per head one fp8e5 DoubleRow matmul on the high-byte view:
      uT[c,e] = sum_d xr_e5[d,c]*oht[d,e] + g_e5[e,c]
    - prelu on ACT, batched per chunk-pair ([128,1024] per op); the
      e5m2 truncation bias is compensated by scaling sgn by 1.09
    - logits: per head a [128,1] matmul  z[e] = sum_c s'T[c,e]*sgn[c]
    - exp batched over 8 chunks (one ACT op per [128,32] group); ezb =
      min(ez, 3e4) fp16 guards fp16/inf for skipped pad edges
    - eg[e,hc] = g * ez via ONE tensor_tensor with a broadcast AP per
      chunk (DVE/POOL round-robin) -> bf16
    - agg += oh^T @ eg, den += oh^T @ ezb (fp8e5 one-hots)
  All stages run in a flattened cross-block per-chunk software pipeline.
  tail per block: alpha = agg*rden, .*(0.25/|att|), head-sum, +residual,
  then mean-center and relu ON DEVICE; the 1/sqrt(var+eps) row scale is
  applied on the HOST (relu commutes with the positive scale), so the ACT
  engine only ever runs {Prelu, Exp, Relu} -> zero act-table swaps.
Host: edge sorting, fp8e5 one-hot (oht|id) and oh arrays, wrapped gather
idxs, per-row LayerNorm scale between layers.
"""

from contextlib import ExitStack

import numpy as np
import ml_dtypes

import concourse.bacc as bacc
import concourse.tile as tile
from concourse import mybir
from concourse.bass_utils import run_bass_kernel_spmd

BF16 = ml_dtypes.bfloat16
FP8E5 = ml_dtypes.float8_e5m2
F16 = np.float16

N_NODES = 10000
D = 128
H = 4
C = 128
HC = H * C
NEG_SLOPE = 0.2
LN_EPS = 1e-5
L = 2
SGN_COMP = 1.09     # compensates the e5m2 truncation shrink of logits
EZ_CLAMP = 30000.0  # keeps exp() of stale pad-edge logits finite in fp16

N_CORES = 8
NODES_PER_CORE = 1280
BLOCKS = 10
BLK = 128
N_PAD = N_CORES * NODES_PER_CORE    # 10240
N_ROWS = 10112                      # 79*128
N_TILES = N_ROWS // 128

_NC_CACHE = {}
LAST_RESULTS = []   # BassKernelResults per launch (for test harness)

# engine round-robin patterns (tuned against the cost model)
EG_PAT = ["DVE", "POOL"]
SABS_PAT = ["ACT", "ACT", "ACT", "ACT", "ACT", "ACT", "DVE"]
GROUP = 8             # chunks per exp batch
OFF_EG = GROUP + 2    # eg stage offset (must trail the group exp)
OFF_AGG = OFF_EG + 2


def _prep_edges(edge_index):
    src = np.concatenate([np.asarray(edge_index[0], np.int64),
                          np.arange(N_NODES, dtype=np.int64)])
    dst = np.concatenate([np.asarray(edge_index[1], np.int64),
                          np.arange(N_NODES, dtype=np.int64)])
    pad_nodes = np.arange(N_NODES, N_PAD, dtype=np.int64)
    src = np.concatenate([src, np.zeros_like(pad_nodes)])
    dst = np.concatenate([dst, pad_nodes])

    order = np.argsort(dst, kind="stable")
    src = src[order]
    dst = dst[order]

    blk_of_edge = dst // BLK
    n_blocks_total = N_PAD // BLK
    counts = np.bincount(blk_of_edge, minlength=n_blocks_total)
    K = int(np.max((counts + BLK - 1) // BLK))
    K += K % 2  # even, so we can process chunk pairs

    cap = K * BLK
    src_arr = np.zeros((n_blocks_total, cap), np.int32)
    dpos_arr = np.full((n_blocks_total, cap), -1, np.int32)
    block_starts = np.zeros(n_blocks_total + 1, np.int64)
    np.cumsum(counts, out=block_starts[1:])
    slot = np.arange(len(dst)) - block_starts[blk_of_edge]
    src_arr[blk_of_edge, slot] = src.astype(np.int32)
    dpos_arr[blk_of_edge, slot] = (dst - blk_of_edge * BLK).astype(np.int32)

    return (K, src_arr.reshape(N_CORES, BLOCKS, cap),
            dpos_arr.reshape(N_CORES, BLOCKS, cap))


def _build_ship_arrays(K, src_arr, dpos_arr):
    cap = K * BLK
    # wrapped gather indices: idx i lives at [i % 16, i // 16]; the 16-row
    # pattern is tiled 8x along partitions (one copy per SWDGE Q7 core).
    # pad slots are -1: the DGE skips trailing negative indices.
    s = src_arr.reshape(N_CORES, BLOCKS, cap // 16, 16)
    s = np.swapaxes(s, 2, 3)                                  # [c,b,16,cap/16]
    sidx = np.tile(s, (1, 1, 8, 1)).astype(np.int16)          # [c,b,128,cap/16]

    # fp8e5 one-hots:
    # ohtid [c,b, d(128), (K+2)*128]: slot 0 = identity, slot 1+k =
    #   oht chunk k (col (1+k)*128+e -> 1 iff dst(chunk k, e) == d),
    #   slot K+1 = identity.  Identities at both ends let the DoubleRow
    #   matmul pair (g, xr@middle) with (id, oht) using positive AP steps.
    # ohflat [c,b, e(128), cap]: col k*128+d -> oh[e, k, d]
    ohtid = np.zeros((N_CORES, BLOCKS, BLK, (K + 2) * BLK), FP8E5)
    ohflat = np.zeros((N_CORES, BLOCKS, BLK, cap), FP8E5)
    cc, bb, ss = np.nonzero(dpos_arr >= 0)
    kk = (ss // BLK).astype(np.int64)
    ee = (ss % BLK).astype(np.int64)
    dd = dpos_arr[cc, bb, ss].astype(np.int64)
    ohtid[cc, bb, dd, (kk + 1) * BLK + ee] = 1
    ohflat[cc, bb, ee, kk * BLK + dd] = 1
    i = np.arange(BLK)
    ohtid[:, :, i, i] = 1
    ohtid[:, :, i, (K + 1) * BLK + i] = 1
    return (np.ascontiguousarray(sidx), np.ascontiguousarray(ohtid),
            np.ascontiguousarray(ohflat))


def _bcast(v, rows=128):
    v = np.asarray(v, np.float32)
    return np.ascontiguousarray(np.broadcast_to(v[None, :], (rows, v.shape[0])))


def _build_nc(K, bias_zero, ln_triv):
    nc = bacc.Bacc("TRN2", target_bir_lowering=False, debug=False,
                   num_devices=N_CORES)
    f32, bf16, i16 = mybir.dt.float32, mybir.dt.bfloat16, mybir.dt.int16
    f16 = mybir.dt.float16
    fp8e5 = mybir.dt.float8e5
    AF = mybir.ActivationFunctionType
    ALU = mybir.AluOpType
    PM = mybir.MatmulPerfMode
    X = mybir.AxisListType.X
    cap = K * BLK

    xT = nc.dram_tensor("xT", [128, N_ROWS], bf16, kind="ExternalInput")
    xlocT = nc.dram_tensor("xlocT", [128, NODES_PER_CORE], bf16,
                           kind="ExternalInput")
    xloc = nc.dram_tensor("xloc", [NODES_PER_CORE, 128], f32,
                          kind="ExternalInput")
    WlS = nc.dram_tensor("WlS", [128, HC], bf16, kind="ExternalInput")
    WrS = nc.dram_tensor("WrS", [128, HC], bf16, kind="ExternalInput")
    blB = nc.dram_tensor("blB", [128, HC], f32, kind="ExternalInput")
    brB = nc.dram_tensor("brB", [128, HC], f32, kind="ExternalInput")
    sgnT = nc.dram_tensor("sgnT", [128, H], bf16, kind="ExternalInput")
    WlQ = nc.dram_tensor("WlQ", [128, H], bf16, kind="ExternalInput")
    WrQ = nc.dram_tensor("WrQ", [128, H], bf16, kind="ExternalInput")
    invatt4B = nc.dram_tensor("invatt4B", [128, HC], f32, kind="ExternalInput")
    biasB = nc.dram_tensor("biasB", [128, 128], f32, kind="ExternalInput")
    lngB = nc.dram_tensor("lngB", [128, 128], f32, kind="ExternalInput")
    lnbB = nc.dram_tensor("lnbB", [128, 128], f32, kind="ExternalInput")
    ohtidd = nc.dram_tensor("ohtidd", [BLOCKS, BLK, (K + 2) * BLK], fp8e5,
                            kind="ExternalInput")
    ohd = nc.dram_tensor("ohd", [BLOCKS, BLK, cap], fp8e5,
                         kind="ExternalInput")
    sidxd = nc.dram_tensor("sidxd", [BLOCKS, 128, cap // 16], i16,
                           kind="ExternalInput")

    xnew = nc.dram_tensor("xnew", [NODES_PER_CORE, 128], f32,
                          kind="ExternalOutput")
    vsd = nc.dram_tensor("vsd", [BLOCKS, 128], f32, kind="ExternalOutput")

    with tile.TileContext(nc) as tc, ExitStack() as ctx:
        consts = ctx.enter_context(tc.tile_pool(name="consts", bufs=1))
        lhsp = ctx.enter_context(tc.tile_pool(name="lhs", bufs=3))
        densep = ctx.enter_context(tc.tile_pool(name="dense", bufs=4))
        g8p = ctx.enter_context(tc.tile_pool(name="g8", bufs=2))
        otp = ctx.enter_context(tc.tile_pool(name="ot", bufs=3))
        ohp = ctx.enter_context(tc.tile_pool(name="ohf", bufs=3))
        sxp = ctx.enter_context(tc.tile_pool(name="sx", bufs=3))
        sp = ctx.enter_context(tc.tile_pool(name="s", bufs=3))
        ezp = ctx.enter_context(tc.tile_pool(name="ez", bufs=3))
        egp = ctx.enter_context(tc.tile_pool(name="eg", bufs=4))
        lnp = ctx.enter_context(tc.tile_pool(name="ln", bufs=2))
        lgp = ctx.enter_context(tc.tile_pool(name="lg", bufs=4))
        outp = ctx.enter_context(tc.tile_pool(name="out", bufs=2))
        dramp = ctx.enter_context(tc.tile_pool(name="dram", bufs=1,
                                               space="DRAM"))
        pup = ctx.enter_context(tc.tile_pool(name="pu", bufs=2, space="PSUM"))
        pzp = ctx.enter_context(tc.tile_pool(name="pz", bufs=1, space="PSUM"))
        pdenp = ctx.enter_context(tc.tile_pool(name="pden", bufs=1,
                                               space="PSUM"))
        paggp = ctx.enter_context(tc.tile_pool(name="pagg", bufs=2,
                                               space="PSUM"))

        def load_const(src_ap, shape, dtype, name):
            t = consts.tile(shape, dtype, tag=name)
            nc.sync.dma_start(t[:], src_ap)
            return t

        wl_sb = load_const(WlS[:], [128, HC], bf16, "wl")
        wr_sb = load_const(WrS[:], [128, HC], bf16, "wr")
        sgn_sb = load_const(sgnT[:], [128, H], bf16, "sgn")
        wlq_sb = load_const(WlQ[:], [128, H], bf16, "wlq")
        wrq_sb = load_const(WrQ[:], [128, H], bf16, "wrq")
        invatt_sb = load_const(invatt4B[:], [128, HC], f32, "invatt")
        if not bias_zero:
            blB_sb = load_const(blB[:], [128, HC], f32, "blB")
            brB_sb = load_const(brB[:], [128, HC], f32, "brB")
            biasB_sb = load_const(biasB[:], [128, 128], f32, "biasB")
        if not ln_triv:
            lngB_sb = load_const(lngB[:], [128, 128], f32, "lngB")
            lnbB_sb = load_const(lnbB[:], [128, 128], f32, "lnbB")

        xl_dram = dramp.tile([N_ROWS, HC], f16)

        alphaP = consts.tile([128, 1], f32, tag="alphaP")
        nc.vector.memset(alphaP[:], NEG_SLOPE)
        epsP = consts.tile([128, 1], f32, tag="epsP")
        nc.vector.memset(epsP[:], LN_EPS)

        blk_loads = {}

        def prefetch_loads(b):
            six = sxp.tile([128, cap // 16], i16, tag="sidx")
            nc.sync.dma_start(six[:], sidxd[b])
            ot = otp.tile([128, K + 2, BLK], fp8e5, tag="ot")
            nc.sync.dma_start(
                ot[:], ohtidd[b].rearrange("p (k e) -> p k e", e=BLK))
            ohb = ohp.tile([128, cap], fp8e5, tag="oh")
            nc.sync.dma_start(ohb[:], ohd[b])
            blk_loads[b] = (six, ot, ohb)

        st = {"g8": {}, "agg": {}, "den": {}, "s": {}, "u": {},
              "ezf": {}, "ezb": {}, "eg": {}, "zp": {}, "p06": None}

        POS = K // 2   # xr'' lives at the middle slot of g8

        def prefetch_gather(b):
            six, ot, ohb = blk_loads[b]
            g8 = g8p.tile([128, K + 1, HC], f16, tag="g8")
            st["g8"][b] = g8
            # chunk k -> tile slot k (k < POS) or k+1 (k >= POS)
            # splits sized under the 1024-descriptor SWDGE FIFO carveout
            ranges = []
            for lo, hi in ((0, POS), (POS, K)):
                n_sp = -(-(hi - lo) * BLK // 1008)
                bnds = [lo + (hi - lo) * i // n_sp for i in range(n_sp + 1)]
                ranges += list(zip(bnds[:-1], bnds[1:]))
            for k0, k1 in ranges:
                s0 = k0 if k1 <= POS else k0 + 1
                n_idx = (k1 - k0) * BLK
                nc.gpsimd.dma_gather(
                    out_ap=g8[:, s0:s0 + (k1 - k0), :], in_ap=xl_dram[:],
                    idxs_ap=six[:, k0 * BLK // 16:k1 * BLK // 16],
                    num_idxs=n_idx, num_idxs_reg=n_idx, elem_size=HC,
                    single_packet=False)

        # ---- dense: xl'' for all nodes -> DRAM fp16 ----
        xT_sb = consts.tile([128, N_ROWS], bf16, tag="xT")
        for q in range(4):
            c0 = (N_ROWS // 4 // 128) * 128 * q
            c1 = N_ROWS if q == 3 else (N_ROWS // 4 // 128) * 128 * (q + 1)
            nc.sync.dma_start(xT_sb[:, c0:c1], xT[:, c0:c1])
        prefetch_loads(0)
        prefetch_loads(1)
        GB = 4
        for t0 in range(0, N_TILES, GB):
            n_sub = min(GB, N_TILES - t0)
            xs4 = densep.tile([128, GB, HC], f16, tag="xs4")
            for j in range(n_sub):
                t_i = t0 + j
                xt_ap = xT_sb[:, t_i * 128:(t_i + 1) * 128]
                ps4 = pup.tile([128, 2, HC], f32, tag="uT")
                ps = ps4[:, 0, :]
                nc.tensor.matmul(ps, xt_ap, wl_sb[:], start=True, stop=True)
                if bias_zero:
                    if t_i % 2 == 0:
                        nc.vector.tensor_scalar(out=xs4[:, j, :HC], in0=ps,
                                                scalar1=1.0, scalar2=None,
                                                op0=ALU.mult)
                    else:
                        nc.scalar.activation(xs4[:, j, :HC], ps, AF.Copy)
                else:
                    nc.vector.tensor_tensor(out=xs4[:, j, :HC], in0=ps,
                                            in1=blB_sb[:], op=ALU.add)
            q_eng = nc.scalar if (t0 // GB) % 2 == 0 else nc.sync
            q_eng.dma_start(
                xl_dram[t0 * 128:(t0 + n_sub) * 128, :].rearrange(
                    "(t p) c -> p t c", p=128),
                xs4[:, :n_sub, :])

        # ---- edge phase: flattened per-chunk software pipeline ----
        prefetch_gather(0)
        P = K // 2

        def blk_state(b):
            if b not in st["agg"]:
                g8 = st["g8"][b]
                # xr'' for this block -> fp16 slot 0
                lhs = lhsp.tile([128, 128], bf16, tag="lhs")
                nc.sync.dma_start(lhs[:], xlocT[:, b * 128:(b + 1) * 128])
                psr4 = pup.tile([128, 2, HC], f32, tag="uT")
                psr = psr4[:, 0, :]
                nc.tensor.matmul(psr, lhs[:], wr_sb[:], start=True,
                                 stop=True)
                if bias_zero:
                    nc.vector.tensor_scalar(out=g8[:, POS, :HC], in0=psr,
                                            scalar1=1.0, scalar2=None,
                                            op0=ALU.mult)
                else:
                    nc.vector.tensor_tensor(out=g8[:, POS, :HC], in0=psr,
                                            in1=brB_sb[:], op=ALU.add)
                agg_t = paggp.tile([128, HC], f32, tag="agg")
                den_t = pdenp.tile([128, 4], f32, tag="den")
                st["agg"][b] = agg_t
                st["den"][b] = den_t

        def front(b, k):
            if k == 0:
                blk_state(b)
            g8 = st["g8"][b]
            ge5 = g8[:].bitcast(fp8e5)     # [128, K+1, 2*HC]
            ot = blk_loads[b][1]
            m, half = divmod(k, 2)
            if half == 0:
                uT_t = pup.tile([128, 2, HC], f32, tag="uT")
                st["u"][(b, m)] = uT_t
            uT = st["u"][(b, m)]
            gslot = k if k < POS else k + 1
            for h in range(H):
                lo = 2 * h * 128 + 1
                out_ap = uT[:, half, h * 128:(h + 1) * 128]
                if k < POS:
                    # lhs halves (g, xr) pair with rhs halves (id, oht)
                    lhs_ap = ge5[:, gslot:POS + 1:POS - gslot, lo:lo + 255:2]
                    rhs_ap = ot[:, 0:k + 2:k + 1, :]
                else:
                    # lhs halves (xr, g) pair with rhs halves (oht, id)
                    lhs_ap = ge5[:, POS:gslot + 1:gslot - POS, lo:lo + 255:2]
                    rhs_ap = ot[:, k + 1:K + 2:K - k, :]
                nc.tensor.matmul(out_ap, lhs_ap, rhs_ap, start=True,
                                 stop=True, perf_mode=PM.DoubleRow)
            if k == 16 and b + 2 < BLOCKS:
                prefetch_loads(b + 2)
            if k == 14 and b + 1 < BLOCKS:
                prefetch_gather(b + 1)

        def sabs(b, m):
            uT = st["u"].pop((b, m))
            s_ = sp.tile([128, 2, HC], bf16, tag="s")
            nc.scalar.activation(s_[:], uT[:], AF.Prelu, alpha=alphaP[:])
            st["s"][(b, m)] = s_

        def zmm(b, k):
            g, slot = divmod(k, GROUP)
            if slot == 0:
                zP_t = pzp.tile([128, 4 * GROUP], f32, tag="zP")
                st["zp"][b] = zP_t
            zP = st["zp"][b]
            m, half = divmod(k, 2)
            s_ = st["s"][(b, m)]
            for h in range(H):
                nc.tensor.matmul(zP[:, slot * 4 + h:slot * 4 + h + 1],
                                 s_[:, half, h * 128:(h + 1) * 128],
                                 sgn_sb[:, h:h + 1],
                                 start=True, stop=True)
            if half == 1:
                st["s"].pop((b, m))
            if slot == GROUP - 1 or k == K - 1:
                n4 = (slot + 1) * 4
                ezf = ezp.tile([128, 4 * GROUP], f32, tag="ezf")
                nc.scalar.activation(ezf[:, :n4], zP[:, :n4], AF.Exp)
                ezb = ezp.tile([128, 4 * GROUP], f16, tag="ezb")
                nc.vector.tensor_scalar(out=ezb[:, :n4], in0=ezf[:, :n4],
                                        scalar1=EZ_CLAMP, scalar2=None,
                                        op0=ALU.min)
                st["ezf"][(b, g)] = ezf
                st["ezb"][(b, g)] = ezb

        def eg_stage(b, m):
            # one 4D-AP multiply per chunk-pair (slots are always adjacent
            # because POS is even)
            g8 = st["g8"][b]
            egt_t = egp.tile([128, 2, HC], bf16, tag="eg")
            st["eg"][(b, m)] = egt_t
            k = 2 * m
            grp, slot = divmod(k, GROUP)
            ezb = st["ezb"][(b, grp)]
            gslot = k if k < POS else k + 1
            in0 = g8[:, gslot:gslot + 2, :HC].rearrange(
                "p j (h c) -> p j h c", h=H)
            in1 = ezb[:, slot * 4:slot * 4 + 8].rearrange(
                "p (j h) -> p j h", j=2).unsqueeze(-1).broadcast_to(
                [128, 2, H, 128])
            out = egt_t[:].rearrange("p j (h c) -> p j h c", h=H)
            eng = EG_PAT[m % len(EG_PAT)]
            if eng == "DVE":
                nc.vector.tensor_tensor(out=out, in0=in0, in1=in1,
                                        op=ALU.mult)
            else:
                nc.gpsimd.tensor_tensor(out=out, in0=in0, in1=in1,
                                        op=ALU.mult)

        def aggden(b, m):
            agg = st["agg"][b]
            den = st["den"][b]
            ohb = blk_loads[b][2]
            egt = st["eg"].pop((b, m))
            for half in (0, 1):
                k = 2 * m + half
                grp, slot = divmod(k, GROUP)
                ezb = st["ezb"][(b, grp)]
                nc.tensor.matmul(agg[:], ohb[:, k * BLK:(k + 1) * BLK],
                                 egt[:, half, :],
                                 start=(k == 0), stop=(k == K - 1))
                nc.tensor.matmul(den[:], ohb[:, k * BLK:(k + 1) * BLK],
                                 ezb[:, slot * 4:slot * 4 + 4],
                                 start=(k == 0), stop=(k == K - 1))
            if m == P - 1:
                tail(b)

        def tail(b):
            agg = st["agg"].pop(b)
            den = st["den"].pop(b)
            st["g8"].pop(b, None)
            st["zp"].pop(b, None)
            blk_loads.pop(b, None)
            for key in [x for x in st["ezf"] if x[0] == b]:
                st["ezf"].pop(key)
            for key in [x for x in st["ezb"] if x[0] == b]:
                st["ezb"].pop(key)
            rden = lgp.tile([128, 4], f32, tag="rden")
            nc.vector.reciprocal(rden[:], den[:])
            # tq = agg * rden (head-broadcast) on DVE
            tq = lnp.tile([128, HC], f32, tag="tq")
            nc.vector.tensor_tensor(
                out=tq[:].rearrange("p (h c) -> p h c", h=H),
                in0=agg[:].rearrange("p (h c) -> p h c", h=H),
                in1=rden[:].unsqueeze(-1).broadcast_to([128, H, 128]),
                op=ALU.mult)
            tq2 = lnp.tile([128, HC], f32, tag="tq2")
            nc.gpsimd.tensor_tensor(out=tq2[:], in0=tq[:], in1=invatt_sb[:],
                                    op=ALU.mult)
            hm = outp.tile([128, 128], f32, tag="hm")
            nc.vector.tensor_reduce(
                out=hm[:], in_=tq2[:].rearrange("p (h c) -> p c h", h=H),
                axis=X, op=ALU.add)
            xt = outp.tile([128, 128], f32, tag="xres")
            nc.sync.dma_start(xt[:], xloc[b * 128:(b + 1) * 128, :])
            if bias_zero:
                r2 = outp.tile([128, 128], f32, tag="r2")
                nc.gpsimd.tensor_tensor(out=r2[:], in0=hm[:], in1=xt[:],
                                        op=ALU.add)
            else:
                r1 = outp.tile([128, 128], f32, tag="r1")
                nc.vector.tensor_tensor(out=r1[:], in0=hm[:],
                                        in1=biasB_sb[:], op=ALU.add)
                r2 = outp.tile([128, 128], f32, tag="r2")
                nc.gpsimd.tensor_tensor(out=r2[:], in0=r1[:], in1=xt[:],
                                        op=ALU.add)
            mu = lgp.tile([128, 1], f32, tag="mu")
            nc.vector.tensor_reduce(out=mu[:], in_=r2[:], axis=X, op=ALU.add)
            mun = lgp.tile([128, 1], f32, tag="mun")
            nc.vector.tensor_scalar_mul(mun[:], mu[:], 1.0 / 128)
            xc = outp.tile([128, 128], f32, tag="xc")
            nc.vector.tensor_scalar(out=xc[:], in0=r2[:], scalar1=mun[:],
                                    scalar2=None, op0=ALU.subtract)
            junk = outp.tile([128, 128], f32, tag="junk")
            vs = lgp.tile([128, 1], f32, tag="vs")
            nc.vector.scalar_tensor_tensor(
                out=junk[:], in0=r2[:], scalar=mun[:], in1=xc[:],
                op0=ALU.subtract, op1=ALU.mult, accum_out=vs[:])
            nc.sync.dma_start(vsd[b], vs[:, 0])
            xout = outp.tile([128, 128], f32, tag="xout")
            if ln_triv:
                # relu only; the 1/sqrt(var+eps) row scale is applied on
                # the host (relu commutes with a positive per-row scale)
                nc.scalar.activation(xout[:], xc[:], AF.Relu)
            else:
                lt = lgp.tile([128, 1], f32, tag="lt")
                nc.scalar.activation(lt[:], vs[:], AF.Ln, bias=epsP[:],
                                     scale=1.0 / 128)
                rstd = lgp.tile([128, 1], f32, tag="rstd")
                nc.scalar.activation(rstd[:], lt[:], AF.Exp, scale=-0.5)
                xn = outp.tile([128, 128], f32, tag="xn")
                nc.vector.tensor_scalar(out=xn[:], in0=xc[:],
                                        scalar1=rstd[:],
                                        scalar2=None, op0=ALU.mult)
                xg = outp.tile([128, 128], f32, tag="xg")
                nc.vector.tensor_tensor(out=xg[:], in0=xn[:], in1=lngB_sb[:],
                                        op=ALU.mult)
                xgb = outp.tile([128, 128], f32, tag="xgb")
                nc.vector.tensor_tensor(out=xgb[:], in0=xg[:],
                                        in1=lnbB_sb[:], op=ALU.add)
                nc.scalar.activation(xout[:], xgb[:], AF.Relu)
            nc.sync.dma_start(xnew[b * 128:(b + 1) * 128, :], xout[:])

        chunks = [(b, k) for b in range(BLOCKS) for k in range(K)]
        NCH = len(chunks)
        for i in range(NCH + OFF_AGG + 1):
            if i < NCH:
                front(*chunks[i])
            if 0 <= i - 1 < NCH and chunks[i - 1][1] % 2 == 1:
                b, k = chunks[i - 1]
                sabs(b, k // 2)
            if 0 <= i - 2 < NCH:
                zmm(*chunks[i - 2])
            if 0 <= i - OFF_EG < NCH and chunks[i - OFF_EG][1] % 2 == 1:
                b, k = chunks[i - OFF_EG]
                eg_stage(b, k // 2)
            if 0 <= i - OFF_AGG < NCH and chunks[i - OFF_AGG][1] % 2 == 0:
                b, k = chunks[i - OFF_AGG]
                aggden(b, k // 2)

    nc.compile()
    return nc


def kernel(x, edge_index, Wl, bl, Wr, br, att, bias, ln_g, ln_b):
    x = np.asarray(x, np.float32)
    edge_index = np.asarray(edge_index)
    Wl = np.asarray(Wl, np.float32); bl = np.asarray(bl, np.float32)
    Wr = np.asarray(Wr, np.float32); br = np.asarray(br, np.float32)
    att = np.asarray(att, np.float32); bias = np.asarray(bias, np.float32)
    ln_g = np.asarray(ln_g, np.float32); ln_b = np.asarray(ln_b, np.float32)

    K, src_arr, dpos_arr = _prep_edges(edge_index)
    sidx, ohtid, ohflat = _build_ship_arrays(K, src_arr, dpos_arr)

    bias_zero = not (np.any(bias) or np.any(bl) or np.any(br))
    ln_triv = (np.all(ln_g == 1.0) and not np.any(ln_b))
    key = (K, bias_zero, ln_triv)
    if key not in _NC_CACHE:
        _NC_CACHE[key] = _build_nc(K, bias_zero, ln_triv)
    nc = _NC_CACHE[key]

    aatt = np.maximum(np.abs(att), 1e-30)
    sgn = np.sign(att).astype(np.float32)
    sgn[sgn == 0] = 1.0

    LAST_RESULTS.clear()
    cur = x
    for l in range(L):
        a_flat = aatt[l].reshape(HC)
        WlSf = Wl[l] * a_flat[None, :]
        WrSf = Wr[l] * a_flat[None, :]
        WlS = WlSf.astype(BF16)
        WrS = WrSf.astype(BF16)
        sgnT = np.ascontiguousarray(
            (sgn[l] * SGN_COMP).T).astype(BF16)   # [C, H]
        WlQ = np.zeros((D, H), np.float32)
        WrQ = np.zeros((D, H), np.float32)

        xpad = np.zeros((N_ROWS, 128), np.float32)
        xpad[:N_NODES] = cur
        xT = np.ascontiguousarray(xpad.T.astype(BF16))
        xloc_full = np.zeros((N_PAD, 128), np.float32)
        xloc_full[:N_NODES] = cur

        common = {
            "xT": xT, "WlS": WlS, "WrS": WrS,
            "blB": _bcast(bl[l] * a_flat), "brB": _bcast(br[l] * a_flat),
            "sgnT": sgnT, "WlQ": WlQ.astype(BF16), "WrQ": WrQ.astype(BF16),
            "invatt4B": _bcast(0.25 / a_flat),
            "biasB": _bcast(bias[l]), "lngB": _bcast(ln_g[l]),
            "lnbB": _bcast(ln_b[l]),
        }
        in_maps = []
        for c in range(N_CORES):
            xl_c = np.ascontiguousarray(
                xloc_full[c * NODES_PER_CORE:(c + 1) * NODES_PER_CORE])
            in_maps.append({
                **common,
                "xloc": xl_c,
                "xlocT": np.ascontiguousarray(xl_c.T.astype(BF16)),
                "ohtidd": ohtid[c], "ohd": ohflat[c], "sidxd": sidx[c],
            })

        res = run_bass_kernel_spmd(nc, in_maps, core_ids=list(range(N_CORES)))
        LAST_RESULTS.append(res)
        nxt = np.concatenate([res.results[c]["xnew"] for c in range(N_CORES)],
                             axis=0)
        if ln_triv:
            vsall = np.concatenate(
                [np.asarray(res.results[c]["vsd"]).reshape(-1)
                 for c in range(N_CORES)])
            rstd = 1.0 / np.sqrt(vsall / 128.0 + LN_EPS)
            nxt = nxt * rstd[:, None]
        cur = np.ascontiguousarray(nxt[:N_NODES]).astype(np.float32)

    return cur.astype(np.float32)


# revision 31
# speedup vs baseline: 1.3132x; 1.0196x over previous
"""GATv2 (2 layers, H=4, C=128, head-mean) on 8 TRN2 cores, dst-partitioned.

v4 design (per layer, one SPMD launch of a shared single-layer NEFF):
  dense: xl'' = x @ (Wl .* |att|) for ALL nodes -> fp16 to DRAM (the value
         path uses full fp16; the logit path uses each fp16's HIGH BYTE,
         which is exactly the e5m2-truncated value, via a stride-2 bitcast
         AP - no separate fp8 conversion pass). xr'' per local block ->
         fp16 slot 0 of the gather tile.
  per 128-edge chunk (edges sorted by dst, 10 blocks x 128 dst/core):
    - SWDGE row-gather of fp16 xl''[src] into slot k+1 of g; trailing
      pad-edge indices are negative so the DGE skips their descriptors
    - front: per head one fp8e5 DoubleRow matmul on the high-byte view:
      uT[c,e] = sum_d xr_e5[d,c]*oht[d,e] + g_e5[e,c]
    - prelu on ACT, batched per chunk-pair ([128,1024] per op); the
      e5m2 truncation bias is compensated by scaling sgn by 1.09
    - logits: per head a [128,1] matmul  z[e] = sum_c s'T[c,e]*sgn[c]
    - exp batched over 8 chunks (one ACT op per [128,32] group); ezb =
      min(ez, 3e4) fp16 guards fp16/inf for skipped pad edges
    - eg[e,hc] = g * ez via ONE tensor_tensor with a broadcast AP per
      chunk (DVE/POOL round-robin) -> bf16
    - agg += oh^T @ eg, den += oh^T @ ezb (fp8e5 one-hots)
  All stages run in a flattened cross-block per-chunk software pipeline.
  tail per block: alpha = agg*rden, .*(0.25/|att|), head-sum, +residual,
  then mean-center and relu ON DEVICE; the 1/sqrt(var+eps) row scale is
  applied on the HOST (relu commutes with the positive scale), so the ACT
  engine only ever runs {Prelu, Exp, Relu} -> zero act-table swaps.
Host: edge sorting, fp8e5 one-hot (oht|id) and oh arrays, wrapped gather
idxs, per-row LayerNorm scale between layers.
"""

from contextlib import ExitStack

import numpy as np
import ml_dtypes

import concourse.bacc as bacc
import concourse.tile as tile
from concourse import mybir
from concourse.bass_utils import run_bass_kernel_spmd

BF16 = ml_dtypes.bfloat16
FP8E5 = ml_dtypes.float8_e5m2
F16 = np.float16

N_NODES = 10000
D = 128
H = 4
C = 128
HC = H * C
NEG_SLOPE = 0.2
LN_EPS = 1e-5
L = 2
SGN_COMP = 1.09     # compensates the e5m2 truncation shrink of logits
EZ_CLAMP = 30000.0  # keeps exp() of stale pad-edge logits finite in fp16

N_CORES = 8
NODES_PER_CORE = 1280
BLOCKS = 10
BLK = 128
N_PAD = N_CORES * NODES_PER_CORE    # 10240
N_ROWS = 10112                      # 79*128
N_TILES = N_ROWS // 128

_NC_CACHE = {}
LAST_RESULTS = []   # BassKernelResults per launch (for test harness)

# engine round-robin patterns (tuned against the cost model)
EG_PAT = ["DVE", "POOL"]
SABS_PAT = ["ACT", "ACT", "ACT", "ACT", "ACT", "ACT", "DVE"]
GROUP = 8             # chunks per exp batch
OFF_EG = GROUP + 2    # eg stage offset (must trail the group exp)
OFF_AGG = OFF_EG + 2


def _prep_edges(edge_index):
    src = np.concatenate([np.asarray(edge_index[0], np.int64),
                          np.arange(N_NODES, dtype=np.int64)])
    dst = np.concatenate([np.asarray(edge_index[1], np.int64),
                          np.arange(N_NODES, dtype=np.int64)])
    pad_nodes = np.arange(N_NODES, N_PAD, dtype=np.int64)
    src = np.concatenate([src, np.zeros_like(pad_nodes)])
    dst = np.concatenate([dst, pad_nodes])

    order = np.argsort(dst, kind="stable")
    src = src[order]
    dst = dst[order]

    blk_of_edge = dst // BLK
    n_blocks_total = N_PAD // BLK
    counts = np.bincount(blk_of_edge, minlength=n_blocks_total)
    K = int(np.max((counts + BLK - 1) // BLK))
    K += K % 2  # even, so we can process chunk pairs

    cap = K * BLK
    src_arr = np.zeros((n_blocks_total, cap), np.int32)
    dpos_arr = np.full((n_blocks_total, cap), -1, np.int32)
    block_starts = np.zeros(n_blocks_total + 1, np.int64)
    np.cumsum(counts, out=block_starts[1:])
    slot = np.arange(len(dst)) - block_starts[blk_of_edge]
    src_arr[blk_of_edge, slot] = src.astype(np.int32)
    dpos_arr[blk_of_edge, slot] = (dst - blk_of_edge * BLK).astype(np.int32)

    return (K, src_arr.reshape(N_CORES, BLOCKS, cap),
            dpos_arr.reshape(N_CORES, BLOCKS, cap))


def _build_ship_arrays(K, src_arr, dpos_arr):
    cap = K * BLK
    # wrapped gather indices: idx i lives at [i % 16, i // 16]; the 16-row
    # pattern is tiled 8x along partitions (one copy per SWDGE Q7 core).
    # pad slots are -1: the DGE skips trailing negative indices.
    s = src_arr.reshape(N_CORES, BLOCKS, cap // 16, 16)
    s = np.swapaxes(s, 2, 3)                                  # [c,b,16,cap/16]
    sidx = np.tile(s, (1, 1, 8, 1)).astype(np.int16)          # [c,b,128,cap/16]

    # fp8e5 one-hots:
    # ohtid [c,b, d(128), (K+2)*128]: slot 0 = identity, slot 1+k =
    #   oht chunk k (col (1+k)*128+e -> 1 iff dst(chunk k, e) == d),
    #   slot K+1 = identity.  Identities at both ends let the DoubleRow
    #   matmul pair (g, xr@middle) with (id, oht) using positive AP steps.
    # ohflat [c,b, e(128), cap]: col k*128+d -> oh[e, k, d]
    ohtid = np.zeros((N_CORES, BLOCKS, BLK, (K + 2) * BLK), FP8E5)
    ohflat = np.zeros((N_CORES, BLOCKS, BLK, cap), FP8E5)
    cc, bb, ss = np.nonzero(dpos_arr >= 0)
    kk = (ss // BLK).astype(np.int64)
    ee = (ss % BLK).astype(np.int64)
    dd = dpos_arr[cc, bb, ss].astype(np.int64)
    ohtid[cc, bb, dd, (kk + 1) * BLK + ee] = 1
    ohflat[cc, bb, ee, kk * BLK + dd] = 1
    i = np.arange(BLK)
    ohtid[:, :, i, i] = 1
    ohtid[:, :, i, (K + 1) * BLK + i] = 1
    return (np.ascontiguousarray(sidx), np.ascontiguousarray(ohtid),
            np.ascontiguousarray(ohflat))


def _bcast(v, rows=128):
    v = np.asarray(v, np.float32)
    return np.ascontiguousarray(np.broadcast_to(v[None, :], (rows, v.shape[0])))


def _build_nc(K, bias_zero, ln_triv):
    nc = bacc.Bacc("TRN2", target_bir_lowering=False, debug=False,
                   num_devices=N_CORES)
    f32, bf16, i16 = mybir.dt.float32, mybir.dt.bfloat16, mybir.dt.int16
    f16 = mybir.dt.float16
    fp8e5 = mybir.dt.float8e5
    AF = mybir.ActivationFunctionType
    ALU = mybir.AluOpType
    PM = mybir.MatmulPerfMode
    X = mybir.AxisListType.X
    cap = K * BLK

    xT = nc.dram_tensor("xT", [128, N_ROWS], bf16, kind="ExternalInput")
    xlocT = nc.dram_tensor("xlocT", [128, NODES_PER_CORE], bf16,
                           kind="ExternalInput")
    xloc = nc.dram_tensor("xloc", [NODES_PER_CORE, 128], f32,
                          kind="ExternalInput")
    WlS = nc.dram_tensor("WlS", [128, HC], bf16, kind="ExternalInput")
    WrS = nc.dram_tensor("WrS", [128, HC], bf16, kind="ExternalInput")
    blB = nc.dram_tensor("blB", [128, HC], f32, kind="ExternalInput")
    brB = nc.dram_tensor("brB", [128, HC], f32, kind="ExternalInput")
    sgnT = nc.dram_tensor("sgnT", [128, H], bf16, kind="ExternalInput")
    WlQ = nc.dram_tensor("WlQ", [128, H], bf16, kind="ExternalInput")
    WrQ = nc.dram_tensor("WrQ", [128, H], bf16, kind="ExternalInput")
    invatt4B = nc.dram_tensor("invatt4B", [128, HC], f32, kind="ExternalInput")
    biasB = nc.dram_tensor("biasB", [128, 128], f32, kind="ExternalInput")
    lngB = nc.dram_tensor("lngB", [128, 128], f32, kind="ExternalInput")
    lnbB = nc.dram_tensor("lnbB", [128, 128], f32, kind="ExternalInput")
    ohtidd = nc.dram_tensor("ohtidd", [BLOCKS, BLK, (K + 2) * BLK], fp8e5,
                            kind="ExternalInput")
    ohd = nc.dram_tensor("ohd", [BLOCKS, BLK, cap], fp8e5,
                         kind="ExternalInput")
    sidxd = nc.dram_tensor("sidxd", [BLOCKS, 128, cap // 16], i16,
                           kind="ExternalInput")

    xnew = nc.dram_tensor("xnew", [NODES_PER_CORE, 128], f32,
                          kind="ExternalOutput")
    vsd = nc.dram_tensor("vsd", [BLOCKS, 128], f32, kind="ExternalOutput")

    with tile.TileContext(nc) as tc, ExitStack() as ctx:
        consts = ctx.enter_context(tc.tile_pool(name="consts", bufs=1))
        lhsp = ctx.enter_context(tc.tile_pool(name="lhs", bufs=3))
        densep = ctx.enter_context(tc.tile_pool(name="dense", bufs=4))
        g8p = ctx.enter_context(tc.tile_pool(name="g8", bufs=2))
        otp = ctx.enter_context(tc.tile_pool(name="ot", bufs=3))
        ohp = ctx.enter_context(tc.tile_pool(name="ohf", bufs=3))
        sxp = ctx.enter_context(tc.tile_pool(name="sx", bufs=3))
        sp = ctx.enter_context(tc.tile_pool(name="s", bufs=3))
        ezp = ctx.enter_context(tc.tile_pool(name="ez", bufs=3))
        egp = ctx.enter_context(tc.tile_pool(name="eg", bufs=4))
        lnp = ctx.enter_context(tc.tile_pool(name="ln", bufs=2))
        lgp = ctx.enter_context(tc.tile_pool(name="lg", bufs=4))
        outp = ctx.enter_context(tc.tile_pool(name="out", bufs=2))
        dramp = ctx.enter_context(tc.tile_pool(name="dram", bufs=1,
                                               space="DRAM"))
        pup = ctx.enter_context(tc.tile_pool(name="pu", bufs=2, space="PSUM"))
        pzp = ctx.enter_context(tc.tile_pool(name="pz", bufs=1, space="PSUM"))
        pdenp = ctx.enter_context(tc.tile_pool(name="pden", bufs=1,
                                               space="PSUM"))
        paggp = ctx.enter_context(tc.tile_pool(name="pagg", bufs=2,
                                               space="PSUM"))

        def load_const(src_ap, shape, dtype, name):
            t = consts.tile(shape, dtype, tag=name)
            nc.sync.dma_start(t[:], src_ap)
            return t

        wl_sb = load_const(WlS[:], [128, HC], bf16, "wl")
        wr_sb = load_const(WrS[:], [128, HC], bf16, "wr")
        sgn_sb = load_const(sgnT[:], [128, H], bf16, "sgn")
        wlq_sb = load_const(WlQ[:], [128, H], bf16, "wlq")
        wrq_sb = load_const(WrQ[:], [128, H], bf16, "wrq")
        invatt_sb = load_const(invatt4B[:], [128, HC], f32, "invatt")
        if not bias_zero:
            blB_sb = load_const(blB[:], [128, HC], f32, "blB")
            brB_sb = load_const(brB[:], [128, HC], f32, "brB")
            biasB_sb = load_const(biasB[:], [128, 128], f32, "biasB")
        if not ln_triv:
            lngB_sb = load_const(lngB[:], [128, 128], f32, "lngB")
            lnbB_sb = load_const(lnbB[:], [128, 128], f32, "lnbB")

        xl_dram = dramp.tile([N_ROWS, HC], f16)

        alphaP = consts.tile([128, 1], f32, tag="alphaP")
        nc.vector.memset(alphaP[:], NEG_SLOPE)
        epsP = consts.tile([128, 1], f32, tag="epsP")
        nc.vector.memset(epsP[:], LN_EPS)

        blk_loads = {}

        def prefetch_loads(b):
            six = sxp.tile([128, cap // 16], i16, tag="sidx")
            nc.sync.dma_start(six[:], sidxd[b])
            ot = otp.tile([128, K + 2, BLK], fp8e5, tag="ot")
            nc.sync.dma_start(
                ot[:], ohtidd[b].rearrange("p (k e) -> p k e", e=BLK))
            ohb = ohp.tile([128, cap], fp8e5, tag="oh")
            nc.sync.dma_start(ohb[:], ohd[b])
            blk_loads[b] = (six, ot, ohb)

        st = {"g8": {}, "agg": {}, "den": {}, "s": {}, "u": {},
              "ezf": {}, "ezb": {}, "eg": {}, "zp": {}, "p06": None}

        POS = K // 2   # xr'' lives at the middle slot of g8

        def prefetch_gather(b):
            six, ot, ohb = blk_loads[b]
            g8 = g8p.tile([128, K + 1, HC], f16, tag="g8")
            st["g8"][b] = g8
            # chunk k -> tile slot k (k < POS) or k+1 (k >= POS)
            # splits sized under the 1024-descriptor SWDGE FIFO carveout
            ranges = []
            for lo, hi in ((0, POS), (POS, K)):
                n_sp = -(-(hi - lo) * BLK // 1008)
                bnds = [lo + (hi - lo) * i // n_sp for i in range(n_sp + 1)]
                ranges += list(zip(bnds[:-1], bnds[1:]))
            for k0, k1 in ranges:
                s0 = k0 if k1 <= POS else k0 + 1
                n_idx = (k1 - k0) * BLK
                nc.gpsimd.dma_gather(
                    out_ap=g8[:, s0:s0 + (k1 - k0), :], in_ap=xl_dram[:],
                    idxs_ap=six[:, k0 * BLK // 16:k1 * BLK // 16],
                    num_idxs=n_idx, num_idxs_reg=n_idx, elem_size=HC,
                    single_packet=False)

        # ---- dense: xl'' for all nodes -> DRAM fp16 ----
        xT_sb = consts.tile([128, N_ROWS], bf16, tag="xT")
        for q in range(4):
            c0 = (N_ROWS // 4 // 128) * 128 * q
            c1 = N_ROWS if q == 3 else (N_ROWS // 4 // 128) * 128 * (q + 1)
            nc.sync.dma_start(xT_sb[:, c0:c1], xT[:, c0:c1])
        prefetch_loads(0)
        prefetch_loads(1)
        GB = 4
        for t0 in range(0, N_TILES, GB):
            n_sub = min(GB, N_TILES - t0)
            xs4 = densep.tile([128, GB, HC], f16, tag="xs4")
            for j0 in range(0, n_sub, 2):
                nj = min(2, n_sub - j0)
                ps4 = pup.tile([128, 2, HC], f32, tag="uT")
                for j in range(j0, j0 + nj):
                    t_i = t0 + j
                    nc.tensor.matmul(ps4[:, j - j0, :],
                                     xT_sb[:, t_i * 128:(t_i + 1) * 128],
                                     wl_sb[:], start=True, stop=True)
                if bias_zero:
                    if (t0 + j0) % 4 < 2:
                        nc.vector.tensor_scalar(
                            out=xs4[:, j0:j0 + nj, :], in0=ps4[:, :nj, :],
                            scalar1=1.0, scalar2=None, op0=ALU.mult)
                    else:
                        nc.scalar.activation(xs4[:, j0:j0 + nj, :],
                                             ps4[:, :nj, :], AF.Copy)
                else:
                    for j in range(j0, j0 + nj):
                        nc.vector.tensor_tensor(out=xs4[:, j, :HC],
                                                in0=ps4[:, j - j0, :],
                                                in1=blB_sb[:], op=ALU.add)
            q_eng = nc.scalar if (t0 // GB) % 2 == 0 else nc.sync
            q_eng.dma_start(
                xl_dram[t0 * 128:(t0 + n_sub) * 128, :].rearrange(
                    "(t p) c -> p t c", p=128),
                xs4[:, :n_sub, :])

        # ---- edge phase: flattened per-chunk software pipeline ----
        prefetch_gather(0)
        P = K // 2

        def blk_state(b):
            if b not in st["agg"]:
                g8 = st["g8"][b]
                # xr'' for this block -> fp16 slot 0
                lhs = lhsp.tile([128, 128], bf16, tag="lhs")
                nc.sync.dma_start(lhs[:], xlocT[:, b * 128:(b + 1) * 128])
                psr4 = pup.tile([128, 2, HC], f32, tag="uT")
                psr = psr4[:, 0, :]
                nc.tensor.matmul(psr, lhs[:], wr_sb[:], start=True,
                                 stop=True)
                if bias_zero:
                    nc.vector.tensor_scalar(out=g8[:, POS, :HC], in0=psr,
                                            scalar1=1.0, scalar2=None,
                                            op0=ALU.mult)
                else:
                    nc.vector.tensor_tensor(out=g8[:, POS, :HC], in0=psr,
                                            in1=brB_sb[:], op=ALU.add)
                agg_t = paggp.tile([128, HC], f32, tag="agg")
                den_t = pdenp.tile([128, 4], f32, tag="den")
                st["agg"][b] = agg_t
                st["den"][b] = den_t

        def front(b, k):
            if k == 0:
                blk_state(b)
            g8 = st["g8"][b]
            ge5 = g8[:].bitcast(fp8e5)     # [128, K+1, 2*HC]
            ot = blk_loads[b][1]
            m, half = divmod(k, 2)
            if half == 0:
                uT_t = pup.tile([128, 2, HC], f32, tag="uT")
                st["u"][(b, m)] = uT_t
            uT = st["u"][(b, m)]
            gslot = k if k < POS else k + 1
            for h in range(H):
                lo = 2 * h * 128 + 1
                out_ap = uT[:, half, h * 128:(h + 1) * 128]
                if k < POS:
                    # lhs halves (g, xr) pair with rhs halves (id, oht)
                    lhs_ap = ge5[:, gslot:POS + 1:POS - gslot, lo:lo + 255:2]
                    rhs_ap = ot[:, 0:k + 2:k + 1, :]
                else:
                    # lhs halves (xr, g) pair with rhs halves (oht, id)
                    lhs_ap = ge5[:, POS:gslot + 1:gslot - POS, lo:lo + 255:2]
                    rhs_ap = ot[:, k + 1:K + 2:K - k, :]
                nc.tensor.matmul(out_ap, lhs_ap, rhs_ap, start=True,
                                 stop=True, perf_mode=PM.DoubleRow)
            if k == 16 and b + 2 < BLOCKS:
                prefetch_loads(b + 2)
            if k == 14 and b + 1 < BLOCKS:
                prefetch_gather(b + 1)

        def sabs(b, m):
            uT = st["u"].pop((b, m))
            s_ = sp.tile([128, 2, HC], bf16, tag="s")
            nc.scalar.activation(s_[:], uT[:], AF.Prelu, alpha=alphaP[:])
            st["s"][(b, m)] = s_

        def zmm(b, k):
            g, slot = divmod(k, GROUP)
            if slot == 0:
                zP_t = pzp.tile([128, 4 * GROUP], f32, tag="zP")
                st["zp"][b] = zP_t
            zP = st["zp"][b]
            m, half = divmod(k, 2)
            s_ = st["s"][(b, m)]
            for h in range(H):
                nc.tensor.matmul(zP[:, slot * 4 + h:slot * 4 + h + 1],
                                 s_[:, half, h * 128:(h + 1) * 128],
                                 sgn_sb[:, h:h + 1],
                                 start=True, stop=True)
            if half == 1:
                st["s"].pop((b, m))
            if slot == GROUP - 1 or k == K - 1:
                n4 = (slot + 1) * 4
                ezf = ezp.tile([128, 4 * GROUP], f32, tag="ezf")
                nc.scalar.activation(ezf[:, :n4], zP[:, :n4], AF.Exp)
                ezb = ezp.tile([128, 4 * GROUP], f16, tag="ezb")
                nc.vector.tensor_scalar(out=ezb[:, :n4], in0=ezf[:, :n4],
                                        scalar1=EZ_CLAMP, scalar2=None,
                                        op0=ALU.min)
                st["ezf"][(b, g)] = ezf
                st["ezb"][(b, g)] = ezb

        def eg_stage(b, m):
            # one 4D-AP multiply per chunk-pair (slots are always adjacent
            # because POS is even)
            g8 = st["g8"][b]
            egt_t = egp.tile([128, 2, HC], bf16, tag="eg")
            st["eg"][(b, m)] = egt_t
            k = 2 * m
            grp, slot = divmod(k, GROUP)
            ezb = st["ezb"][(b, grp)]
            gslot = k if k < POS else k + 1
            in0 = g8[:, gslot:gslot + 2, :HC].rearrange(
                "p j (h c) -> p j h c", h=H)
            in1 = ezb[:, slot * 4:slot * 4 + 8].rearrange(
                "p (j h) -> p j h", j=2).unsqueeze(-1).broadcast_to(
                [128, 2, H, 128])
            out = egt_t[:].rearrange("p j (h c) -> p j h c", h=H)
            eng = EG_PAT[m % len(EG_PAT)]
            if eng == "DVE":
                nc.vector.tensor_tensor(out=out, in0=in0, in1=in1,
                                        op=ALU.mult)
            else:
                nc.gpsimd.tensor_tensor(out=out, in0=in0, in1=in1,
                                        op=ALU.mult)

        def aggden(b, m):
            agg = st["agg"][b]
            den = st["den"][b]
            ohb = blk_loads[b][2]
            egt = st["eg"].pop((b, m))
            for half in (0, 1):
                k = 2 * m + half
                grp, slot = divmod(k, GROUP)
                ezb = st["ezb"][(b, grp)]
                nc.tensor.matmul(agg[:], ohb[:, k * BLK:(k + 1) * BLK],
                                 egt[:, half, :],
                                 start=(k == 0), stop=(k == K - 1))
                nc.tensor.matmul(den[:], ohb[:, k * BLK:(k + 1) * BLK],
                                 ezb[:, slot * 4:slot * 4 + 4],
                                 start=(k == 0), stop=(k == K - 1))
            if m == P - 1:
                tail(b)

        def tail(b):
            agg = st["agg"].pop(b)
            den = st["den"].pop(b)
            st["g8"].pop(b, None)
            st["zp"].pop(b, None)
            blk_loads.pop(b, None)
            for key in [x for x in st["ezf"] if x[0] == b]:
                st["ezf"].pop(key)
            for key in [x for x in st["ezb"] if x[0] == b]:
                st["ezb"].pop(key)
            rden = lgp.tile([128, 4], f32, tag="rden")
            nc.vector.reciprocal(rden[:], den[:])
            # tq = agg * rden (head-broadcast) on DVE
            tq = lnp.tile([128, HC], f32, tag="tq")
            nc.vector.tensor_tensor(
                out=tq[:].rearrange("p (h c) -> p h c", h=H),
                in0=agg[:].rearrange("p (h c) -> p h c", h=H),
                in1=rden[:].unsqueeze(-1).broadcast_to([128, H, 128]),
                op=ALU.mult)
            tq2 = lnp.tile([128, HC], f32, tag="tq2")
            nc.gpsimd.tensor_tensor(out=tq2[:], in0=tq[:], in1=invatt_sb[:],
                                    op=ALU.mult)
            hm = outp.tile([128, 128], f32, tag="hm")
            nc.vector.tensor_reduce(
                out=hm[:], in_=tq2[:].rearrange("p (h c) -> p c h", h=H),
                axis=X, op=ALU.add)
            xt = outp.tile([128, 128], f32, tag="xres")
            nc.sync.dma_start(xt[:], xloc[b * 128:(b + 1) * 128, :])
            if bias_zero:
                r2 = outp.tile([128, 128], f32, tag="r2")
                nc.gpsimd.tensor_tensor(out=r2[:], in0=hm[:], in1=xt[:],
                                        op=ALU.add)
            else:
                r1 = outp.tile([128, 128], f32, tag="r1")
                nc.vector.tensor_tensor(out=r1[:], in0=hm[:],
                                        in1=biasB_sb[:], op=ALU.add)
                r2 = outp.tile([128, 128], f32, tag="r2")
                nc.gpsimd.tensor_tensor(out=r2[:], in0=r1[:], in1=xt[:],
                                        op=ALU.add)
            mu = lgp.tile([128, 1], f32, tag="mu")
            nc.vector.tensor_reduce(out=mu[:], in_=r2[:], axis=X, op=ALU.add)
            mun = lgp.tile([128, 1], f32, tag="mun")
            nc.vector.tensor_scalar_mul(mun[:], mu[:], 1.0 / 128)
            xc = outp.tile([128, 128], f32, tag="xc")
            nc.vector.tensor_scalar(out=xc[:], in0=r2[:], scalar1=mun[:],
                                    scalar2=None, op0=ALU.subtract)
            junk = outp.tile([128, 128], f32, tag="junk")
            vs = lgp.tile([128, 1], f32, tag="vs")
            nc.vector.scalar_tensor_tensor(
                out=junk[:], in0=r2[:], scalar=mun[:], in1=xc[:],
                op0=ALU.subtract, op1=ALU.mult, accum_out=vs[:])
            nc.sync.dma_start(vsd[b], vs[:, 0])
            xout = outp.tile([128, 128], f32, tag="xout")
            if ln_triv:
                # relu only; the 1/sqrt(var+eps) row scale is applied on
                # the host (relu commutes with a positive per-row scale)
                nc.scalar.activation(xout[:], xc[:], AF.Relu)
            else:
                lt = lgp.tile([128, 1], f32, tag="lt")
                nc.scalar.activation(lt[:], vs[:], AF.Ln, bias=epsP[:],
                                     scale=1.0 / 128)
                rstd = lgp.tile([128, 1], f32, tag="rstd")
                nc.scalar.activation(rstd[:], lt[:], AF.Exp, scale=-0.5)
                xn = outp.tile([128, 128], f32, tag="xn")
                nc.vector.tensor_scalar(out=xn[:], in0=xc[:],
                                        scalar1=rstd[:],
                                        scalar2=None, op0=ALU.mult)
                xg = outp.tile([128, 128], f32, tag="xg")
                nc.vector.tensor_tensor(out=xg[:], in0=xn[:], in1=lngB_sb[:],
                                        op=ALU.mult)
                xgb = outp.tile([128, 128], f32, tag="xgb")
                nc.vector.tensor_tensor(out=xgb[:], in0=xg[:],
                                        in1=lnbB_sb[:], op=ALU.add)
                nc.scalar.activation(xout[:], xgb[:], AF.Relu)
            nc.sync.dma_start(xnew[b * 128:(b + 1) * 128, :], xout[:])

        chunks = [(b, k) for b in range(BLOCKS) for k in range(K)]
        NCH = len(chunks)
        for i in range(NCH + OFF_AGG + 1):
            if i < NCH:
                front(*chunks[i])
            if 0 <= i - 1 < NCH and chunks[i - 1][1] % 2 == 1:
                b, k = chunks[i - 1]
                sabs(b, k // 2)
            if 0 <= i - 2 < NCH:
                zmm(*chunks[i - 2])
            if 0 <= i - OFF_EG < NCH and chunks[i - OFF_EG][1] % 2 == 1:
                b, k = chunks[i - OFF_EG]
                eg_stage(b, k // 2)
            if 0 <= i - OFF_AGG < NCH and chunks[i - OFF_AGG][1] % 2 == 0:
                b, k = chunks[i - OFF_AGG]
                aggden(b, k // 2)

    nc.compile()
    return nc


def kernel(x, edge_index, Wl, bl, Wr, br, att, bias, ln_g, ln_b):
    x = np.asarray(x, np.float32)
    edge_index = np.asarray(edge_index)
    Wl = np.asarray(Wl, np.float32); bl = np.asarray(bl, np.float32)
    Wr = np.asarray(Wr, np.float32); br = np.asarray(br, np.float32)
    att = np.asarray(att, np.float32); bias = np.asarray(bias, np.float32)
    ln_g = np.asarray(ln_g, np.float32); ln_b = np.asarray(ln_b, np.float32)

    K, src_arr, dpos_arr = _prep_edges(edge_index)
    sidx, ohtid, ohflat = _build_ship_arrays(K, src_arr, dpos_arr)

    bias_zero = not (np.any(bias) or np.any(bl) or np.any(br))
    ln_triv = (np.all(ln_g == 1.0) and not np.any(ln_b))
    key = (K, bias_zero, ln_triv)
    if key not in _NC_CACHE:
        _NC_CACHE[key] = _build_nc(K, bias_zero, ln_triv)
    nc = _NC_CACHE[key]

    aatt = np.maximum(np.abs(att), 1e-30)
    sgn = np.sign(att).astype(np.float32)
    sgn[sgn == 0] = 1.0

    LAST_RESULTS.clear()
    cur = x
    for l in range(L):
        a_flat = aatt[l].reshape(HC)
        WlSf = Wl[l] * a_flat[None, :]
        WrSf = Wr[l] * a_flat[None, :]
        WlS = WlSf.astype(BF16)
        WrS = WrSf.astype(BF16)
        sgnT = np.ascontiguousarray(
            (sgn[l] * SGN_COMP).T).astype(BF16)   # [C, H]
        WlQ = np.zeros((D, H), np.float32)
        WrQ = np.zeros((D, H), np.float32)

        xpad = np.zeros((N_ROWS, 128), np.float32)
        xpad[:N_NODES] = cur
        xT = np.ascontiguousarray(xpad.T.astype(BF16))
        xloc_full = np.zeros((N_PAD, 128), np.float32)
        xloc_full[:N_NODES] = cur

        common = {
            "xT": xT, "WlS": WlS, "WrS": WrS,
            "blB": _bcast(bl[l] * a_flat), "brB": _bcast(br[l] * a_flat),
            "sgnT": sgnT, "WlQ": WlQ.astype(BF16), "WrQ": WrQ.astype(BF16),
            "invatt4B": _bcast(0.25 / a_flat),
            "biasB": _bcast(bias[l]), "lngB": _bcast(ln_g[l]),
            "lnbB": _bcast(ln_b[l]),
        }
        in_maps = []
        for c in range(N_CORES):
            xl_c = np.ascontiguousarray(
                xloc_full[c * NODES_PER_CORE:(c + 1) * NODES_PER_CORE])
            in_maps.append({
                **common,
                "xloc": xl_c,
                "xlocT": np.ascontiguousarray(xl_c.T.astype(BF16)),
                "ohtidd": ohtid[c], "ohd": ohflat[c], "sidxd": sidx[c],
            })

        res = run_bass_kernel_spmd(nc, in_maps, core_ids=list(range(N_CORES)))
        LAST_RESULTS.append(res)
        nxt = np.concatenate([res.results[c]["xnew"] for c in range(N_CORES)],
                             axis=0)
        if ln_triv:
            vsall = np.concatenate(
                [np.asarray(res.results[c]["vsd"]).reshape(-1)
                 for c in range(N_CORES)])
            rstd = 1.0 / np.sqrt(vsall / 128.0 + LN_EPS)
            nxt = nxt * rstd[:, None]
        cur = np.ascontiguousarray(nxt[:N_NODES]).astype(np.float32)

    return cur.astype(np.float32)
